# revision 1
# baseline (speedup 1.0000x reference)
"""Trainium2 Bass kernel for one dense transformer block.

Full (unsharded) IO: x [4, 2048, 1024] -> out [4, 2048, 1024].
Sharding: 8 cores = 4 batches x 2 query-chunk-pair sets. Each core owns one
batch's K/V (2048 rows) and 1024 query rows chosen as causally-balanced
128-row chunk pairs (set A: chunks {4j, 4j+3}, set B: {4j+1, 4j+2}), so every
core runs an identical instruction stream; only data (incl. the causal mask)
differs. No collectives.

On-core dataflow is feature-on-partition ("transposed") throughout:
  LN1 -> hT -> {Q,K}T per head pair -> scores S.T[keys, q] -> exp -> AV with
  an appended ones-column for the softmax denominator -> O.T -> proj ->
  residual -> LN2 -> FFN (streamed W1/W2) -> residual -> transposed DMA out.
Matmuls run in float32r (full PE rate); softmax skips max-subtraction (scores
for this block are O(10); masked lanes get -30000 so exp underflows to 0).
LayerNorm gamma/beta are folded into the weights/biases host-side.
"""

import sys

sys.path.insert(0, "/opt/trn_rl_repo")

import numpy as np

import concourse.bass as bass
import concourse.mybir as mybir
import concourse.tile as tile
from concourse.bass_utils import run_bass_kernel_spmd

f32 = mybir.dt.float32
f32r = mybir.dt.float32r
AL = mybir.AluOpType
AF = mybir.ActivationFunctionType

B, T, C = 4, 2048, 1024
H, D = 16, 64
F = 4 * C
P = 128
TQ = 1024            # query rows per core
NCHUNK = T // P      # 16 chunks of 128 per batch
NEG = -30000.0
LN_EPS = 1e-5


def _split_sync_waits(nc):
    """This container's walrus supports one sync-wait per instruction; Tile
    emits up to ~3. Hoist extras onto NoOps inserted before the owner."""
    ctr = 0
    for fn in nc.m.functions:
        for bb in fn.blocks:
            out, changed = [], False
            for ins in bb.instructions:
                si = ins.sync_info
                waits = list(si.on_wait) if si is not None and si.on_wait else []
                if len(waits) > 1:
                    changed = True
                    for w in waits[:-1]:
                        ctr += 1
                        nop = mybir.InstNoOp(name=f"waitsplit_{ctr}", ins=[], outs=[])
                        nop.engine = ins.engine
                        nop.sync_info = mybir.SyncInfo(on_wait=[w], on_update=[])
                        out.append(nop)
                        nc.register_instruction(nop, overwrite=True)
                    ins.sync_info = mybir.SyncInfo(
                        on_wait=[waits[-1]], on_update=list(si.on_update or [])
                    )
                out.append(ins)
            if changed:
                bb.instructions = out


def _chunk_pairs(s):
    # set A (s=0): (4j, 4j+3); set B (s=1): (4j+1, 4j+2) -- both need
    # key tiles [0, 4j+4) for 256-row local chunk j.
    if s == 0:
        return [(4 * j, 4 * j + 3) for j in range(4)]
    return [(4 * j + 1, 4 * j + 2) for j in range(4)]


def _emit_ln(nc, pools, src_fn, dst, n_rc, ones, eps_sb):
    """LayerNorm over features in transposed layout.

    src_fn(ft, rc) -> [128, 512] f32r AP of input features ft*128.. for row
    chunk rc. dst: [128, 8, n_rc*512] f32r tile receiving (x-mu)*rstd.
    """
    sb, small, ps_s, ps_b = pools
    for rc in range(n_rc):
        # pass 1: stats (x and x^2 tiles are transient)
        psum_m = ps_s.tile([1, 512], f32, tag="ln_m")
        psum_q = ps_s.tile([1, 512], f32, tag="ln_q")
        for ft in range(8):
            xt = src_fn(ft, rc)
            sq = sb.tile([P, 512], f32r, tag="ln_sq")
            nc.vector.tensor_tensor(sq, xt, xt, AL.mult)
            nc.tensor.matmul(psum_m, ones, xt, start=(ft == 0), stop=(ft == 7))
            nc.tensor.matmul(psum_q, ones, sq, start=(ft == 0), stop=(ft == 7))
        mean = small.tile([1, 512], f32, tag="ln_mean")
        nc.vector.tensor_scalar_mul(mean, psum_m, 1.0 / C)
        msq = small.tile([1, 512], f32, tag="ln_msq")
        nc.vector.tensor_scalar_mul(msq, psum_q, 1.0 / C)
        var = small.tile([1, 512], f32, tag="ln_var")
        nc.vector.tensor_tensor(var, mean, mean, AL.mult)
        nc.vector.tensor_tensor(var, msq, var, AL.subtract)
        std = small.tile([1, 512], f32, tag="ln_std")
        nc.scalar.activation(std, var, AF.Sqrt, bias=eps_sb[:, :], scale=1.0)
        a = small.tile([1, 512], f32r, tag="ln_a")
        with nc.allow_low_precision(reason="f32r has f32 bits"):
            nc.vector.reciprocal(a, std)
        bneg = small.tile([1, 512], f32r, tag="ln_b")
        nc.vector.tensor_tensor(bneg, mean, a, AL.mult)
        psum_abc = ps_b.tile([P, 512], f32, tag="ln_abc")
        nc.tensor.matmul(psum_abc, ones[0:1, 0:1].broadcast_to((1, P)), a,
                         start=True, stop=True)
        psum_bbc = ps_b.tile([P, 512], f32, tag="ln_bbc")
        nc.tensor.matmul(psum_bbc, ones[0:1, 0:1].broadcast_to((1, P)), bneg,
                         start=True, stop=True)
        # pass 2: normalize (re-fetch source)
        for ft in range(8):
            xt = src_fn(ft, rc)
            tmp = sb.tile([P, 512], f32, tag="ln_tmp")
            nc.vector.tensor_tensor(tmp, xt, psum_abc, AL.mult)
            nc.vector.tensor_tensor(
                dst[:, ft, rc * 512:(rc + 1) * 512], tmp, psum_bbc, AL.subtract
            )


def build_program(phases=("ln", "attn", "proj", "ffn")):
    nc = bass.Bass()
    xq_d = nc.dram_tensor("xq", [TQ, C], f32r, kind="ExternalInput")
    xkv_d = nc.dram_tensor("xkv", [T, C], f32r, kind="ExternalInput")
    mask_d = nc.dram_tensor("maskc", [4, 4, P, 256], f32, kind="ExternalInput")
    wq_d = nc.dram_tensor("wq", [8, P, 8, P], f32r, kind="ExternalInput")
    wk_d = nc.dram_tensor("wk", [8, P, 8, P], f32r, kind="ExternalInput")
    wv_d = nc.dram_tensor("wv", [8, P, 8, P], f32r, kind="ExternalInput")
    wp_d = nc.dram_tensor("wp", [C, C], f32r, kind="ExternalInput")
    w1_d = nc.dram_tensor("w1", [32, P, 8, P], f32r, kind="ExternalInput")
    w2_d = nc.dram_tensor("w2", [8, P, 32, P], f32r, kind="ExternalInput")
    bq_d = nc.dram_tensor("bq", [C], f32, kind="ExternalInput")
    bk_d = nc.dram_tensor("bk", [C], f32, kind="ExternalInput")
    bv_d = nc.dram_tensor("bv", [C], f32, kind="ExternalInput")
    bp_d = nc.dram_tensor("bp", [C], f32, kind="ExternalInput")
    b1_d = nc.dram_tensor("b1", [F], f32, kind="ExternalInput")
    b2_d = nc.dram_tensor("b2", [C], f32, kind="ExternalInput")
    ones_d = nc.dram_tensor("onesc", [P, 1], f32r, kind="ExternalInput")
    ident_d = nc.dram_tensor("identc", [P, P], f32r, kind="ExternalInput")
    y_d = nc.dram_tensor("y", [C, TQ], f32, kind="ExternalOutput")
    x2s_d = nc.dram_tensor("x2scratch", [C, TQ], f32r)
    xqs_d = nc.dram_tensor("xqscratch", [C, TQ], f32r)

    wp_r = wp_d.rearrange("(ko p) o -> p ko o", p=P)

    with tile.TileContext(nc) as tc:
        with tc.tile_pool(name="consts", bufs=1) as cpool, \
             tc.tile_pool(name="persist", bufs=1) as pers:
            ones = cpool.tile([P, 1], f32r)
            nc.sync.dma_start(ones, ones_d[:, :])
            ident = cpool.tile([P, P], f32r)
            nc.sync.dma_start(ident, ident_d[:, :])
            eps_sb = cpool.tile([1, 1], f32)
            nc.vector.memset(eps_sb, LN_EPS)
            bq_sb = cpool.tile([P, 8], f32)
            nc.sync.dma_start(bq_sb, bq_d.rearrange("(o p) -> p o", p=P))
            bk_sb = cpool.tile([P, 8], f32)
            nc.sync.dma_start(bk_sb, bk_d.rearrange("(o p) -> p o", p=P))
            bv_sb = cpool.tile([P, 8], f32)
            nc.sync.dma_start(bv_sb, bv_d.rearrange("(o p) -> p o", p=P))
            bp_sb = cpool.tile([P, 8], f32)
            nc.sync.dma_start(bp_sb, bp_d.rearrange("(o p) -> p o", p=P))
            b1_sb = cpool.tile([P, 32], f32)
            nc.sync.dma_start(b1_sb, b1_d.rearrange("(o p) -> p o", p=P))
            b2_sb = cpool.tile([P, 8], f32)
            nc.sync.dma_start(b2_sb, b2_d.rearrange("(o p) -> p o", p=P))

            OT = pers.tile([P, 8, TQ], f32r)       # attn out, transposed

            # ---------------- Phase 0 + A: LN1 and attention ----------------
            with tc.tile_pool(name="attn_sb", bufs=1) as apool:
                hkvT = apool.tile([P, 8, T], f32r)
                hqT = apool.tile([P, 8, TQ], f32r)
                with tc.tile_pool(name="ln_sb", bufs=4) as lnsb, \
                     tc.tile_pool(name="ln_small", bufs=4) as lnsmall, \
                     tc.tile_pool(name="ln_ps", bufs=4, space="PSUM") as lnps:
                    eps128 = lnsmall.tile([P, 1], f32, tag="eps128")
                    nc.vector.memset(eps128, LN_EPS)

                    def ln_row_tile(src_ap, dstT, rt, transpose_raw=None):
                        """Load one 128-row tile row-major, LN it, PE-transpose
                        into dstT[:, ft, rt*128...]. Optionally also transpose
                        the raw rows into transpose_raw slices."""
                        xrow = lnsb.tile([P, C], f32r, tag="xrow")
                        nc.sync.dma_start(xrow, src_ap)
                        stats = lnsmall.tile([P, 2, 6], f32, tag="stats")
                        for sg in range(2):
                            nc.vector.bn_stats(stats[:, sg, :], xrow[:, sg * 512:(sg + 1) * 512])
                        mv = lnsmall.tile([P, 2], f32, tag="mv")
                        nc.vector.bn_aggr(mv, stats)
                        rstd = lnsmall.tile([P, 1], f32, tag="rstd")
                        nc.scalar.activation(rstd, mv[:, 1:2], AF.Sqrt,
                                             bias=eps128, scale=1.0)
                        nc.vector.reciprocal(rstd, rstd)
                        hrow = lnsb.tile([P, C], f32r, tag="hrow")
                        nc.vector.tensor_scalar(hrow, xrow, mv[:, 0:1], rstd,
                                                op0=AL.subtract, op1=AL.mult)
                        for ft in range(8):
                            psum_t = lnps.tile([P, P], f32r, tag="tr")
                            nc.tensor.matmul(psum_t, hrow[:, ft * P:(ft + 1) * P],
                                             ident, is_transpose=True,
                                             start=True, stop=True)
                            nc.vector.tensor_copy(
                                dstT[:, ft, rt * P:(rt + 1) * P], psum_t)
                        if transpose_raw is not None:
                            for ft in range(8):
                                psum_t = lnps.tile([P, P], f32r, tag="tr")
                                nc.tensor.matmul(psum_t, xrow[:, ft * P:(ft + 1) * P],
                                                 ident, is_transpose=True,
                                                 start=True, stop=True)
                                xqt = lnsb.tile([P, P], f32r, tag="xqt")
                                nc.vector.tensor_copy(xqt, psum_t)
                                nc.sync.dma_start(
                                    xqs_d[ft * P:(ft + 1) * P,
                                          rt * P:(rt + 1) * P], xqt)

                    if "ln" in phases:
                        for rt in range(16):
                            ln_row_tile(xkv_d[rt * P:(rt + 1) * P, :], hkvT, rt)
                        for rt in range(8):
                            ln_row_tile(xq_d[rt * P:(rt + 1) * P, :], hqT, rt,
                                        transpose_raw=True)

                with tc.tile_pool(name="maskp", bufs=1) as maskp, \
                     tc.tile_pool(name="pair_w", bufs=2) as wpool, \
                     tc.tile_pool(name="pair_big", bufs=1) as gpool, \
                     tc.tile_pool(name="pt_sb", bufs=3) as ptpool, \
                     tc.tile_pool(name="o_sb", bufs=2) as opool, \
                     tc.tile_pool(name="ps_kqv", bufs=2, space="PSUM") as ps_kqv, \
                     tc.tile_pool(name="ps_score", bufs=3, space="PSUM") as ps_sc, \
                       tc.tile_pool(name="ps_bcast", bufs=1, space="PSUM") as ps_bc, \
                     tc.tile_pool(name="ps_o", bufs=2, space="PSUM") as ps_o:
                    mask_sb = maskp.tile([P, 4, 4, 256], f32)
                    for j in range(4):
                        for t in range(4):
                            nc.sync.dma_start(mask_sb[:, j, t, :], mask_d[j, t])
                    for g in (range(8) if "attn" in phases else []):
                        wk_t = wpool.tile([P, 8, P], f32r, tag="wk")
                        nc.sync.dma_start(wk_t, wk_d[g])
                        wq_t = wpool.tile([P, 8, P], f32r, tag="wqt")
                        nc.sync.dma_start(wq_t, wq_d[g])
                        wv_t = wpool.tile([P, 8, P], f32r, tag="wv")
                        nc.sync.dma_start(wv_t, wv_d[g])

                        KT = gpool.tile([P, T], f32r, tag="KT")
                        for rc in range(4):
                            psum = ps_kqv.tile([P, 512], f32, tag="kqv")
                            for k in range(8):
                                nc.tensor.matmul(
                                    psum, wk_t[:, k, :],
                                    hkvT[:, k, rc * 512:(rc + 1) * 512],
                                    start=(k == 0), stop=(k == 7))
                            nc.vector.tensor_scalar_add(
                                KT[:, rc * 512:(rc + 1) * 512], psum,
                                bk_sb[:, g:g + 1])
                        QT = gpool.tile([P, TQ], f32r, tag="QT")
                        for rc in range(2):
                            psum = ps_kqv.tile([P, 512], f32, tag="kqv")
                            for k in range(8):
                                nc.tensor.matmul(
                                    psum, wq_t[:, k, :],
                                    hqT[:, k, rc * 512:(rc + 1) * 512],
                                    start=(k == 0), stop=(k == 7))
                            nc.vector.tensor_scalar_add(
                                QT[:, rc * 512:(rc + 1) * 512], psum,
                                bq_sb[:, g:g + 1])
                        VT = gpool.tile([P, 4, 512], f32r, tag="VT")
                        for rc in range(4):
                            psum = ps_kqv.tile([P, 512], f32, tag="kqv")
                            for k in range(8):
                                nc.tensor.matmul(
                                    psum, wv_t[:, k, :],
                                    hkvT[:, k, rc * 512:(rc + 1) * 512],
                                    start=(k == 0), stop=(k == 7))
                            nc.vector.tensor_scalar_add(
                                VT[:, rc, :], psum, bv_sb[:, g:g + 1])
                        # V row-major (+ ones col per head) via PE transpose
                        vaug = gpool.tile([P, 16, 130], f32r, tag="vaug")
                        for kt in range(16):
                            psum_t = ps_kqv.tile([P, P], f32r, tag="kqv")
                            nc.tensor.matmul(
                                psum_t, VT[:, kt // 4, (kt % 4) * P:(kt % 4 + 1) * P],
                                ident, is_transpose=True, start=True, stop=True)
                            nc.vector.tensor_copy(vaug[:, kt, 0:64], psum_t[:, 0:64])
                            nc.vector.tensor_copy(vaug[:, kt, 65:129], psum_t[:, 64:128])
                            nc.vector.tensor_copy(vaug[:, kt, 64:65], ones[:, :])
                            nc.vector.tensor_copy(vaug[:, kt, 129:130], ones[:, :])

                        for hh in range(2):
                            base = 64 * hh
                            for j in range(4):
                                nkt = 4 * j + 4
                                psum_o = ps_o.tile([65, 256], f32, tag="po")
                                for kt in range(nkt):
                                    psum_s = ps_sc.tile([P, 256], f32, tag="sc")
                                    nc.tensor.matmul(
                                        psum_s,
                                        KT[base:base + 64, kt * P:(kt + 1) * P],
                                        QT[base:base + 64, j * 256:(j + 1) * 256],
                                        start=True, stop=True)
                                    pt = ptpool.tile([P, 256], f32r, tag="pt")
                                    if kt >= 4 * j:
                                        ssb = ptpool.tile([P, 256], f32, tag="ssb")
                                        nc.vector.scalar_tensor_tensor(
                                            ssb, psum_s, 1.0,
                                            mask_sb[:, j, kt - 4 * j, :],
                                            op0=AL.bypass, op1=AL.add)
                                        nc.scalar.activation(pt, ssb, AF.Exp)
                                    else:
                                        nc.scalar.activation(pt, psum_s, AF.Exp)
                                    nc.tensor.matmul(
                                        psum_o, vaug[:, kt, 65 * hh:65 * hh + 65],
                                        pt, start=(kt == 0), stop=(kt == nkt - 1))
                                o_sb = opool.tile([65, 256], f32, tag="osb")
                                nc.vector.tensor_copy(o_sb, psum_o)
                                rec = opool.tile([1, 256], f32r, tag="rec")
                                with nc.allow_low_precision(reason="f32r bits"):
                                    nc.vector.reciprocal(rec, o_sb[64:65, :])
                                psum_bc = ps_bc.tile([64, 256], f32, tag="bc")
                                nc.tensor.matmul(
                                    psum_bc, ones[0:1, 0:1].broadcast_to((1, 64)),
                                    rec, start=True, stop=True)
                                nc.vector.tensor_tensor(
                                    OT[base:base + 64, g, j * 256:(j + 1) * 256],
                                    o_sb[0:64, :], psum_bc, AL.mult)

            # ---------------- Phase B: proj + residual + LN2 ----------------
            with tc.tile_pool(name="late", bufs=1) as late:
              h2T = late.tile([P, 8, TQ], f32r)    # LN2 output (reuses attn space)
              with tc.tile_pool(name="proj_sb", bufs=1) as prpool, \
                 tc.tile_pool(name="proj_tmp", bufs=3) as prtmp, \
                 tc.tile_pool(name="ps_proj", bufs=2, space="PSUM") as ps_pr:
                wp_t = prpool.tile([P, 8, C], f32r)
                nc.sync.dma_start(wp_t, wp_r)
                xqT = prpool.tile([P, 8, TQ], f32r)
                for ft in range(8):
                    nc.sync.dma_start(xqT[:, ft, :], xqs_d[ft * P:(ft + 1) * P, :])
                for of in (range(8) if "proj" in phases else []):
                    for rc in range(2):
                        psum = ps_pr.tile([P, 512], f32, tag="pr")
                        for k in range(8):
                            nc.tensor.matmul(
                                psum, wp_t[:, k, of * P:(of + 1) * P],
                                OT[:, k, rc * 512:(rc + 1) * 512],
                                start=(k == 0), stop=(k == 7))
                        x2sb = prtmp.tile([P, 512], f32r, tag="x2sb")
                        nc.vector.scalar_tensor_tensor(
                            x2sb, psum, bp_sb[:, of:of + 1],
                            xqT[:, of, rc * 512:(rc + 1) * 512],
                            op0=AL.add, op1=AL.add)
                        nc.sync.dma_start(
                            x2s_d[of * P:(of + 1) * P,
                                  rc * 512:(rc + 1) * 512], x2sb)
                with tc.tile_pool(name="ln2_sb", bufs=2) as lnsb2, \
                     tc.tile_pool(name="ln2_small", bufs=1) as lnsmall2, \
                     tc.tile_pool(name="ln2_pss", bufs=1, space="PSUM") as lnpss2, \
                     tc.tile_pool(name="ln2_psb", bufs=2, space="PSUM") as lnpsb2:
                    def src_x2(ft, rc, _p=lnsb2):
                        xt = _p.tile([P, 512], f32r, tag="ln_x")
                        nc.sync.dma_start(
                            xt, x2s_d[ft * P:(ft + 1) * P,
                                      rc * 512:(rc + 1) * 512])
                        return xt
                    _emit_ln(nc, (lnsb2, lnsmall2, lnpss2, lnpsb2),
                             src_x2, h2T, 2, ones, eps_sb)

              # ---------------- Phase C: FFN + residual + store ---------------
              with tc.tile_pool(name="w2_sb", bufs=3) as w2pool, \
                 tc.tile_pool(name="ffn_sb", bufs=2) as fpool, \
                   tc.tile_pool(name="relu_sb", bufs=1) as rpool, \
                   tc.tile_pool(name="y_sb", bufs=3) as ypool, \
                   tc.tile_pool(name="ps_f1", bufs=2, space="PSUM") as ps_f1, \
                   tc.tile_pool(name="ps_f2", bufs=2, space="PSUM") as ps_f2:
                  for rc in (range(2) if "ffn" in phases else []):
                      relu1T = rpool.tile([P, 32, 512], f32r, tag="relu")
                      for fk in range(32):
                          w1_t = fpool.tile([P, 8, P], f32r, tag="w1")
                          nc.sync.dma_start(w1_t, w1_d[fk])
                          psum = ps_f1.tile([P, 512], f32, tag="f1")
                          for k in range(8):
                              nc.tensor.matmul(
                                  psum, w1_t[:, k, :],
                                  h2T[:, k, rc * 512:(rc + 1) * 512],
                                  start=(k == 0), stop=(k == 7))
                          nc.scalar.activation(relu1T[:, fk, :], psum, AF.Relu,
                                               bias=b1_sb[:, fk:fk + 1], scale=1.0)
                      for of in range(8):
                          w2_t = w2pool.tile([P, 32, P], f32r, tag="w2")
                          nc.sync.dma_start(w2_t, w2_d[of])
                          psum = ps_f2.tile([P, 512], f32, tag="f2")
                          for fk in range(32):
                              nc.tensor.matmul(psum, w2_t[:, fk, :], relu1T[:, fk, :],
                                               start=(fk == 0), stop=(fk == 31))
                          x2c = ypool.tile([P, 512], f32r, tag="x2c")
                          nc.sync.dma_start(
                              x2c, x2s_d[of * P:(of + 1) * P,
                                         rc * 512:(rc + 1) * 512])
                          y_sb = ypool.tile([P, 512], f32, tag="y")
                          nc.vector.scalar_tensor_tensor(
                              y_sb, psum, b2_sb[:, of:of + 1], x2c,
                              op0=AL.add, op1=AL.add)
                          nc.sync.dma_start(
                              y_d[of * P:(of + 1) * P,
                                  rc * 512:(rc + 1) * 512], y_sb)
    _split_sync_waits(nc)
    return nc


_PROGRAM = None


def _get_program():
    global _PROGRAM
    if _PROGRAM is None:
        _PROGRAM = build_program()
    return _PROGRAM


def _host_prep(x, Wk, Wq, Wv, Wproj, bproj, W1, b1, W2, b2, g1, beta1, g2, beta2):
    """Fold LN affine params into weights; build per-core shards."""
    x = np.asarray(x, np.float32)
    scale = 1.0 / np.sqrt(D)
    Wq_f = (g1[:, None] * np.asarray(Wq, np.float32)) * scale
    bq_f = (beta1 @ np.asarray(Wq, np.float32)) * scale
    Wk_f = g1[:, None] * np.asarray(Wk, np.float32)
    bk_f = beta1 @ np.asarray(Wk, np.float32)
    Wv_f = g1[:, None] * np.asarray(Wv, np.float32)
    bv_f = beta1 @ np.asarray(Wv, np.float32)
    W1_f = g2[:, None] * np.asarray(W1, np.float32)
    b1_f = np.asarray(b1, np.float32) + beta2 @ np.asarray(W1, np.float32)

    def tile_in_out(W, n_in, n_out):
        # [in, out] -> [n_out, 128, n_in, 128]: dram block per out-tile with
        # one contiguous 4KB+ run per partition
        return np.ascontiguousarray(
            np.asarray(W, np.float32).reshape(n_in, P, n_out, P).transpose(2, 1, 0, 3))

    common = {
        "wq": tile_in_out(Wq_f, 8, 8),
        "wk": tile_in_out(Wk_f, 8, 8),
        "wv": tile_in_out(Wv_f, 8, 8),
        "wp": np.ascontiguousarray(np.asarray(Wproj, np.float32)),
        "w1": tile_in_out(W1_f, 8, 32),
        "w2": tile_in_out(np.asarray(W2, np.float32), 32, 8),
        "bq": np.ascontiguousarray(bq_f, np.float32),
        "bk": np.ascontiguousarray(bk_f, np.float32),
        "bv": np.ascontiguousarray(bv_f, np.float32),
        "bp": np.ascontiguousarray(np.asarray(bproj, np.float32)),
        "b1": np.ascontiguousarray(b1_f, np.float32),
        "b2": np.ascontiguousarray(np.asarray(b2, np.float32)),
        "onesc": np.ones((P, 1), np.float32),
        "identc": np.eye(P, dtype=np.float32),
    }

    in_maps = []
    row_maps = []
    for core in range(8):
        b, s = core // 2, core % 2
        pairs = _chunk_pairs(s)
        rows = []
        for (a1, a2) in pairs:
            rows.extend(range(a1 * P, a1 * P + P))
            rows.extend(range(a2 * P, a2 * P + P))
        rows = np.array(rows)
        row_maps.append((b, rows))
        xq = np.ascontiguousarray(x[b][rows])
        # mask[j, t, k, q]: key pos 128*(4j+t)+k vs query pos rows[256j+q]
        mask = np.empty((4, 4, P, 256), np.float32)
        for j in range(4):
            qpos = rows[256 * j:256 * j + 256]
            for t in range(4):
                kpos = np.arange(P * (4 * j + t), P * (4 * j + t + 1))
                mask[j, t] = np.where(kpos[:, None] <= qpos[None, :], 0.0, NEG)
        in_maps.append({
            "xq": xq,
            "xkv": np.ascontiguousarray(x[b]),
            "maskc": mask,
            **common,
        })
    return in_maps, row_maps


def kernel(**inputs):
    nc = _get_program()
    in_maps, row_maps = _host_prep(**inputs)
    res = run_bass_kernel_spmd(nc, in_maps, core_ids=list(range(8)))
    out = np.empty((B, T, C), np.float32)
    for core in range(8):
        b, rows = row_maps[core]
        out[b][rows] = res.results[core]["y"].T
    return out



# revision 45
# speedup vs baseline: 2.1681x; 2.1681x over previous
"""Trainium2 Bass kernel for one dense transformer block.

Full (unsharded) IO: x [4, 2048, 1024] -> out [4, 2048, 1024].
Sharding: 8 cores = 4 batches x 2 query-chunk sets. Each core owns one
batch's K/V (2048 rows) and 1024 query rows chosen as causally-balanced
128-row chunks (set A: global chunks {4j, 4j+3}, set B: {4j+1, 4j+2}).
Local chunk slot i attends to exactly 2*(i+1) key tiles on every core, so
the instruction stream is identical across cores (SPMD); a per-core data
mask handles the causal boundary in the last two key tiles. No collectives.

On-core dataflow is feature-on-partition ("transposed") throughout. Matmul
dtypes: Q/K projections and scores in bf16 (f32 psum), attention
probabilities bf16, V / output-projection / FFN in fp8-e4m3 with DoubleRow
perf mode (two 128-deep k-tiles per instruction) and power-of-two weight
scaling to avoid fp8 subnormals. LayerNorm runs in f32. Residuals in f32.
LN affine params and biases are folded host-side (exact); V bias is folded
into the projection bias (exact).
"""

import sys

sys.path.insert(0, "/opt/trn_rl_repo")

import numpy as np
import ml_dtypes

import concourse.bass as bass
import concourse.mybir as mybir
import concourse.tile as tile
from concourse.bass_utils import run_bass_kernel_spmd

f32 = mybir.dt.float32
f32r = mybir.dt.float32r
bf16 = mybir.dt.bfloat16
f8 = mybir.dt.float8e4
AL = mybir.AluOpType
AF = mybir.ActivationFunctionType
DR = mybir.MatmulPerfMode.DoubleRow

B, T, C = 4, 2048, 1024
H, D = 16, 64
F = 4 * C
P = 128
TQ = 1024
NEG = -30000.0
LN_EPS = 1e-5

# dtype strategy flags (validated against the 2e-2 rel-err budget)
V_FP8 = True      # V projection via fp8 DoubleRow
PROJ_FP8 = True   # output projection via fp8 DoubleRow
FFN_FP8 = True    # both FFN matmuls via fp8 DoubleRow
SV = 16.0         # host scale on Wv before fp8 cast
SP_ = 16.0        # host scale on Wproj; OT is scaled by SO on-chip
SO = 16.0         # scale folded into OT (via V scaling path)
S1 = 16.0         # host scale on W1
S2 = 64.0         # host scale on W2

# local chunk slot -> number of key tiles computed (set-independent)
NKT = [2 * (i + 1) for i in range(8)]


def _chunk_map(s):
    """Global 128-row chunk indices owned by set s, in slot order."""
    if s == 0:
        out = []
        for j in range(4):
            out += [4 * j, 4 * j + 3]
        return sorted(out)
    out = []
    for j in range(4):
        out += [4 * j + 1, 4 * j + 2]
    return sorted(out)


def _split_sync_waits(nc):
    """This container's walrus supports one sync-wait per instruction; Tile
    emits up to ~3. Hoist extras onto NoOps inserted before the owner."""
    ctr = 0
    for fn in nc.m.functions:
        for bb in fn.blocks:
            out, changed = [], False
            for ins in bb.instructions:
                si = ins.sync_info
                waits = list(si.on_wait) if si is not None and si.on_wait else []
                if len(waits) > 1:
                    changed = True
                    for w in waits[:-1]:
                        ctr += 1
                        nop = mybir.InstNoOp(name=f"waitsplit_{ctr}", ins=[], outs=[])
                        nop.engine = ins.engine
                        nop.sync_info = mybir.SyncInfo(on_wait=[w], on_update=[])
                        out.append(nop)
                        nc.register_instruction(nop, overwrite=True)
                    ins.sync_info = mybir.SyncInfo(
                        on_wait=[waits[-1]], on_update=list(si.on_update or [])
                    )
                out.append(ins)
            if changed:
                bb.instructions = out


def build_program():
    nc = bass.Bass()
    xkv_d = nc.dram_tensor("xkv", [T, C], f32r, kind="ExternalInput")
    xq_d = nc.dram_tensor("xq", [TQ, C], f32r, kind="ExternalInput")
    mask_d = nc.dram_tensor("maskc", [8, P, 256], bf16, kind="ExternalInput")
    wq_d = nc.dram_tensor("wq", [8, P, 8, P], bf16, kind="ExternalInput")
    wk_d = nc.dram_tensor("wk", [8, P, 8, P], bf16, kind="ExternalInput")
    wv_d = nc.dram_tensor("wv", [P, 8, C], f8 if V_FP8 else bf16,
                          kind="ExternalInput")
    wp_d = nc.dram_tensor("wp", [P, 8, C], f8 if PROJ_FP8 else bf16,
                          kind="ExternalInput")
    w1_d = nc.dram_tensor("w1", [32, P, 8, P], f8 if FFN_FP8 else bf16,
                          kind="ExternalInput")
    w2_d = nc.dram_tensor("w2", [8, P, 32, P], f8 if FFN_FP8 else bf16,
                          kind="ExternalInput")
    bq_d = nc.dram_tensor("bq", [C], f32, kind="ExternalInput")
    bk_d = nc.dram_tensor("bk", [C], f32, kind="ExternalInput")
    bp_d = nc.dram_tensor("bp", [C], f32, kind="ExternalInput")
    b1_d = nc.dram_tensor("b1", [F], f32, kind="ExternalInput")
    b2_d = nc.dram_tensor("b2", [C], f32, kind="ExternalInput")
    identr_d = nc.dram_tensor("identr", [P, P], f32r, kind="ExternalInput")
    identb_d = nc.dram_tensor("identb", [P, P], bf16, kind="ExternalInput")
    onesb_d = nc.dram_tensor("onesb", [P, 16, 16, 1], bf16, kind="ExternalInput")
    onesr_d = nc.dram_tensor("onesr", [P, 1], f32r, kind="ExternalInput")
    y_d = nc.dram_tensor("y", [C, TQ], f32, kind="ExternalOutput")

    fv = f8 if V_FP8 else bf16
    fp = f8 if PROJ_FP8 else bf16
    ff_ = f8 if FFN_FP8 else bf16

    with tile.TileContext(nc) as tc:
      with tc.tile_pool(name="consts", bufs=1) as cpool, \
           tc.tile_pool(name="persist", bufs=1) as pers:
        identr = cpool.tile([P, P], f32r)
        nc.sync.dma_start(identr, identr_d[:, :])
        identb = cpool.tile([P, P], bf16)
        nc.sync.dma_start(identb, identb_d[:, :])
        onesbt = cpool.tile([P, 16, 16, 1], bf16)
        nc.sync.dma_start(onesbt, onesb_d[:, :, :, :])
        onesr = cpool.tile([P, 1], f32r)
        nc.sync.dma_start(onesr, onesr_d[:, :])
        eps128 = cpool.tile([P, 1], f32)
        nc.vector.memset(eps128, LN_EPS)
        eps1 = cpool.tile([1, 1], f32)
        nc.vector.memset(eps1, LN_EPS)
        bq_sb = cpool.tile([P, 8], f32)
        nc.sync.dma_start(bq_sb, bq_d.rearrange("(o p) -> p o", p=P))
        bk_sb = cpool.tile([P, 8], f32)
        nc.sync.dma_start(bk_sb, bk_d.rearrange("(o p) -> p o", p=P))
        bp_sb = cpool.tile([P, 8], f32)
        nc.sync.dma_start(bp_sb, bp_d.rearrange("(o p) -> p o", p=P))
        b1_sb = cpool.tile([P, 32], f32)
        nc.sync.dma_start(b1_sb, b1_d.rearrange("(o p) -> p o", p=P))
        b2_sb = cpool.tile([P, 8], f32)
        nc.sync.dma_start(b2_sb, b2_d.rearrange("(o p) -> p o", p=P))
        mask_sb = cpool.tile([P, 8, 256], bf16)
        for i in range(8):
            nc.sync.dma_start(mask_sb[:, i, :], mask_d[i])

        OT = pers.tile([P, 8, TQ], fp)        # attn out (x SV), transposed
        otq = pers.tile([P, 8, 16, 64], bf16)  # attn out row-major (q, head, d)
        xqT = pers.tile([P, 8, TQ], f32r)     # raw queries, transposed

        # ---------------- Phase A: LN1 + transposes + V + attention --------
        with tc.tile_pool(name="attn_big", bufs=1) as apool:
            hkvT = apool.tile([P, 8, T], bf16)
            hqT = apool.tile([P, 8, TQ], bf16)
            if V_FP8:
                hkv8 = apool.tile([P, 8, T], f8)
            else:
                hkv8 = None
            vaug = apool.tile([P, 16, 16, 65], bf16)
            nc.vector.tensor_copy(vaug[:, :, :, 64:65], onesbt)

            with tc.tile_pool(name="ln_sb", bufs=3) as lnsb, \
                 tc.tile_pool(name="ln_small", bufs=6) as lnsm, \
                 tc.tile_pool(name="ps_a", bufs=2, space="PSUM") as ps_a, \
                 tc.tile_pool(name="ps_v", bufs=2, space="PSUM") as ps_v:

                def ln_row_tile(src_ap, rt, is_q):
                    xrow = lnsb.tile([P, C], f32r, tag="xrow")
                    nc.sync.dma_start(xrow, src_ap)
                    stats = lnsm.tile([P, 2, 6], f32, tag="stats")
                    for sg in range(2):
                        nc.vector.bn_stats(stats[:, sg, :],
                                           xrow[:, sg * 512:(sg + 1) * 512])
                    mv = lnsm.tile([P, 2], f32, tag="mv")
                    nc.vector.bn_aggr(mv, stats)
                    rstd = lnsm.tile([P, 1], f32, tag="rstd")
                    nc.scalar.activation(rstd, mv[:, 1:2], AF.Sqrt,
                                         bias=eps128, scale=1.0)
                    nc.vector.reciprocal(rstd, rstd)
                    hrow = lnsb.tile([P, C], bf16, tag="hrow")
                    nc.gpsimd.tensor_scalar(hrow, xrow, mv[:, 0:1], rstd,
                                            op0=AL.subtract, op1=AL.mult)
                    dstT = hqT if is_q else hkvT
                    # one accumulation "start" per 2KB PSUM bank: later
                    # region-disjoint writes use start=False (pending-zero
                    # bytes are overwritten, not accumulated)
                    pstb = ps_a.tile([P, 8, P], bf16, tag="trb")
                    for ft in range(8):
                        nc.tensor.matmul(pstb[:, ft, :],
                                         hrow[:, ft * P:(ft + 1) * P],
                                         identb, is_transpose=True,
                                         start=(ft == 0), stop=True,
                                         skip_group_check=True)
                    nc.vector.tensor_copy(dstT[:, :, rt * P:(rt + 1) * P],
                                          pstb)
                    if not is_q and V_FP8:
                        nc.scalar.copy(hkv8[:, :, rt * P:(rt + 1) * P], pstb)
                    if is_q:
                        pstr = ps_a.tile([P, 8, P], f32r, tag="trr")
                        for ft in range(8):
                            nc.tensor.matmul(pstr[:, ft, :],
                                             xrow[:, ft * P:(ft + 1) * P],
                                             identr, is_transpose=True,
                                             start=(ft % 4 == 0), stop=True,
                                             skip_group_check=True)
                        nc.scalar.copy(xqT[:, :, rt * P:(rt + 1) * P], pstr)

                for rt in range(16):
                    ln_row_tile(xkv_d[rt * P:(rt + 1) * P, :], rt, False)
                for rt in range(8):
                    ln_row_tile(xq_d[rt * P:(rt + 1) * P, :], rt, True)

                # V projection, row-major for all heads at once
                wv_sb = lnsb.tile([P, 8, C], fv, tag="wv")
                nc.sync.dma_start(wv_sb, wv_d[:, :, :])
                for tc_i in range(16):
                    for fh in range(2):
                        psv = ps_v.tile([P, 8, 64], f32, tag="v")
                        if V_FP8:
                            for q in range(2):
                                for kp in range(4):
                                    nc.tensor.matmul(
                                        psv[:, q * 4:(q + 1) * 4, :],
                                        hkv8[:, 2 * kp:2 * kp + 2,
                                             tc_i * P:(tc_i + 1) * P],
                                        wv_sb[:, 2 * kp:2 * kp + 2,
                                              fh * 512 + q * 256:
                                              fh * 512 + (q + 1) * 256],
                                        perf_mode=DR,
                                        start=(q == 0 and kp == 0),
                                        stop=(kp == 3),
                                        skip_group_check=True)
                        else:
                            for k in range(8):
                                nc.tensor.matmul(
                                    psv[:, :, :],
                                    hkvT[:, k, tc_i * P:(tc_i + 1) * P],
                                    wv_sb[:, k, fh * 512:(fh + 1) * 512],
                                    start=(k == 0), stop=(k == 7))
                        if (tc_i + fh) % 2 == 0:
                            nc.vector.tensor_copy(
                                vaug[:, tc_i, fh * 8:(fh + 1) * 8, 0:64], psv)
                        else:
                            nc.scalar.copy(
                                vaug[:, tc_i, fh * 8:(fh + 1) * 8, 0:64], psv)

            # -------- per-group K/Q projection + attention --------
            with tc.tile_pool(name="kq_w", bufs=2) as wpool, \
                 tc.tile_pool(name="kq_sb", bufs=2) as gpool, \
                 tc.tile_pool(name="pt_sb", bufs=2) as ptpool, \
                 tc.tile_pool(name="sm_sb", bufs=3) as smpool, \
                 tc.tile_pool(name="ps_kqv", bufs=2, space="PSUM") as ps_kqv, \
                 tc.tile_pool(name="ps_sc", bufs=2, space="PSUM") as ps_sc, \
                 tc.tile_pool(name="ps_ob", bufs=2, space="PSUM") as ps_ob:
                for g in range(8):
                    wk_t = wpool.tile([P, 8, P], bf16, tag="wk")
                    nc.sync.dma_start(wk_t, wk_d[g])
                    wq_t = wpool.tile([P, 8, P], bf16, tag="wq")
                    nc.sync.dma_start(wq_t, wq_d[g])
                    KT = gpool.tile([P, T], bf16, tag="KT")
                    for rc in range(4):
                        psum = ps_kqv.tile([P, 512], f32, tag="kqv")
                        for k in range(8):
                            nc.tensor.matmul(
                                psum, wk_t[:, k, :],
                                hkvT[:, k, rc * 512:(rc + 1) * 512],
                                start=(k == 0), stop=(k == 7))
                        if rc % 2 == 0:
                            nc.vector.tensor_scalar_add(
                                KT[:, rc * 512:(rc + 1) * 512], psum,
                                bk_sb[:, g:g + 1])
                        else:
                            nc.scalar.activation(
                                KT[:, rc * 512:(rc + 1) * 512], psum,
                                AF.Identity, bias=bk_sb[:, g:g + 1], scale=1.0)
                    QT = gpool.tile([P, TQ], bf16, tag="QT")
                    for rc in range(2):
                        psum = ps_kqv.tile([P, 512], f32, tag="kqv")
                        for k in range(8):
                            nc.tensor.matmul(
                                psum, wq_t[:, k, :],
                                hqT[:, k, rc * 512:(rc + 1) * 512],
                                start=(k == 0), stop=(k == 7))
                        if rc % 2 == 0:
                            nc.scalar.activation(
                                QT[:, rc * 512:(rc + 1) * 512], psum,
                                AF.Identity, bias=bq_sb[:, g:g + 1], scale=1.0)
                        else:
                            nc.vector.tensor_scalar_add(
                                QT[:, rc * 512:(rc + 1) * 512], psum,
                                bq_sb[:, g:g + 1])

                    for hh in range(2):
                        base = 64 * hh
                        for lc in range(8):
                            nkt = NKT[lc]
                            ptb = ptpool.tile([P, 16, P], bf16, tag="ptb")
                            ob = ps_ob.tile([P, 65], f32, tag="ob")
                            for sg in range((nkt + 7) // 8):
                                n_in_g = min(8, nkt - sg * 8)
                                ps_s = ps_sc.tile([P, 8, P], f32, tag="sc")
                                for sl in range(n_in_g):
                                    kt = sg * 8 + sl
                                    last2 = (kt >= nkt - 2)
                                    nc.tensor.matmul(
                                        ps_s[:, sl, :],
                                        KT[base:base + 64, kt * P:(kt + 1) * P],
                                        QT[base:base + 64, lc * P:(lc + 1) * P],
                                        start=(sl % 4 == 0), stop=not last2,
                                        skip_group_check=True)
                                # causal mask on the last two key tiles
                                sl0 = (nkt - 2) % 8
                                if sg == (nkt - 1) // 8:
                                    nc.tensor.matmul(
                                        ps_s[:, sl0:sl0 + 2, :],
                                        identb, mask_sb[:, lc, :],
                                        start=False, stop=True,
                                        skip_group_check=True)
                                nc.scalar.activation(
                                    ptb[:, sg * 8:sg * 8 + n_in_g, :],
                                    ps_s[:, 0:n_in_g, :], AF.Exp)
                            h_idx = 2 * g + hh
                            for kt in range(nkt):
                                # out[q, d] = sum_k pt[k, q] * vaug[k, d];
                                # column 64 accumulates the softmax denom
                                nc.tensor.matmul(
                                    ob, ptb[:, kt, :],
                                    vaug[:, kt, h_idx, :],
                                    start=(kt == 0), stop=(kt == nkt - 1))
                            rec = smpool.tile([P, 1], f32, tag="rec")
                            nc.vector.reciprocal(rec, ob[:, 64:65])
                            nc.vector.tensor_scalar_mul(
                                otq[:, lc, h_idx, :], ob[:, 0:64], rec)

        # transpose attn out back to feature-on-partition for proj
        with tc.tile_pool(name="ps_trq", bufs=2, space="PSUM") as ps_trq:
            for lc in range(8):
                pq = ps_trq.tile([P, 8, P], bf16, tag="trq")
                for ft in range(8):
                    nc.tensor.matmul(
                        pq[:, ft, :], otq[:, lc, 2 * ft:2 * ft + 2, :],
                        identb, is_transpose=True, start=(ft == 0), stop=True,
                        skip_group_check=True)
                if lc % 2 == 0:
                    nc.vector.tensor_copy(OT[:, :, lc * P:(lc + 1) * P], pq)
                else:
                    nc.scalar.copy(OT[:, :, lc * P:(lc + 1) * P], pq)

        # ---------------- Phase B: proj + residual + LN2 + FFN -------------
        with tc.tile_pool(name="late", bufs=1) as late:
            x2T = late.tile([P, 8, TQ], f32r)
            h2 = late.tile([P, 8, TQ], ff_)
            with tc.tile_pool(name="pr_w", bufs=1) as prpool, \
                 tc.tile_pool(name="pr_tmp", bufs=3) as prtmp, \
                 tc.tile_pool(name="ps_pr", bufs=3, space="PSUM") as ps_pr:
                wp_sb = prpool.tile([P, 8, C], fp)
                nc.sync.dma_start(wp_sb, wp_d[:, :, :])
                # OT carries SV from the V-weight scaling; wp carries SP_
                inv = 1.0 / ((SV if V_FP8 else 1.0) * (SP_ if PROJ_FP8 else 1.0))
                for rc in range(4):
                    for of in range(8):
                        psum = ps_pr.tile([P, 256], f32, tag="pr")
                        if PROJ_FP8:
                            for kp in range(4):
                                nc.tensor.matmul(
                                    psum,
                                    wp_sb[:, 2 * kp:2 * kp + 2,
                                          of * P:(of + 1) * P],
                                    OT[:, 2 * kp:2 * kp + 2,
                                       rc * 256:(rc + 1) * 256],
                                    perf_mode=DR,
                                    start=(kp == 0), stop=(kp == 3))
                        else:
                            for k in range(8):
                                nc.tensor.matmul(
                                    psum, wp_sb[:, k, of * P:(of + 1) * P],
                                    OT[:, k, rc * 256:(rc + 1) * 256],
                                    start=(k == 0), stop=(k == 7))
                        t1 = prtmp.tile([P, 256], f32, tag="t1")
                        nc.scalar.activation(t1, psum, AF.Identity,
                                             bias=bp_sb[:, of:of + 1],
                                             scale=inv)
                        nc.gpsimd.tensor_tensor(
                            x2T[:, of, rc * 256:(rc + 1) * 256], t1,
                            xqT[:, of, rc * 256:(rc + 1) * 256], AL.add)

            # LN2 in transposed layout
            with tc.tile_pool(name="ln2_sb", bufs=4) as ln2sb, \
                 tc.tile_pool(name="ln2_sm", bufs=4) as ln2sm, \
                 tc.tile_pool(name="ps_l2s", bufs=2, space="PSUM") as ps_l2s, \
                 tc.tile_pool(name="ps_l2b", bufs=2, space="PSUM") as ps_l2b:
                for rc in range(2):
                    psum_m = ps_l2s.tile([1, 512], f32, tag="m")
                    psum_q = ps_l2s.tile([1, 512], f32, tag="q")
                    for ft in range(8):
                        xt = x2T[:, ft, rc * 512:(rc + 1) * 512]
                        sq = ln2sb.tile([P, 512], f32r, tag="sq")
                        nc.gpsimd.tensor_tensor(sq, xt, xt, AL.mult)
                        nc.tensor.matmul(psum_m, onesr, xt,
                                         start=(ft == 0), stop=(ft == 7))
                        nc.tensor.matmul(psum_q, onesr, sq,
                                         start=(ft == 0), stop=(ft == 7))
                    mean = ln2sm.tile([1, 512], f32, tag="mean")
                    nc.vector.tensor_scalar_mul(mean, psum_m, 1.0 / C)
                    msq = ln2sm.tile([1, 512], f32, tag="msq")
                    nc.vector.tensor_scalar_mul(msq, psum_q, 1.0 / C)
                    var = ln2sm.tile([1, 512], f32, tag="var")
                    nc.vector.tensor_tensor(var, mean, mean, AL.mult)
                    nc.vector.tensor_tensor(var, msq, var, AL.subtract)
                    std = ln2sm.tile([1, 512], f32, tag="std")
                    nc.scalar.activation(std, var, AF.Sqrt,
                                         bias=eps1[:, :], scale=1.0)
                    a = ln2sm.tile([1, 512], f32r, tag="a")
                    with nc.allow_low_precision(reason="f32r has f32 bits"):
                        nc.vector.reciprocal(a, std)
                    bneg = ln2sm.tile([1, 512], f32r, tag="b")
                    nc.vector.tensor_tensor(bneg, mean, a, AL.mult)
                    ps_abc = ps_l2b.tile([P, 512], f32, tag="abc")
                    nc.tensor.matmul(ps_abc,
                                     onesr[0:1, 0:1].broadcast_to((1, P)), a,
                                     start=True, stop=True)
                    ps_bbc = ps_l2b.tile([P, 512], f32, tag="bbc")
                    nc.tensor.matmul(ps_bbc,
                                     onesr[0:1, 0:1].broadcast_to((1, P)),
                                     bneg, start=True, stop=True)
                    for ft in range(8):
                        tmp = ln2sb.tile([P, 512], f32r, tag="tmp")
                        nc.vector.tensor_tensor(
                            tmp, x2T[:, ft, rc * 512:(rc + 1) * 512],
                            ps_abc, AL.mult)
                        nc.vector.tensor_tensor(
                            h2[:, ft, rc * 512:(rc + 1) * 512], tmp,
                            ps_bbc, AL.subtract)

            # FFN
            with tc.tile_pool(name="f_w1", bufs=3) as w1pool, \
                 tc.tile_pool(name="f_w2", bufs=2) as w2pool, \
                 tc.tile_pool(name="f_relu", bufs=1) as rpool, \
                 tc.tile_pool(name="f_y", bufs=3) as ypool, \
                 tc.tile_pool(name="ps_f1", bufs=4, space="PSUM") as ps_f1, \
                 tc.tile_pool(name="ps_f2", bufs=4, space="PSUM") as ps_f2:
                relu1 = rpool.tile([P, 32, TQ], ff_)
                for fk in range(32):
                    w1_t = w1pool.tile([P, 8, P], ff_, tag="w1")
                    nc.sync.dma_start(w1_t, w1_d[fk])
                    for rc in range(2):
                        psum = ps_f1.tile([P, 512], f32, tag="f1")
                        if FFN_FP8:
                            for q in range(2):
                                for kp in range(4):
                                    nc.tensor.matmul(
                                        psum[:, q * 256:(q + 1) * 256],
                                        w1_t[:, 2 * kp:2 * kp + 2, :],
                                        h2[:, 2 * kp:2 * kp + 2,
                                           rc * 512 + q * 256:
                                           rc * 512 + (q + 1) * 256],
                                        perf_mode=DR,
                                        start=(q == 0 and kp == 0),
                                        stop=(kp == 3),
                                        skip_group_check=True)
                        else:
                            for k in range(8):
                                nc.tensor.matmul(
                                    psum, w1_t[:, k, :],
                                    h2[:, k, rc * 512:(rc + 1) * 512],
                                    start=(k == 0), stop=(k == 7))
                        out_sl = relu1[:, fk, rc * 512:(rc + 1) * 512]
                        if fk % 2 == 0:
                            nc.scalar.activation(out_sl, psum, AF.Relu,
                                                 bias=b1_sb[:, fk:fk + 1],
                                                 scale=1.0)
                        else:
                            nc.vector.tensor_scalar(out_sl, psum,
                                                    b1_sb[:, fk:fk + 1], 0.0,
                                                    op0=AL.add, op1=AL.max)
                for of in range(8):
                    w2_t = w2pool.tile([P, 32, P], ff_, tag="w2")
                    nc.sync.dma_start(w2_t, w2_d[of])
                    for rc in range(2):
                        psum = ps_f2.tile([P, 512], f32, tag="f2")
                        if FFN_FP8:
                            for q in range(2):
                                for kp in range(16):
                                    nc.tensor.matmul(
                                        psum[:, q * 256:(q + 1) * 256],
                                        w2_t[:, 2 * kp:2 * kp + 2, :],
                                        relu1[:, 2 * kp:2 * kp + 2,
                                              rc * 512 + q * 256:
                                              rc * 512 + (q + 1) * 256],
                                        perf_mode=DR,
                                        start=(q == 0 and kp == 0),
                                        stop=(kp == 15),
                                        skip_group_check=True)
                        else:
                            for fk in range(32):
                                nc.tensor.matmul(
                                    psum, w2_t[:, fk, :],
                                    relu1[:, fk, rc * 512:(rc + 1) * 512],
                                    start=(fk == 0), stop=(fk == 31))
                        t2 = ypool.tile([P, 512], f32, tag="t2")
                        inv2 = 1.0 / (S1 * S2) if FFN_FP8 else 1.0
                        nc.scalar.activation(t2, psum, AF.Identity,
                                             bias=b2_sb[:, of:of + 1],
                                             scale=inv2)
                        y_sb = ypool.tile([P, 512], f32, tag="y")
                        nc.gpsimd.tensor_tensor(
                            y_sb, t2, x2T[:, of, rc * 512:(rc + 1) * 512],
                            AL.add)
                        nc.sync.dma_start(
                            y_d[of * P:(of + 1) * P,
                                rc * 512:(rc + 1) * 512], y_sb)
    _split_sync_waits(nc)
    return nc


_PROGRAM = None


def _get_program():
    global _PROGRAM
    if _PROGRAM is None:
        _PROGRAM = build_program()
    return _PROGRAM


def _to_bf16(a):
    return np.asarray(a, np.float32).astype(ml_dtypes.bfloat16)


def _to_f8(a):
    return np.asarray(a, np.float32).astype(ml_dtypes.float8_e4m3)


def _host_prep(x, Wk, Wq, Wv, Wproj, bproj, W1, b1, W2, b2, g1, beta1, g2, beta2):
    """Fold LN affine params / V bias into weights (exact); quantize; shard."""
    x = np.asarray(x, np.float32)
    scale = 1.0 / np.sqrt(D)
    Wq_f = (g1[:, None] * np.asarray(Wq, np.float32)) * scale
    bq_f = (beta1 @ np.asarray(Wq, np.float32)) * scale
    Wk_f = g1[:, None] * np.asarray(Wk, np.float32)
    bk_f = beta1 @ np.asarray(Wk, np.float32)
    Wv_f = g1[:, None] * np.asarray(Wv, np.float32)
    bv_f = beta1 @ np.asarray(Wv, np.float32)
    Wp_f = np.asarray(Wproj, np.float32)
    bp_f = np.asarray(bproj, np.float32) + bv_f @ Wp_f
    W1_f = g2[:, None] * np.asarray(W1, np.float32)
    b1_f = np.asarray(b1, np.float32) + beta2 @ np.asarray(W1, np.float32)
    W2_f = np.asarray(W2, np.float32)
    b2_f = np.asarray(b2, np.float32)

    def tile_in_out(W, n_in, n_out, q):
        # [in, out] -> [n_out, 128, n_in, 128]
        return np.ascontiguousarray(q(
            W.reshape(n_in, P, n_out, P).transpose(2, 1, 0, 3)))

    def pko(W, q):
        # [in, out] -> [128, n_in_tiles, out]
        return np.ascontiguousarray(q(
            W.reshape(8, P, W.shape[1]).transpose(1, 0, 2)))

    common = {
        "wq": tile_in_out(Wq_f, 8, 8, _to_bf16),
        "wk": tile_in_out(Wk_f, 8, 8, _to_bf16),
        "wv": pko(Wv_f * (SV if V_FP8 else 1.0),
                  _to_f8 if V_FP8 else _to_bf16),
        "wp": pko(Wp_f * (SP_ if PROJ_FP8 else 1.0),
                  _to_f8 if PROJ_FP8 else _to_bf16),
        "w1": tile_in_out(W1_f * (S1 if FFN_FP8 else 1.0), 8, 32,
                          _to_f8 if FFN_FP8 else _to_bf16),
        "w2": tile_in_out(W2_f * (S2 if FFN_FP8 else 1.0), 32, 8,
                          _to_f8 if FFN_FP8 else _to_bf16),
        "bq": np.ascontiguousarray(bq_f, np.float32),
        "bk": np.ascontiguousarray(bk_f, np.float32),
        "bp": np.ascontiguousarray(bp_f, np.float32),
        "b1": np.ascontiguousarray(b1_f * (S1 if FFN_FP8 else 1.0), np.float32),
        "b2": np.ascontiguousarray(b2_f, np.float32),
        "identr": np.eye(P, dtype=np.float32),
        "identb": np.eye(P, dtype=np.float32).astype(ml_dtypes.bfloat16),
        "onesb": np.ones((P, 16, 16, 1), np.float32).astype(ml_dtypes.bfloat16),
        "onesr": np.ones((P, 1), np.float32),
    }

    in_maps = []
    row_maps = []
    for core in range(8):
        b, s = core // 2, core % 2
        gmap = _chunk_map(s)
        rows = np.concatenate([np.arange(G * P, (G + 1) * P) for G in gmap])
        row_maps.append((b, rows))
        # mask[i, k, slot*128+q]: key pos 256i+128*slot+k vs query pos 128*G+q
        mask = np.empty((8, P, 256), np.float32)
        for i, G in enumerate(gmap):
            kpos = (256 * i + np.arange(256)).reshape(2, P).transpose(1, 0)
            qpos = G * P + np.arange(P)
            m = np.where(kpos[:, :, None] <= qpos[None, None, :], 0.0, NEG)
            mask[i] = m.reshape(P, 2, P).transpose(0, 1, 2).reshape(P, 256)
        in_maps.append({
            "xkv": np.ascontiguousarray(x[b]),
            "xq": np.ascontiguousarray(x[b][rows]),
            "maskc": mask.astype(ml_dtypes.bfloat16),
            **common,
        })
    return in_maps, row_maps


def kernel(**inputs):
    nc = _get_program()
    in_maps, row_maps = _host_prep(**inputs)
    res = run_bass_kernel_spmd(nc, in_maps, core_ids=list(range(8)))
    out = np.empty((B, T, C), np.float32)
    for core in range(8):
        b, rows = row_maps[core]
        out[b][rows] = res.results[core]["y"].T
    return out


# revision 49
# speedup vs baseline: 2.2366x; 1.0316x over previous
"""Trainium2 Bass kernel for one dense transformer block.

Full (unsharded) IO: x [4, 2048, 1024] -> out [4, 2048, 1024].
Sharding: 8 cores = 4 batches x 2 query-chunk sets. Each core owns one
batch's K/V (2048 rows) and 1024 query rows chosen as causally-balanced
128-row chunks (set A: global chunks {4j, 4j+3}, set B: {4j+1, 4j+2}).
Local chunk slot i attends to exactly 2*(i+1) key tiles on every core, so
the instruction stream is identical across cores (SPMD); a per-core data
mask handles the causal boundary in the last two key tiles. No collectives.

On-core dataflow is feature-on-partition ("transposed") throughout. Matmul
dtypes: Q/K projections and scores in bf16 (f32 psum), attention
probabilities bf16, V / output-projection / FFN in fp8-e4m3 with DoubleRow
perf mode (two 128-deep k-tiles per instruction) and power-of-two weight
scaling to avoid fp8 subnormals. LayerNorm runs in f32. Residuals in f32.
LN affine params and biases are folded host-side (exact); V bias is folded
into the projection bias (exact).
"""

import sys

sys.path.insert(0, "/opt/trn_rl_repo")

import numpy as np
import ml_dtypes

import concourse.bass as bass
import concourse.mybir as mybir
import concourse.tile as tile
from concourse.bass_utils import run_bass_kernel_spmd

f32 = mybir.dt.float32
f32r = mybir.dt.float32r
bf16 = mybir.dt.bfloat16
f8 = mybir.dt.float8e4
AL = mybir.AluOpType
AF = mybir.ActivationFunctionType
DR = mybir.MatmulPerfMode.DoubleRow

B, T, C = 4, 2048, 1024
H, D = 16, 64
F = 4 * C
P = 128
TQ = 1024
NEG = -30000.0
LN_EPS = 1e-5

# dtype strategy flags (validated against the 2e-2 rel-err budget)
V_FP8 = True      # V projection via fp8 DoubleRow
PROJ_FP8 = True   # output projection via fp8 DoubleRow
FFN_FP8 = True    # both FFN matmuls via fp8 DoubleRow
SV = 16.0         # host scale on Wv before fp8 cast (carried into OT)
SP_ = 16.0        # host scale on Wproj
S1 = 16.0         # host scale on W1
S2 = 64.0         # host scale on W2

# local chunk slot -> number of key tiles computed (set-independent)
NKT = [2 * (i + 1) for i in range(8)]


def _chunk_map(s):
    """Global 128-row chunk indices owned by set s, in slot order."""
    if s == 0:
        out = []
        for j in range(4):
            out += [4 * j, 4 * j + 3]
        return sorted(out)
    out = []
    for j in range(4):
        out += [4 * j + 1, 4 * j + 2]
    return sorted(out)


def _split_sync_waits(nc):
    """This container's walrus supports one sync-wait per instruction; Tile
    emits up to ~3. Hoist extras onto NoOps inserted before the owner."""
    ctr = 0
    for fn in nc.m.functions:
        for bb in fn.blocks:
            out, changed = [], False
            for ins in bb.instructions:
                si = ins.sync_info
                waits = list(si.on_wait) if si is not None and si.on_wait else []
                if len(waits) > 1:
                    changed = True
                    for w in waits[:-1]:
                        ctr += 1
                        nop = mybir.InstNoOp(name=f"waitsplit_{ctr}", ins=[], outs=[])
                        nop.engine = ins.engine
                        nop.sync_info = mybir.SyncInfo(on_wait=[w], on_update=[])
                        out.append(nop)
                        nc.register_instruction(nop, overwrite=True)
                    ins.sync_info = mybir.SyncInfo(
                        on_wait=[waits[-1]], on_update=list(si.on_update or [])
                    )
                out.append(ins)
            if changed:
                bb.instructions = out


def build_program():
    nc = bass.Bass()
    xkv_d = nc.dram_tensor("xkv", [T, C], f32r, kind="ExternalInput")
    xq_d = nc.dram_tensor("xq", [TQ, C], f32r, kind="ExternalInput")
    mask_d = nc.dram_tensor("maskc", [8, P, 256], bf16, kind="ExternalInput")
    wq_d = nc.dram_tensor("wq", [8, P, 8, P], bf16, kind="ExternalInput")
    wk_d = nc.dram_tensor("wk", [8, P, 8, P], bf16, kind="ExternalInput")
    wv_d = nc.dram_tensor("wv", [P, 8, C], f8 if V_FP8 else bf16,
                          kind="ExternalInput")
    wp_d = nc.dram_tensor("wp", [P, 8, C], f8 if PROJ_FP8 else bf16,
                          kind="ExternalInput")
    w1_d = nc.dram_tensor("w1", [32, P, 8, P], f8 if FFN_FP8 else bf16,
                          kind="ExternalInput")
    w2_d = nc.dram_tensor("w2", [8, P, 32, P], f8 if FFN_FP8 else bf16,
                          kind="ExternalInput")
    bq_d = nc.dram_tensor("bq", [C], f32, kind="ExternalInput")
    bk_d = nc.dram_tensor("bk", [C], f32, kind="ExternalInput")
    bp_d = nc.dram_tensor("bp", [C], f32, kind="ExternalInput")
    b1_d = nc.dram_tensor("b1", [F], f32, kind="ExternalInput")
    b2_d = nc.dram_tensor("b2", [C], f32, kind="ExternalInput")
    identr_d = nc.dram_tensor("identr", [P, P], f32r, kind="ExternalInput")
    identb_d = nc.dram_tensor("identb", [P, P], bf16, kind="ExternalInput")
    onesb_d = nc.dram_tensor("onesb", [P, 16, 16, 1], bf16, kind="ExternalInput")
    onesr_d = nc.dram_tensor("onesr", [P, 1], f32r, kind="ExternalInput")
    y_d = nc.dram_tensor("y", [C, TQ], f32, kind="ExternalOutput")

    fv = f8 if V_FP8 else bf16
    fp = f8 if PROJ_FP8 else bf16
    ff_ = f8 if FFN_FP8 else bf16

    with tile.TileContext(nc) as tc:
      with tc.tile_pool(name="consts", bufs=1) as cpool, \
           tc.tile_pool(name="persist", bufs=1) as pers:
        identr = cpool.tile([P, P], f32r)
        nc.sync.dma_start(identr, identr_d[:, :])
        identb = cpool.tile([P, P], bf16)
        nc.sync.dma_start(identb, identb_d[:, :])
        onesbt = cpool.tile([P, 16, 16, 1], bf16)
        nc.sync.dma_start(onesbt, onesb_d[:, :, :, :])
        onesr = cpool.tile([P, 1], f32r)
        nc.sync.dma_start(onesr, onesr_d[:, :])
        eps128 = cpool.tile([P, 1], f32)
        nc.vector.memset(eps128, LN_EPS)
        eps1 = cpool.tile([1, 1], f32)
        nc.vector.memset(eps1, LN_EPS)
        bq_sb = cpool.tile([P, 8], f32)
        nc.sync.dma_start(bq_sb, bq_d.rearrange("(o p) -> p o", p=P))
        bk_sb = cpool.tile([P, 8], f32)
        nc.sync.dma_start(bk_sb, bk_d.rearrange("(o p) -> p o", p=P))
        bp_sb = cpool.tile([P, 8], f32)
        nc.sync.dma_start(bp_sb, bp_d.rearrange("(o p) -> p o", p=P))
        b1_sb = cpool.tile([P, 32], f32)
        nc.sync.dma_start(b1_sb, b1_d.rearrange("(o p) -> p o", p=P))
        b2_sb = cpool.tile([P, 8], f32)
        nc.sync.dma_start(b2_sb, b2_d.rearrange("(o p) -> p o", p=P))
        mask_sb = cpool.tile([P, 8, 256], bf16)
        for i in range(8):
            nc.sync.dma_start(mask_sb[:, i, :], mask_d[i])

        OT = pers.tile([P, 8, TQ], fp)        # attn out (x SV), transposed
        otq = pers.tile([P, 8, 16, 64], bf16)  # attn out row-major (q, head, d)
        xqT = pers.tile([P, 8, TQ], f32r)     # raw queries, transposed

        # ---------------- Phase A: LN1 + transposes + V + attention --------
        with tc.tile_pool(name="attn_big", bufs=1) as apool:
            hkvT = apool.tile([P, 8, T], bf16)
            hqT = apool.tile([P, 8, TQ], bf16)
            if V_FP8:
                hkv8 = apool.tile([P, 8, T], f8)
            else:
                hkv8 = None
            vaug = apool.tile([P, 16, 16, 65], bf16)
            nc.vector.tensor_copy(vaug[:, :, :, 64:65], onesbt)

            with tc.tile_pool(name="ln_sb", bufs=3) as lnsb, \
                 tc.tile_pool(name="ln_small", bufs=6) as lnsm, \
                 tc.tile_pool(name="ps_a", bufs=2, space="PSUM") as ps_a, \
                 tc.tile_pool(name="ps_v", bufs=2, space="PSUM") as ps_v:

                def ln_row_tile(src_ap, rt, is_q):
                    xrow = lnsb.tile([P, C], f32r, tag="xrow")
                    nc.sync.dma_start(xrow, src_ap)
                    stats = lnsm.tile([P, 2, 6], f32, tag="stats")
                    for sg in range(2):
                        nc.vector.bn_stats(stats[:, sg, :],
                                           xrow[:, sg * 512:(sg + 1) * 512])
                    mv = lnsm.tile([P, 2], f32, tag="mv")
                    nc.vector.bn_aggr(mv, stats)
                    rstd = lnsm.tile([P, 1], f32, tag="rstd")
                    nc.scalar.activation(rstd, mv[:, 1:2], AF.Sqrt,
                                         bias=eps128, scale=1.0)
                    nc.vector.reciprocal(rstd, rstd)
                    hrow = lnsb.tile([P, C], bf16, tag="hrow")
                    nc.gpsimd.tensor_scalar(hrow, xrow, mv[:, 0:1], rstd,
                                            op0=AL.subtract, op1=AL.mult)
                    dstT = hqT if is_q else hkvT
                    # one accumulation "start" per 2KB PSUM bank: later
                    # region-disjoint writes use start=False (pending-zero
                    # bytes are overwritten, not accumulated)
                    pstb = ps_a.tile([P, 8, P], bf16, tag="trb")
                    for ft in range(8):
                        nc.tensor.matmul(pstb[:, ft, :],
                                         hrow[:, ft * P:(ft + 1) * P],
                                         identb, is_transpose=True,
                                         start=(ft == 0), stop=True,
                                         skip_group_check=True)
                    if rt % 2 == 0:
                        nc.vector.tensor_copy(dstT[:, :, rt * P:(rt + 1) * P],
                                              pstb)
                        if not is_q and V_FP8:
                            nc.scalar.copy(hkv8[:, :, rt * P:(rt + 1) * P],
                                           pstb)
                    else:
                        nc.scalar.copy(dstT[:, :, rt * P:(rt + 1) * P], pstb)
                        if not is_q and V_FP8:
                            nc.vector.tensor_copy(
                                hkv8[:, :, rt * P:(rt + 1) * P], pstb)
                    if is_q:
                        pstr = ps_a.tile([P, 8, P], f32r, tag="trr")
                        for ft in range(8):
                            nc.tensor.matmul(pstr[:, ft, :],
                                             xrow[:, ft * P:(ft + 1) * P],
                                             identr, is_transpose=True,
                                             start=(ft % 4 == 0), stop=True,
                                             skip_group_check=True)
                        nc.scalar.copy(xqT[:, :, rt * P:(rt + 1) * P], pstr)

                for rt in range(16):
                    ln_row_tile(xkv_d[rt * P:(rt + 1) * P, :], rt, False)
                for rt in range(8):
                    ln_row_tile(xq_d[rt * P:(rt + 1) * P, :], rt, True)

                # V projection, row-major for all heads at once
                wv_sb = lnsb.tile([P, 8, C], fv, tag="wv")
                nc.sync.dma_start(wv_sb, wv_d[:, :, :])
                for tc_i in range(16):
                    for fh in range(2):
                        psv = ps_v.tile([P, 8, 64], f32, tag="v")
                        if V_FP8:
                            for q in range(2):
                                for kp in range(4):
                                    nc.tensor.matmul(
                                        psv[:, q * 4:(q + 1) * 4, :],
                                        hkv8[:, 2 * kp:2 * kp + 2,
                                             tc_i * P:(tc_i + 1) * P],
                                        wv_sb[:, 2 * kp:2 * kp + 2,
                                              fh * 512 + q * 256:
                                              fh * 512 + (q + 1) * 256],
                                        perf_mode=DR,
                                        start=(q == 0 and kp == 0),
                                        stop=(kp == 3),
                                        skip_group_check=True)
                        else:
                            for k in range(8):
                                nc.tensor.matmul(
                                    psv[:, :, :],
                                    hkvT[:, k, tc_i * P:(tc_i + 1) * P],
                                    wv_sb[:, k, fh * 512:(fh + 1) * 512],
                                    start=(k == 0), stop=(k == 7))
                        if (tc_i + fh) % 2 == 0:
                            nc.vector.tensor_copy(
                                vaug[:, tc_i, fh * 8:(fh + 1) * 8, 0:64], psv)
                        else:
                            nc.scalar.copy(
                                vaug[:, tc_i, fh * 8:(fh + 1) * 8, 0:64], psv)

            # -------- per-group K/Q projection + attention --------
            with tc.tile_pool(name="kq_w", bufs=2) as wpool, \
                 tc.tile_pool(name="kq_sb", bufs=2) as gpool, \
                 tc.tile_pool(name="pt_sb", bufs=2) as ptpool, \
                 tc.tile_pool(name="sm_sb", bufs=3) as smpool, \
                 tc.tile_pool(name="ps_kqv", bufs=2, space="PSUM") as ps_kqv, \
                 tc.tile_pool(name="ps_sc", bufs=2, space="PSUM") as ps_sc, \
                 tc.tile_pool(name="ps_ob", bufs=2, space="PSUM") as ps_ob:
                for g in range(8):
                    wk_t = wpool.tile([P, 8, P], bf16, tag="wk")
                    nc.sync.dma_start(wk_t, wk_d[g])
                    wq_t = wpool.tile([P, 8, P], bf16, tag="wq")
                    nc.sync.dma_start(wq_t, wq_d[g])
                    KT = gpool.tile([P, T], bf16, tag="KT")
                    for rc in range(4):
                        psum = ps_kqv.tile([P, 512], f32, tag="kqv")
                        for k in range(8):
                            nc.tensor.matmul(
                                psum, wk_t[:, k, :],
                                hkvT[:, k, rc * 512:(rc + 1) * 512],
                                start=(k == 0), stop=(k == 7))
                        if rc % 2 == 0:
                            nc.vector.tensor_scalar_add(
                                KT[:, rc * 512:(rc + 1) * 512], psum,
                                bk_sb[:, g:g + 1])
                        else:
                            nc.scalar.activation(
                                KT[:, rc * 512:(rc + 1) * 512], psum,
                                AF.Identity, bias=bk_sb[:, g:g + 1], scale=1.0)
                    QT = gpool.tile([P, TQ], bf16, tag="QT")
                    for rc in range(2):
                        psum = ps_kqv.tile([P, 512], f32, tag="kqv")
                        for k in range(8):
                            nc.tensor.matmul(
                                psum, wq_t[:, k, :],
                                hqT[:, k, rc * 512:(rc + 1) * 512],
                                start=(k == 0), stop=(k == 7))
                        if rc % 2 == 0:
                            nc.scalar.activation(
                                QT[:, rc * 512:(rc + 1) * 512], psum,
                                AF.Identity, bias=bq_sb[:, g:g + 1], scale=1.0)
                        else:
                            nc.vector.tensor_scalar_add(
                                QT[:, rc * 512:(rc + 1) * 512], psum,
                                bq_sb[:, g:g + 1])

                    for hh in range(2):
                        base = 64 * hh
                        for lc in range(8):
                            nkt = NKT[lc]
                            ptb = ptpool.tile([P, 16, P], bf16, tag="ptb")
                            ob = ps_ob.tile([P, 65], f32, tag="ob")
                            for sg in range((nkt + 7) // 8):
                                n_in_g = min(8, nkt - sg * 8)
                                ps_s = ps_sc.tile([P, 8, P], f32, tag="sc")
                                for sl in range(n_in_g):
                                    kt = sg * 8 + sl
                                    last2 = (kt >= nkt - 2)
                                    nc.tensor.matmul(
                                        ps_s[:, sl, :],
                                        KT[base:base + 64, kt * P:(kt + 1) * P],
                                        QT[base:base + 64, lc * P:(lc + 1) * P],
                                        start=(sl % 4 == 0), stop=not last2,
                                        skip_group_check=True)
                                # causal mask on the last two key tiles
                                sl0 = (nkt - 2) % 8
                                if sg == (nkt - 1) // 8:
                                    nc.tensor.matmul(
                                        ps_s[:, sl0:sl0 + 2, :],
                                        identb, mask_sb[:, lc, :],
                                        start=False, stop=True,
                                        skip_group_check=True)
                                nc.scalar.activation(
                                    ptb[:, sg * 8:sg * 8 + n_in_g, :],
                                    ps_s[:, 0:n_in_g, :], AF.Exp)
                            h_idx = 2 * g + hh
                            for kt in range(nkt):
                                # out[q, d] = sum_k pt[k, q] * vaug[k, d];
                                # column 64 accumulates the softmax denom
                                nc.tensor.matmul(
                                    ob, ptb[:, kt, :],
                                    vaug[:, kt, h_idx, :],
                                    start=(kt == 0), stop=(kt == nkt - 1))
                            rec = smpool.tile([P, 1], f32, tag="rec")
                            nc.vector.reciprocal(rec, ob[:, 64:65])
                            nc.vector.tensor_scalar_mul(
                                otq[:, lc, h_idx, :], ob[:, 0:64], rec)

        # transpose attn out back to feature-on-partition for proj
        with tc.tile_pool(name="ps_trq", bufs=2, space="PSUM") as ps_trq:
            for lc in range(8):
                pq = ps_trq.tile([P, 8, P], bf16, tag="trq")
                for ft in range(8):
                    nc.tensor.matmul(
                        pq[:, ft, :], otq[:, lc, 2 * ft:2 * ft + 2, :],
                        identb, is_transpose=True, start=(ft == 0), stop=True,
                        skip_group_check=True)
                if lc % 2 == 0:
                    nc.vector.tensor_copy(OT[:, :, lc * P:(lc + 1) * P], pq)
                else:
                    nc.scalar.copy(OT[:, :, lc * P:(lc + 1) * P], pq)

        # ---------------- Phase B: proj + residual + LN2 + FFN -------------
        with tc.tile_pool(name="late", bufs=1) as late:
            x2T = late.tile([P, 8, TQ], f32r)
            h2 = late.tile([P, 8, TQ], ff_)
            with tc.tile_pool(name="pr_w", bufs=1) as prpool, \
                 tc.tile_pool(name="pr_tmp", bufs=3) as prtmp, \
                 tc.tile_pool(name="ps_pr", bufs=3, space="PSUM") as ps_pr:
                wp_sb = prpool.tile([P, 8, C], fp)
                nc.sync.dma_start(wp_sb, wp_d[:, :, :])
                # OT carries SV from the V-weight scaling; wp carries SP_
                inv = 1.0 / ((SV if V_FP8 else 1.0) * (SP_ if PROJ_FP8 else 1.0))
                for rc in range(4):
                    for of in range(8):
                        psum = ps_pr.tile([P, 256], f32, tag="pr")
                        if PROJ_FP8:
                            for kp in range(4):
                                nc.tensor.matmul(
                                    psum,
                                    wp_sb[:, 2 * kp:2 * kp + 2,
                                          of * P:(of + 1) * P],
                                    OT[:, 2 * kp:2 * kp + 2,
                                       rc * 256:(rc + 1) * 256],
                                    perf_mode=DR,
                                    start=(kp == 0), stop=(kp == 3))
                        else:
                            for k in range(8):
                                nc.tensor.matmul(
                                    psum, wp_sb[:, k, of * P:(of + 1) * P],
                                    OT[:, k, rc * 256:(rc + 1) * 256],
                                    start=(k == 0), stop=(k == 7))
                        t1 = prtmp.tile([P, 256], f32, tag="t1")
                        nc.scalar.activation(t1, psum, AF.Identity,
                                             bias=bp_sb[:, of:of + 1],
                                             scale=inv)
                        nc.gpsimd.tensor_tensor(
                            x2T[:, of, rc * 256:(rc + 1) * 256], t1,
                            xqT[:, of, rc * 256:(rc + 1) * 256], AL.add)

            # LN2 in transposed layout
            with tc.tile_pool(name="ln2_sb", bufs=4) as ln2sb, \
                 tc.tile_pool(name="ln2_sm", bufs=4) as ln2sm, \
                 tc.tile_pool(name="ps_l2s", bufs=2, space="PSUM") as ps_l2s, \
                 tc.tile_pool(name="ps_l2b", bufs=2, space="PSUM") as ps_l2b:
                for rc in range(2):
                    psum_m = ps_l2s.tile([1, 512], f32, tag="m")
                    psum_q = ps_l2s.tile([1, 512], f32, tag="q")
                    for ft in range(8):
                        xt = x2T[:, ft, rc * 512:(rc + 1) * 512]
                        sq = ln2sb.tile([P, 512], f32r, tag="sq")
                        nc.scalar.square(sq, xt)
                        nc.tensor.matmul(psum_m, onesr, xt,
                                         start=(ft == 0), stop=(ft == 7))
                        nc.tensor.matmul(psum_q, onesr, sq,
                                         start=(ft == 0), stop=(ft == 7))
                    mean = ln2sm.tile([1, 512], f32, tag="mean")
                    nc.vector.tensor_scalar_mul(mean, psum_m, 1.0 / C)
                    msq = ln2sm.tile([1, 512], f32, tag="msq")
                    nc.vector.tensor_scalar_mul(msq, psum_q, 1.0 / C)
                    var = ln2sm.tile([1, 512], f32, tag="var")
                    nc.vector.tensor_tensor(var, mean, mean, AL.mult)
                    nc.vector.tensor_tensor(var, msq, var, AL.subtract)
                    std = ln2sm.tile([1, 512], f32, tag="std")
                    nc.scalar.activation(std, var, AF.Sqrt,
                                         bias=eps1[:, :], scale=1.0)
                    a = ln2sm.tile([1, 512], f32r, tag="a")
                    with nc.allow_low_precision(reason="f32r has f32 bits"):
                        nc.vector.reciprocal(a, std)
                    bneg = ln2sm.tile([1, 512], f32r, tag="b")
                    nc.vector.tensor_tensor(bneg, mean, a, AL.mult)
                    ps_abc = ps_l2b.tile([P, 512], f32, tag="abc")
                    nc.tensor.matmul(ps_abc,
                                     onesr[0:1, 0:1].broadcast_to((1, P)), a,
                                     start=True, stop=True)
                    ps_bbc = ps_l2b.tile([P, 512], f32, tag="bbc")
                    nc.tensor.matmul(ps_bbc,
                                     onesr[0:1, 0:1].broadcast_to((1, P)),
                                     bneg, start=True, stop=True)
                    # SBUF copies of the broadcasts so gpsimd (no PSUM
                    # access) can share the normalize work with DVE
                    abc_sb = ln2sm.tile([P, 512], f32r, tag="abcs")
                    nc.scalar.copy(abc_sb, ps_abc)
                    bbc_sb = ln2sm.tile([P, 512], f32r, tag="bbcs")
                    nc.scalar.copy(bbc_sb, ps_bbc)
                    for ft in range(8):
                        tmp = ln2sb.tile([P, 512], f32r, tag="tmp")
                        if ft % 2 == 0:
                            nc.vector.tensor_tensor(
                                tmp, x2T[:, ft, rc * 512:(rc + 1) * 512],
                                ps_abc, AL.mult)
                            nc.vector.tensor_tensor(
                                h2[:, ft, rc * 512:(rc + 1) * 512], tmp,
                                ps_bbc, AL.subtract)
                        else:
                            nc.gpsimd.tensor_tensor(
                                tmp, x2T[:, ft, rc * 512:(rc + 1) * 512],
                                abc_sb, AL.mult)
                            nc.gpsimd.tensor_tensor(
                                h2[:, ft, rc * 512:(rc + 1) * 512], tmp,
                                bbc_sb, AL.subtract)

            # FFN
            with tc.tile_pool(name="f_w1", bufs=3) as w1pool, \
                 tc.tile_pool(name="f_w2", bufs=2) as w2pool, \
                 tc.tile_pool(name="f_relu", bufs=1) as rpool, \
                 tc.tile_pool(name="f_y", bufs=3) as ypool, \
                 tc.tile_pool(name="ps_f1", bufs=4, space="PSUM") as ps_f1, \
                 tc.tile_pool(name="ps_f2", bufs=4, space="PSUM") as ps_f2:
                relu1 = rpool.tile([P, 32, TQ], ff_)
                for fk in range(32):
                    w1_t = w1pool.tile([P, 8, P], ff_, tag="w1")
                    nc.sync.dma_start(w1_t, w1_d[fk])
                    for rc in range(2):
                        psum = ps_f1.tile([P, 512], f32, tag="f1")
                        if FFN_FP8:
                            for q in range(2):
                                for kp in range(4):
                                    nc.tensor.matmul(
                                        psum[:, q * 256:(q + 1) * 256],
                                        w1_t[:, 2 * kp:2 * kp + 2, :],
                                        h2[:, 2 * kp:2 * kp + 2,
                                           rc * 512 + q * 256:
                                           rc * 512 + (q + 1) * 256],
                                        perf_mode=DR,
                                        start=(q == 0 and kp == 0),
                                        stop=(kp == 3),
                                        skip_group_check=True)
                        else:
                            for k in range(8):
                                nc.tensor.matmul(
                                    psum, w1_t[:, k, :],
                                    h2[:, k, rc * 512:(rc + 1) * 512],
                                    start=(k == 0), stop=(k == 7))
                        out_sl = relu1[:, fk, rc * 512:(rc + 1) * 512]
                        if fk % 2 == 0:
                            nc.scalar.activation(out_sl, psum, AF.Relu,
                                                 bias=b1_sb[:, fk:fk + 1],
                                                 scale=1.0)
                        else:
                            nc.vector.tensor_scalar(out_sl, psum,
                                                    b1_sb[:, fk:fk + 1], 0.0,
                                                    op0=AL.add, op1=AL.max)
                for of in range(8):
                    w2_t = w2pool.tile([P, 32, P], ff_, tag="w2")
                    nc.sync.dma_start(w2_t, w2_d[of])
                    for rc in range(2):
                        psum = ps_f2.tile([P, 512], f32, tag="f2")
                        if FFN_FP8:
                            for q in range(2):
                                for kp in range(16):
                                    nc.tensor.matmul(
                                        psum[:, q * 256:(q + 1) * 256],
                                        w2_t[:, 2 * kp:2 * kp + 2, :],
                                        relu1[:, 2 * kp:2 * kp + 2,
                                              rc * 512 + q * 256:
                                              rc * 512 + (q + 1) * 256],
                                        perf_mode=DR,
                                        start=(q == 0 and kp == 0),
                                        stop=(kp == 15),
                                        skip_group_check=True)
                        else:
                            for fk in range(32):
                                nc.tensor.matmul(
                                    psum, w2_t[:, fk, :],
                                    relu1[:, fk, rc * 512:(rc + 1) * 512],
                                    start=(fk == 0), stop=(fk == 31))
                        t2 = ypool.tile([P, 512], f32, tag="t2")
                        inv2 = 1.0 / (S1 * S2) if FFN_FP8 else 1.0
                        nc.scalar.activation(t2, psum, AF.Identity,
                                             bias=b2_sb[:, of:of + 1],
                                             scale=inv2)
                        y_sb = ypool.tile([P, 512], f32, tag="y")
                        nc.gpsimd.tensor_tensor(
                            y_sb, t2, x2T[:, of, rc * 512:(rc + 1) * 512],
                            AL.add)
                        nc.sync.dma_start(
                            y_d[of * P:(of + 1) * P,
                                rc * 512:(rc + 1) * 512], y_sb)
    _split_sync_waits(nc)
    return nc


_PROGRAM = None


def _get_program():
    global _PROGRAM
    if _PROGRAM is None:
        _PROGRAM = build_program()
    return _PROGRAM


def _to_bf16(a):
    return np.asarray(a, np.float32).astype(ml_dtypes.bfloat16)


def _to_f8(a):
    return np.asarray(a, np.float32).astype(ml_dtypes.float8_e4m3)


def _host_prep(x, Wk, Wq, Wv, Wproj, bproj, W1, b1, W2, b2, g1, beta1, g2, beta2):
    """Fold LN affine params / V bias into weights (exact); quantize; shard."""
    x = np.asarray(x, np.float32)
    scale = 1.0 / np.sqrt(D)
    Wq_f = (g1[:, None] * np.asarray(Wq, np.float32)) * scale
    bq_f = (beta1 @ np.asarray(Wq, np.float32)) * scale
    Wk_f = g1[:, None] * np.asarray(Wk, np.float32)
    bk_f = beta1 @ np.asarray(Wk, np.float32)
    Wv_f = g1[:, None] * np.asarray(Wv, np.float32)
    bv_f = beta1 @ np.asarray(Wv, np.float32)
    Wp_f = np.asarray(Wproj, np.float32)
    bp_f = np.asarray(bproj, np.float32) + bv_f @ Wp_f
    W1_f = g2[:, None] * np.asarray(W1, np.float32)
    b1_f = np.asarray(b1, np.float32) + beta2 @ np.asarray(W1, np.float32)
    W2_f = np.asarray(W2, np.float32)
    b2_f = np.asarray(b2, np.float32)

    def tile_in_out(W, n_in, n_out, q):
        # [in, out] -> [n_out, 128, n_in, 128]
        return np.ascontiguousarray(q(
            W.reshape(n_in, P, n_out, P).transpose(2, 1, 0, 3)))

    def pko(W, q):
        # [in, out] -> [128, n_in_tiles, out]
        return np.ascontiguousarray(q(
            W.reshape(8, P, W.shape[1]).transpose(1, 0, 2)))

    common = {
        "wq": tile_in_out(Wq_f, 8, 8, _to_bf16),
        "wk": tile_in_out(Wk_f, 8, 8, _to_bf16),
        "wv": pko(Wv_f * (SV if V_FP8 else 1.0),
                  _to_f8 if V_FP8 else _to_bf16),
        "wp": pko(Wp_f * (SP_ if PROJ_FP8 else 1.0),
                  _to_f8 if PROJ_FP8 else _to_bf16),
        "w1": tile_in_out(W1_f * (S1 if FFN_FP8 else 1.0), 8, 32,
                          _to_f8 if FFN_FP8 else _to_bf16),
        "w2": tile_in_out(W2_f * (S2 if FFN_FP8 else 1.0), 32, 8,
                          _to_f8 if FFN_FP8 else _to_bf16),
        "bq": np.ascontiguousarray(bq_f, np.float32),
        "bk": np.ascontiguousarray(bk_f, np.float32),
        "bp": np.ascontiguousarray(bp_f, np.float32),
        "b1": np.ascontiguousarray(b1_f * (S1 if FFN_FP8 else 1.0), np.float32),
        "b2": np.ascontiguousarray(b2_f, np.float32),
        "identr": np.eye(P, dtype=np.float32),
        "identb": np.eye(P, dtype=np.float32).astype(ml_dtypes.bfloat16),
        "onesb": np.ones((P, 16, 16, 1), np.float32).astype(ml_dtypes.bfloat16),
        "onesr": np.ones((P, 1), np.float32),
    }

    in_maps = []
    row_maps = []
    for core in range(8):
        b, s = core // 2, core % 2
        gmap = _chunk_map(s)
        rows = np.concatenate([np.arange(G * P, (G + 1) * P) for G in gmap])
        row_maps.append((b, rows))
        # mask[i, k, slot*128+q]: key pos 256i+128*slot+k vs query pos 128*G+q
        mask = np.empty((8, P, 256), np.float32)
        for i, G in enumerate(gmap):
            kpos = (256 * i + np.arange(256)).reshape(2, P).transpose(1, 0)
            qpos = G * P + np.arange(P)
            m = np.where(kpos[:, :, None] <= qpos[None, None, :], 0.0, NEG)
            mask[i] = m.reshape(P, 2, P).transpose(0, 1, 2).reshape(P, 256)
        in_maps.append({
            "xkv": np.ascontiguousarray(x[b]),
            "xq": np.ascontiguousarray(x[b][rows]),
            "maskc": mask.astype(ml_dtypes.bfloat16),
            **common,
        })
    return in_maps, row_maps


def kernel(**inputs):
    nc = _get_program()
    in_maps, row_maps = _host_prep(**inputs)
    res = run_bass_kernel_spmd(nc, in_maps, core_ids=list(range(8)))
    out = np.empty((B, T, C), np.float32)
    for core in range(8):
        b, rows = row_maps[core]
        out[b][rows] = res.results[core]["y"].T
    return out


# revision 71
# speedup vs baseline: 2.2530x; 1.0073x over previous
"""Trainium2 Bass kernel for one dense transformer block.

Full (unsharded) IO: x [4, 2048, 1024] -> out [4, 2048, 1024].
Sharding: 8 cores = 4 batches x 2 query-chunk sets. Each core owns one
batch's K/V (2048 rows) and 1024 query rows chosen as causally-balanced
128-row chunks (set A: global chunks {4j, 4j+3}, set B: {4j+1, 4j+2}).
Local chunk slot i attends to exactly 2*(i+1) key tiles on every core, so
the instruction stream is identical across cores (SPMD); a per-core data
mask handles the causal boundary in the last two key tiles. No collectives.

On-core dataflow is feature-on-partition ("transposed") throughout. Matmul
dtypes: Q/K projections and scores in bf16 (f32 psum), attention
probabilities bf16, V / output-projection / FFN in fp8-e4m3 with DoubleRow
perf mode (two 128-deep k-tiles per instruction) and power-of-two weight
scaling to avoid fp8 subnormals. LayerNorm runs in f32. Residuals in f32.
LN affine params and biases are folded host-side (exact); V bias is folded
into the projection bias (exact).
"""

import sys

sys.path.insert(0, "/opt/trn_rl_repo")

import numpy as np
import ml_dtypes

import concourse.bass as bass
import concourse.mybir as mybir
import concourse.tile as tile
from concourse.bass_utils import run_bass_kernel_spmd

f32 = mybir.dt.float32
f32r = mybir.dt.float32r
bf16 = mybir.dt.bfloat16
f8 = mybir.dt.float8e4
AL = mybir.AluOpType
AF = mybir.ActivationFunctionType
DR = mybir.MatmulPerfMode.DoubleRow

B, T, C = 4, 2048, 1024
H, D = 16, 64
F = 4 * C
P = 128
TQ = 1024
NEG = -30000.0
LN_EPS = 1e-5

# dtype strategy flags (validated against the 2e-2 rel-err budget)
V_FP8 = True      # V projection via fp8 DoubleRow
PROJ_FP8 = True   # output projection via fp8 DoubleRow
FFN_FP8 = True    # both FFN matmuls via fp8 DoubleRow
SV = 16.0         # host scale on Wv before fp8 cast (carried into OT)
SP_ = 16.0        # host scale on Wproj
S1 = 16.0         # host scale on W1
S2 = 64.0         # host scale on W2

# local chunk slot -> number of key tiles computed (set-independent)
NKT = [2 * (i + 1) for i in range(8)]


def _chunk_map(s):
    """Global 128-row chunk indices owned by set s, in slot order."""
    if s == 0:
        out = []
        for j in range(4):
            out += [4 * j, 4 * j + 3]
        return sorted(out)
    out = []
    for j in range(4):
        out += [4 * j + 1, 4 * j + 2]
    return sorted(out)


def _split_sync_waits(nc):
    """This container's walrus supports one sync-wait per instruction; Tile
    emits up to ~3. Hoist extras onto NoOps inserted before the owner."""
    ctr = 0
    for fn in nc.m.functions:
        for bb in fn.blocks:
            out, changed = [], False
            for ins in bb.instructions:
                si = ins.sync_info
                waits = list(si.on_wait) if si is not None and si.on_wait else []
                if len(waits) > 1:
                    changed = True
                    for w in waits[:-1]:
                        ctr += 1
                        nop = mybir.InstNoOp(name=f"waitsplit_{ctr}", ins=[], outs=[])
                        nop.engine = ins.engine
                        nop.sync_info = mybir.SyncInfo(on_wait=[w], on_update=[])
                        out.append(nop)
                        nc.register_instruction(nop, overwrite=True)
                    ins.sync_info = mybir.SyncInfo(
                        on_wait=[waits[-1]], on_update=list(si.on_update or [])
                    )
                out.append(ins)
            if changed:
                bb.instructions = out


def build_program():
    nc = bass.Bass()
    xkv_d = nc.dram_tensor("xkv", [T, C], f32r, kind="ExternalInput")
    xq_d = nc.dram_tensor("xq", [TQ, C], f32r, kind="ExternalInput")
    mask_d = nc.dram_tensor("maskc", [8, P, 256], bf16, kind="ExternalInput")
    wq_d = nc.dram_tensor("wq", [8, P, 8, P], bf16, kind="ExternalInput")
    wk_d = nc.dram_tensor("wk", [8, P, 8, P], bf16, kind="ExternalInput")
    wv_d = nc.dram_tensor("wv", [P, 8, C], f8 if V_FP8 else bf16,
                          kind="ExternalInput")
    wp_d = nc.dram_tensor("wp", [P, 8, C], f8 if PROJ_FP8 else bf16,
                          kind="ExternalInput")
    w1_d = nc.dram_tensor("w1", [32, P, 8, P], f8 if FFN_FP8 else bf16,
                          kind="ExternalInput")
    w2_d = nc.dram_tensor("w2", [8, P, 32, P], f8 if FFN_FP8 else bf16,
                          kind="ExternalInput")
    bq_d = nc.dram_tensor("bq", [C], f32, kind="ExternalInput")
    bk_d = nc.dram_tensor("bk", [C], f32, kind="ExternalInput")
    bp_d = nc.dram_tensor("bp", [C], f32, kind="ExternalInput")
    b1_d = nc.dram_tensor("b1", [F], f32, kind="ExternalInput")
    b2_d = nc.dram_tensor("b2", [C], f32, kind="ExternalInput")
    identr_d = nc.dram_tensor("identr", [P, P], f32r, kind="ExternalInput")
    identb_d = nc.dram_tensor("identb", [P, P], bf16, kind="ExternalInput")
    onesb_d = nc.dram_tensor("onesb", [P, 16, 16, 1], bf16, kind="ExternalInput")
    onesr_d = nc.dram_tensor("onesr", [P, 1], f32r, kind="ExternalInput")
    y_d = nc.dram_tensor("y", [C, TQ], f32, kind="ExternalOutput")

    fv = f8 if V_FP8 else bf16
    fp = f8 if PROJ_FP8 else bf16
    ff_ = f8 if FFN_FP8 else bf16

    with tile.TileContext(nc) as tc:
      with tc.tile_pool(name="consts", bufs=1) as cpool, \
           tc.tile_pool(name="persist", bufs=1) as pers:
        identr = cpool.tile([P, P], f32r)
        nc.sync.dma_start(identr, identr_d[:, :])
        identb = cpool.tile([P, P], bf16)
        nc.sync.dma_start(identb, identb_d[:, :])
        onesbt = cpool.tile([P, 16, 16, 1], bf16)
        nc.sync.dma_start(onesbt, onesb_d[:, :, :, :])
        onesr = cpool.tile([P, 1], f32r)
        nc.sync.dma_start(onesr, onesr_d[:, :])
        eps128 = cpool.tile([P, 1], f32)
        nc.vector.memset(eps128, LN_EPS)
        eps1 = cpool.tile([1, 1], f32)
        nc.vector.memset(eps1, LN_EPS)
        bq_sb = cpool.tile([P, 8], f32)
        nc.sync.dma_start(bq_sb, bq_d.rearrange("(o p) -> p o", p=P))
        bk_sb = cpool.tile([P, 8], f32)
        nc.sync.dma_start(bk_sb, bk_d.rearrange("(o p) -> p o", p=P))
        bp_sb = cpool.tile([P, 8], f32)
        nc.sync.dma_start(bp_sb, bp_d.rearrange("(o p) -> p o", p=P))
        b1_sb = cpool.tile([P, 32], f32)
        nc.sync.dma_start(b1_sb, b1_d.rearrange("(o p) -> p o", p=P))
        b2_sb = cpool.tile([P, 8], f32)
        nc.sync.dma_start(b2_sb, b2_d.rearrange("(o p) -> p o", p=P))
        mask_sb = cpool.tile([P, 8, 256], bf16)
        for i in range(8):
            nc.sync.dma_start(mask_sb[:, i, :], mask_d[i])

        OT = pers.tile([P, 8, TQ], fp)        # attn out (x SV), transposed
        otq = pers.tile([P, 8, 16, 64], bf16)  # attn out row-major (q, head, d)
        xqT = pers.tile([P, 8, TQ], f32r)     # raw queries, transposed

        # ---------------- Phase A: LN1 + transposes + V + attention --------
        with tc.tile_pool(name="attn_big", bufs=1) as apool:
            hkvT = apool.tile([P, 8, T], bf16)
            hqT = apool.tile([P, 8, TQ], bf16)
            if V_FP8:
                hkv8 = apool.tile([P, 8, T], f8)
            else:
                hkv8 = None
            vaug = apool.tile([P, 16, 16, 65], bf16)
            nc.vector.tensor_copy(vaug[:, :, :, 64:65], onesbt)

            cm_wpool = tc.tile_pool(name="kq_w", bufs=2)
            cm_gpool = tc.tile_pool(name="kq_sb", bufs=2)
            cm_kqv = tc.tile_pool(name="ps_kqv", bufs=2, space="PSUM")
            wpool = cm_wpool.__enter__()
            gpool = cm_gpool.__enter__()
            ps_kqv = cm_kqv.__enter__()

            def emit_k(g, wk_t, KT, rcs):
                for rc in rcs:
                    psum = ps_kqv.tile([P, 512], f32, tag="kqv")
                    for k in range(8):
                        nc.tensor.matmul(
                            psum, wk_t[:, k, :],
                            hkvT[:, k, rc * 512:(rc + 1) * 512],
                            start=(k == 0), stop=(k == 7))
                    if rc % 2 == 0:
                        nc.vector.tensor_scalar_add(
                            KT[:, rc * 512:(rc + 1) * 512], psum,
                            bk_sb[:, g:g + 1])
                    else:
                        nc.scalar.activation(
                            KT[:, rc * 512:(rc + 1) * 512], psum,
                            AF.Identity, bias=bk_sb[:, g:g + 1], scale=1.0)

            def emit_q(g, wq_t, QT, rcs):
                for rc in rcs:
                    psum = ps_kqv.tile([P, 512], f32, tag="kqv")
                    for k in range(8):
                        nc.tensor.matmul(
                            psum, wq_t[:, k, :],
                            hqT[:, k, rc * 512:(rc + 1) * 512],
                            start=(k == 0), stop=(k == 7))
                    if rc % 2 == 0:
                        nc.scalar.activation(
                            QT[:, rc * 512:(rc + 1) * 512], psum,
                            AF.Identity, bias=bq_sb[:, g:g + 1], scale=1.0)
                    else:
                        nc.vector.tensor_scalar_add(
                            QT[:, rc * 512:(rc + 1) * 512], psum,
                            bq_sb[:, g:g + 1])

            with tc.tile_pool(name="ln_sb", bufs=3) as lnsb, \
                 tc.tile_pool(name="ln_small", bufs=6) as lnsm, \
                 tc.tile_pool(name="ps_a", bufs=1, space="PSUM") as ps_a, \
                 tc.tile_pool(name="ps_v", bufs=2, space="PSUM") as ps_v:

                def ln_row_tile(src_ap, rt, is_q):
                    xrow = lnsb.tile([P, C], f32r, tag="xrow")
                    nc.sync.dma_start(xrow, src_ap)
                    stats = lnsm.tile([P, 2, 6], f32, tag="stats")
                    for sg in range(2):
                        nc.vector.bn_stats(stats[:, sg, :],
                                           xrow[:, sg * 512:(sg + 1) * 512])
                    mv = lnsm.tile([P, 2], f32, tag="mv")
                    nc.vector.bn_aggr(mv, stats)
                    rstd = lnsm.tile([P, 1], f32, tag="rstd")
                    nc.scalar.activation(rstd, mv[:, 1:2], AF.Sqrt,
                                         bias=eps128, scale=1.0)
                    nc.vector.reciprocal(rstd, rstd)
                    hrow = lnsb.tile([P, C], bf16, tag="hrow")
                    nc.gpsimd.tensor_scalar(hrow, xrow, mv[:, 0:1], rstd,
                                            op0=AL.subtract, op1=AL.mult)
                    dstT = hqT if is_q else hkvT
                    # one accumulation "start" per 2KB PSUM bank: later
                    # region-disjoint writes use start=False (pending-zero
                    # bytes are overwritten, not accumulated)
                    pstb = ps_a.tile([P, 8, P], bf16, tag="trb")
                    for ft in range(8):
                        nc.tensor.matmul(pstb[:, ft, :],
                                         hrow[:, ft * P:(ft + 1) * P],
                                         identb, is_transpose=True,
                                         start=(ft == 0), stop=True,
                                         skip_group_check=True)
                    if rt % 2 == 0:
                        nc.vector.tensor_copy(dstT[:, :, rt * P:(rt + 1) * P],
                                              pstb)
                        if not is_q and V_FP8:
                            nc.scalar.copy(hkv8[:, :, rt * P:(rt + 1) * P],
                                           pstb)
                    else:
                        nc.scalar.copy(dstT[:, :, rt * P:(rt + 1) * P], pstb)
                        if not is_q and V_FP8:
                            nc.vector.tensor_copy(
                                hkv8[:, :, rt * P:(rt + 1) * P], pstb)
                    if is_q:
                        pstr = ps_a.tile([P, 8, P], f32r, tag="trr")
                        for ft in range(8):
                            nc.tensor.matmul(pstr[:, ft, :],
                                             xrow[:, ft * P:(ft + 1) * P],
                                             identr, is_transpose=True,
                                             start=(ft % 4 == 0), stop=True,
                                             skip_group_check=True)
                        nc.scalar.copy(xqT[:, :, rt * P:(rt + 1) * P], pstr)

                def emit_v(tc_i):
                    for fh in range(2):
                        psv = ps_v.tile([P, 8, 64], f32, tag="v")
                        if V_FP8:
                            for q in range(2):
                                for kp in range(4):
                                    nc.tensor.matmul(
                                        psv[:, q * 4:(q + 1) * 4, :],
                                        hkv8[:, 2 * kp:2 * kp + 2,
                                             tc_i * P:(tc_i + 1) * P],
                                        wv_sb[:, 2 * kp:2 * kp + 2,
                                              fh * 512 + q * 256:
                                              fh * 512 + (q + 1) * 256],
                                        perf_mode=DR,
                                        start=(q == 0 and kp == 0),
                                        stop=(kp == 3),
                                        skip_group_check=True)
                        else:
                            for k in range(8):
                                nc.tensor.matmul(
                                    psv[:, :, :],
                                    hkvT[:, k, tc_i * P:(tc_i + 1) * P],
                                    wv_sb[:, k, fh * 512:(fh + 1) * 512],
                                    start=(k == 0), stop=(k == 7))
                        if (tc_i + fh) % 2 == 0:
                            nc.vector.tensor_copy(
                                vaug[:, tc_i, fh * 8:(fh + 1) * 8, 0:64], psv)
                        else:
                            nc.scalar.copy(
                                vaug[:, tc_i, fh * 8:(fh + 1) * 8, 0:64], psv)

                # software-pipelined emission: the engine queues are in-order,
                # so interleave V and the first two groups' K/Q projections
                # between LN tiles to keep PE fed during the DVE-bound LN.
                wv_sb = apool.tile([P, 8, C], fv)
                kq_early = []
                for g in (0, 1):
                    wk_t = wpool.tile([P, 8, P], bf16, tag="wk")
                    wq_t = wpool.tile([P, 8, P], bf16, tag="wq")
                    KT = gpool.tile([P, T], bf16, tag="KT")
                    QT = gpool.tile([P, TQ], bf16, tag="QT")
                    kq_early.append((wk_t, wq_t, KT, QT))
                for rt in range(16):
                    ln_row_tile(xkv_d[rt * P:(rt + 1) * P, :], rt, False)
                    if rt == 0:
                        nc.sync.dma_start(wv_sb, wv_d[:, :, :])
                    elif rt == 1:
                        nc.sync.dma_start(kq_early[0][0], wk_d[0])
                        nc.sync.dma_start(kq_early[0][1], wq_d[0])
                    elif rt == 2:
                        nc.sync.dma_start(kq_early[1][0], wk_d[1])
                        nc.sync.dma_start(kq_early[1][1], wq_d[1])
                    if rt >= 2:
                        emit_v(rt - 2)
                    if rt % 4 == 3:
                        emit_k(0, kq_early[0][0], kq_early[0][2], [rt // 4])
                for rt in range(8):
                    ln_row_tile(xq_d[rt * P:(rt + 1) * P, :], rt, True)
                    if rt < 2:
                        emit_v(14 + rt)
                    if rt % 2 == 1:
                        emit_k(1, kq_early[1][0], kq_early[1][2], [rt // 2])
                emit_q(0, kq_early[0][1], kq_early[0][3], [0, 1])
                emit_q(1, kq_early[1][1], kq_early[1][3], [0, 1])

            # -------- per-group K/Q projection + attention --------
            with tc.tile_pool(name="pt_sb", bufs=2) as ptpool, \
                 tc.tile_pool(name="sm_sb", bufs=3) as smpool, \
                 tc.tile_pool(name="ps_sc", bufs=2, space="PSUM") as ps_sc, \
                 tc.tile_pool(name="ps_ob", bufs=2, space="PSUM") as ps_ob:
                for g in range(8):
                    if g < 2:
                        _, _, KT, QT = kq_early[g]
                    else:
                        wk_t = wpool.tile([P, 8, P], bf16, tag="wk")
                        nc.sync.dma_start(wk_t, wk_d[g])
                        wq_t = wpool.tile([P, 8, P], bf16, tag="wq")
                        nc.sync.dma_start(wq_t, wq_d[g])
                        KT = gpool.tile([P, T], bf16, tag="KT")
                        emit_k(g, wk_t, KT, range(4))
                        QT = gpool.tile([P, TQ], bf16, tag="QT")
                        emit_q(g, wq_t, QT, range(2))

                    for hh in range(2):
                        base = 64 * hh
                        for lc in range(8):
                            nkt = NKT[lc]
                            ptb = ptpool.tile([P, 16, P], bf16, tag="ptb")
                            ob = ps_ob.tile([P, 65], f32, tag="ob")
                            for sg in range((nkt + 7) // 8):
                                n_in_g = min(8, nkt - sg * 8)
                                ps_s = ps_sc.tile([P, 8, P], f32, tag="sc")
                                for sl in range(n_in_g):
                                    kt = sg * 8 + sl
                                    last2 = (kt >= nkt - 2)
                                    nc.tensor.matmul(
                                        ps_s[:, sl, :],
                                        KT[base:base + 64, kt * P:(kt + 1) * P],
                                        QT[base:base + 64, lc * P:(lc + 1) * P],
                                        start=(sl % 4 == 0), stop=not last2,
                                        skip_group_check=True)
                                # causal mask on the last two key tiles
                                sl0 = (nkt - 2) % 8
                                if sg == (nkt - 1) // 8:
                                    nc.tensor.matmul(
                                        ps_s[:, sl0:sl0 + 2, :],
                                        identb, mask_sb[:, lc, :],
                                        start=False, stop=True,
                                        skip_group_check=True)
                                nc.scalar.activation(
                                    ptb[:, sg * 8:sg * 8 + n_in_g, :],
                                    ps_s[:, 0:n_in_g, :], AF.Exp)
                            h_idx = 2 * g + hh
                            for kt in range(nkt):
                                # out[q, d] = sum_k pt[k, q] * vaug[k, d];
                                # column 64 accumulates the softmax denom
                                nc.tensor.matmul(
                                    ob, ptb[:, kt, :],
                                    vaug[:, kt, h_idx, :],
                                    start=(kt == 0), stop=(kt == nkt - 1))
                            rec = smpool.tile([P, 1], f32, tag="rec")
                            nc.vector.reciprocal(rec, ob[:, 64:65])
                            nc.vector.tensor_scalar_mul(
                                otq[:, lc, h_idx, :], ob[:, 0:64], rec)

            cm_kqv.__exit__(None, None, None)
            cm_gpool.__exit__(None, None, None)
            cm_wpool.__exit__(None, None, None)

        # transpose attn out back to feature-on-partition for proj
        with tc.tile_pool(name="ps_trq", bufs=2, space="PSUM") as ps_trq:
            for lc in range(8):
                pq = ps_trq.tile([P, 8, P], bf16, tag="trq")
                for ft in range(8):
                    nc.tensor.matmul(
                        pq[:, ft, :], otq[:, lc, 2 * ft:2 * ft + 2, :],
                        identb, is_transpose=True, start=(ft == 0), stop=True,
                        skip_group_check=True)
                if lc % 2 == 0:
                    nc.vector.tensor_copy(OT[:, :, lc * P:(lc + 1) * P], pq)
                else:
                    nc.scalar.copy(OT[:, :, lc * P:(lc + 1) * P], pq)

        # ---------------- Phase B: proj + residual + LN2 + FFN -------------
        with tc.tile_pool(name="late", bufs=1) as late:
            x2T = late.tile([P, 8, TQ], f32r)
            h2 = late.tile([P, 8, TQ], ff_)
            with tc.tile_pool(name="pr_w", bufs=1) as prpool, \
                 tc.tile_pool(name="pr_tmp", bufs=3) as prtmp, \
                 tc.tile_pool(name="ps_pr", bufs=3, space="PSUM") as ps_pr:
                wp_sb = prpool.tile([P, 8, C], fp)
                nc.sync.dma_start(wp_sb, wp_d[:, :, :])
                # OT carries SV from the V-weight scaling; wp carries SP_
                inv = 1.0 / ((SV if V_FP8 else 1.0) * (SP_ if PROJ_FP8 else 1.0))
                for rc in range(4):
                    for of in range(8):
                        psum = ps_pr.tile([P, 256], f32, tag="pr")
                        if PROJ_FP8:
                            for kp in range(4):
                                nc.tensor.matmul(
                                    psum,
                                    wp_sb[:, 2 * kp:2 * kp + 2,
                                          of * P:(of + 1) * P],
                                    OT[:, 2 * kp:2 * kp + 2,
                                       rc * 256:(rc + 1) * 256],
                                    perf_mode=DR,
                                    start=(kp == 0), stop=(kp == 3))
                        else:
                            for k in range(8):
                                nc.tensor.matmul(
                                    psum, wp_sb[:, k, of * P:(of + 1) * P],
                                    OT[:, k, rc * 256:(rc + 1) * 256],
                                    start=(k == 0), stop=(k == 7))
                        t1 = prtmp.tile([P, 256], f32, tag="t1")
                        nc.scalar.activation(t1, psum, AF.Identity,
                                             bias=bp_sb[:, of:of + 1],
                                             scale=inv)
                        nc.gpsimd.tensor_tensor(
                            x2T[:, of, rc * 256:(rc + 1) * 256], t1,
                            xqT[:, of, rc * 256:(rc + 1) * 256], AL.add)

            # LN2 in transposed layout
            with tc.tile_pool(name="ln2_sb", bufs=4) as ln2sb, \
                 tc.tile_pool(name="ln2_sm", bufs=4) as ln2sm, \
                 tc.tile_pool(name="ps_l2s", bufs=2, space="PSUM") as ps_l2s, \
                 tc.tile_pool(name="ps_l2b", bufs=2, space="PSUM") as ps_l2b:
                for rc in range(2):
                    psum_m = ps_l2s.tile([1, 512], f32, tag="m")
                    psum_q = ps_l2s.tile([1, 512], f32, tag="q")
                    for ft in range(8):
                        xt = x2T[:, ft, rc * 512:(rc + 1) * 512]
                        sq = ln2sb.tile([P, 512], f32r, tag="sq")
                        nc.scalar.square(sq, xt)
                        nc.tensor.matmul(psum_m, onesr, xt,
                                         start=(ft == 0), stop=(ft == 7))
                        nc.tensor.matmul(psum_q, onesr, sq,
                                         start=(ft == 0), stop=(ft == 7))
                    mean = ln2sm.tile([1, 512], f32, tag="mean")
                    nc.vector.tensor_scalar_mul(mean, psum_m, 1.0 / C)
                    msq = ln2sm.tile([1, 512], f32, tag="msq")
                    nc.vector.tensor_scalar_mul(msq, psum_q, 1.0 / C)
                    var = ln2sm.tile([1, 512], f32, tag="var")
                    nc.vector.tensor_tensor(var, mean, mean, AL.mult)
                    nc.vector.tensor_tensor(var, msq, var, AL.subtract)
                    std = ln2sm.tile([1, 512], f32, tag="std")
                    nc.scalar.activation(std, var, AF.Sqrt,
                                         bias=eps1[:, :], scale=1.0)
                    a = ln2sm.tile([1, 512], f32r, tag="a")
                    with nc.allow_low_precision(reason="f32r has f32 bits"):
                        nc.vector.reciprocal(a, std)
                    bneg = ln2sm.tile([1, 512], f32r, tag="b")
                    nc.vector.tensor_tensor(bneg, mean, a, AL.mult)
                    ps_abc = ps_l2b.tile([P, 512], f32, tag="abc")
                    nc.tensor.matmul(ps_abc,
                                     onesr[0:1, 0:1].broadcast_to((1, P)), a,
                                     start=True, stop=True)
                    ps_bbc = ps_l2b.tile([P, 512], f32, tag="bbc")
                    nc.tensor.matmul(ps_bbc,
                                     onesr[0:1, 0:1].broadcast_to((1, P)),
                                     bneg, start=True, stop=True)
                    # SBUF copies of the broadcasts so gpsimd (no PSUM
                    # access) can share the normalize work with DVE
                    abc_sb = ln2sm.tile([P, 512], f32r, tag="abcs")
                    nc.scalar.copy(abc_sb, ps_abc)
                    bbc_sb = ln2sm.tile([P, 512], f32r, tag="bbcs")
                    nc.scalar.copy(bbc_sb, ps_bbc)
                    for ft in range(8):
                        tmp = ln2sb.tile([P, 512], f32r, tag="tmp")
                        if ft % 2 == 0:
                            nc.vector.tensor_tensor(
                                tmp, x2T[:, ft, rc * 512:(rc + 1) * 512],
                                ps_abc, AL.mult)
                            nc.vector.tensor_tensor(
                                h2[:, ft, rc * 512:(rc + 1) * 512], tmp,
                                ps_bbc, AL.subtract)
                        else:
                            nc.gpsimd.tensor_tensor(
                                tmp, x2T[:, ft, rc * 512:(rc + 1) * 512],
                                abc_sb, AL.mult)
                            nc.gpsimd.tensor_tensor(
                                h2[:, ft, rc * 512:(rc + 1) * 512], tmp,
                                bbc_sb, AL.subtract)

            # FFN
            with tc.tile_pool(name="f_w1", bufs=3) as w1pool, \
                 tc.tile_pool(name="f_w2", bufs=2) as w2pool, \
                 tc.tile_pool(name="f_relu", bufs=1) as rpool, \
                 tc.tile_pool(name="f_y", bufs=3) as ypool, \
                 tc.tile_pool(name="ps_f1", bufs=4, space="PSUM") as ps_f1, \
                 tc.tile_pool(name="ps_f2", bufs=4, space="PSUM") as ps_f2:
                relu1 = rpool.tile([P, 32, TQ], ff_)
                for fk in range(32):
                    w1_t = w1pool.tile([P, 8, P], ff_, tag="w1")
                    nc.sync.dma_start(w1_t, w1_d[fk])
                    for rc in range(2):
                        psum = ps_f1.tile([P, 512], f32, tag="f1")
                        if FFN_FP8:
                            for q in range(2):
                                for kp in range(4):
                                    nc.tensor.matmul(
                                        psum[:, q * 256:(q + 1) * 256],
                                        w1_t[:, 2 * kp:2 * kp + 2, :],
                                        h2[:, 2 * kp:2 * kp + 2,
                                           rc * 512 + q * 256:
                                           rc * 512 + (q + 1) * 256],
                                        perf_mode=DR,
                                        start=(q == 0 and kp == 0),
                                        stop=(kp == 3),
                                        skip_group_check=True)
                        else:
                            for k in range(8):
                                nc.tensor.matmul(
                                    psum, w1_t[:, k, :],
                                    h2[:, k, rc * 512:(rc + 1) * 512],
                                    start=(k == 0), stop=(k == 7))
                        out_sl = relu1[:, fk, rc * 512:(rc + 1) * 512]
                        if fk % 2 == 0:
                            nc.scalar.activation(out_sl, psum, AF.Relu,
                                                 bias=b1_sb[:, fk:fk + 1],
                                                 scale=1.0)
                        else:
                            nc.vector.tensor_scalar(out_sl, psum,
                                                    b1_sb[:, fk:fk + 1], 0.0,
                                                    op0=AL.add, op1=AL.max)
                for of in range(8):
                    w2_t = w2pool.tile([P, 32, P], ff_, tag="w2")
                    nc.sync.dma_start(w2_t, w2_d[of])
                    for rc in range(2):
                        psum = ps_f2.tile([P, 512], f32, tag="f2")
                        if FFN_FP8:
                            for q in range(2):
                                for kp in range(16):
                                    nc.tensor.matmul(
                                        psum[:, q * 256:(q + 1) * 256],
                                        w2_t[:, 2 * kp:2 * kp + 2, :],
                                        relu1[:, 2 * kp:2 * kp + 2,
                                              rc * 512 + q * 256:
                                              rc * 512 + (q + 1) * 256],
                                        perf_mode=DR,
                                        start=(q == 0 and kp == 0),
                                        stop=(kp == 15),
                                        skip_group_check=True)
                        else:
                            for fk in range(32):
                                nc.tensor.matmul(
                                    psum, w2_t[:, fk, :],
                                    relu1[:, fk, rc * 512:(rc + 1) * 512],
                                    start=(fk == 0), stop=(fk == 31))
                        t2 = ypool.tile([P, 512], f32, tag="t2")
                        inv2 = 1.0 / (S1 * S2) if FFN_FP8 else 1.0
                        nc.scalar.activation(t2, psum, AF.Identity,
                                             bias=b2_sb[:, of:of + 1],
                                             scale=inv2)
                        y_sb = ypool.tile([P, 512], f32, tag="y")
                        nc.gpsimd.tensor_tensor(
                            y_sb, t2, x2T[:, of, rc * 512:(rc + 1) * 512],
                            AL.add)
                        nc.sync.dma_start(
                            y_d[of * P:(of + 1) * P,
                                rc * 512:(rc + 1) * 512], y_sb)
    _split_sync_waits(nc)
    return nc


_PROGRAM = None


def _get_program():
    global _PROGRAM
    if _PROGRAM is None:
        _PROGRAM = build_program()
    return _PROGRAM


def _to_bf16(a):
    return np.asarray(a, np.float32).astype(ml_dtypes.bfloat16)


def _to_f8(a):
    return np.asarray(a, np.float32).astype(ml_dtypes.float8_e4m3)


def _host_prep(x, Wk, Wq, Wv, Wproj, bproj, W1, b1, W2, b2, g1, beta1, g2, beta2):
    """Fold LN affine params / V bias into weights (exact); quantize; shard."""
    x = np.asarray(x, np.float32)
    scale = 1.0 / np.sqrt(D)
    Wq_f = (g1[:, None] * np.asarray(Wq, np.float32)) * scale
    bq_f = (beta1 @ np.asarray(Wq, np.float32)) * scale
    Wk_f = g1[:, None] * np.asarray(Wk, np.float32)
    bk_f = beta1 @ np.asarray(Wk, np.float32)
    Wv_f = g1[:, None] * np.asarray(Wv, np.float32)
    bv_f = beta1 @ np.asarray(Wv, np.float32)
    Wp_f = np.asarray(Wproj, np.float32)
    bp_f = np.asarray(bproj, np.float32) + bv_f @ Wp_f
    W1_f = g2[:, None] * np.asarray(W1, np.float32)
    b1_f = np.asarray(b1, np.float32) + beta2 @ np.asarray(W1, np.float32)
    W2_f = np.asarray(W2, np.float32)
    b2_f = np.asarray(b2, np.float32)

    def tile_in_out(W, n_in, n_out, q):
        # [in, out] -> [n_out, 128, n_in, 128]
        return np.ascontiguousarray(q(
            W.reshape(n_in, P, n_out, P).transpose(2, 1, 0, 3)))

    def pko(W, q):
        # [in, out] -> [128, n_in_tiles, out]
        return np.ascontiguousarray(q(
            W.reshape(8, P, W.shape[1]).transpose(1, 0, 2)))

    common = {
        "wq": tile_in_out(Wq_f, 8, 8, _to_bf16),
        "wk": tile_in_out(Wk_f, 8, 8, _to_bf16),
        "wv": pko(Wv_f * (SV if V_FP8 else 1.0),
                  _to_f8 if V_FP8 else _to_bf16),
        "wp": pko(Wp_f * (SP_ if PROJ_FP8 else 1.0),
                  _to_f8 if PROJ_FP8 else _to_bf16),
        "w1": tile_in_out(W1_f * (S1 if FFN_FP8 else 1.0), 8, 32,
                          _to_f8 if FFN_FP8 else _to_bf16),
        "w2": tile_in_out(W2_f * (S2 if FFN_FP8 else 1.0), 32, 8,
                          _to_f8 if FFN_FP8 else _to_bf16),
        "bq": np.ascontiguousarray(bq_f, np.float32),
        "bk": np.ascontiguousarray(bk_f, np.float32),
        "bp": np.ascontiguousarray(bp_f, np.float32),
        "b1": np.ascontiguousarray(b1_f * (S1 if FFN_FP8 else 1.0), np.float32),
        "b2": np.ascontiguousarray(b2_f, np.float32),
        "identr": np.eye(P, dtype=np.float32),
        "identb": np.eye(P, dtype=np.float32).astype(ml_dtypes.bfloat16),
        "onesb": np.ones((P, 16, 16, 1), np.float32).astype(ml_dtypes.bfloat16),
        "onesr": np.ones((P, 1), np.float32),
    }

    in_maps = []
    row_maps = []
    for core in range(8):
        b, s = core // 2, core % 2
        gmap = _chunk_map(s)
        rows = np.concatenate([np.arange(G * P, (G + 1) * P) for G in gmap])
        row_maps.append((b, rows))
        # mask[i, k, slot*128+q]: key pos 256i+128*slot+k vs query pos 128*G+q
        mask = np.empty((8, P, 256), np.float32)
        for i, G in enumerate(gmap):
            kpos = (256 * i + np.arange(256)).reshape(2, P).transpose(1, 0)
            qpos = G * P + np.arange(P)
            m = np.where(kpos[:, :, None] <= qpos[None, None, :], 0.0, NEG)
            mask[i] = m.reshape(P, 2, P).transpose(0, 1, 2).reshape(P, 256)
        in_maps.append({
            "xkv": np.ascontiguousarray(x[b]),
            "xq": np.ascontiguousarray(x[b][rows]),
            "maskc": mask.astype(ml_dtypes.bfloat16),
            **common,
        })
    return in_maps, row_maps


def kernel(**inputs):
    nc = _get_program()
    in_maps, row_maps = _host_prep(**inputs)
    res = run_bass_kernel_spmd(nc, in_maps, core_ids=list(range(8)))
    out = np.empty((B, T, C), np.float32)
    for core in range(8):
        b, rows = row_maps[core]
        out[b][rows] = res.results[core]["y"].T
    return out


# revision 81
# speedup vs baseline: 2.2656x; 1.0056x over previous
"""Trainium2 Bass kernel for one dense transformer block.

Full (unsharded) IO: x [4, 2048, 1024] -> out [4, 2048, 1024].
Sharding: 8 cores = 4 batches x 2 query-chunk sets. Each core owns one
batch's K/V (2048 rows) and 1024 query rows chosen as causally-balanced
128-row chunks (set A: global chunks {4j, 4j+3}, set B: {4j+1, 4j+2}).
Local chunk slot i attends to exactly 2*(i+1) key tiles on every core, so
the instruction stream is identical across cores (SPMD); a per-core data
mask handles the causal boundary in the last two key tiles. No collectives.

On-core dataflow is feature-on-partition ("transposed") throughout. Matmul
dtypes: Q/K projections and scores in bf16 (f32 psum), attention
probabilities bf16, V / output-projection / FFN in fp8-e4m3 with DoubleRow
perf mode (two 128-deep k-tiles per instruction) and power-of-two weight
scaling to avoid fp8 subnormals. LayerNorm runs in f32. Residuals in f32.
LN affine params and biases are folded host-side (exact); V bias is folded
into the projection bias (exact).
"""

import sys

sys.path.insert(0, "/opt/trn_rl_repo")

import numpy as np
import ml_dtypes

import concourse.bass as bass
import concourse.mybir as mybir
import concourse.tile as tile
from concourse.bass_utils import run_bass_kernel_spmd

f32 = mybir.dt.float32
f32r = mybir.dt.float32r
bf16 = mybir.dt.bfloat16
f8 = mybir.dt.float8e4
AL = mybir.AluOpType
AF = mybir.ActivationFunctionType
DR = mybir.MatmulPerfMode.DoubleRow

B, T, C = 4, 2048, 1024
H, D = 16, 64
F = 4 * C
P = 128
TQ = 1024
NEG = -30000.0
LN_EPS = 1e-5

# dtype strategy flags (validated against the 2e-2 rel-err budget)
V_FP8 = True      # V projection via fp8 DoubleRow
PROJ_FP8 = True   # output projection via fp8 DoubleRow
FFN_FP8 = True    # both FFN matmuls via fp8 DoubleRow
SV = 16.0         # host scale on Wv before fp8 cast (carried into OT)
SP_ = 16.0        # host scale on Wproj
S1 = 16.0         # host scale on W1
S2 = 64.0         # host scale on W2

# local chunk slot -> number of key tiles computed (set-independent)
NKT = [2 * (i + 1) for i in range(8)]


def _chunk_map(s):
    """Global 128-row chunk indices owned by set s, in slot order."""
    if s == 0:
        out = []
        for j in range(4):
            out += [4 * j, 4 * j + 3]
        return sorted(out)
    out = []
    for j in range(4):
        out += [4 * j + 1, 4 * j + 2]
    return sorted(out)


def _split_sync_waits(nc):
    """This container's walrus supports one sync-wait per instruction; Tile
    emits up to ~3. Hoist extras onto NoOps inserted before the owner."""
    ctr = 0
    for fn in nc.m.functions:
        for bb in fn.blocks:
            out, changed = [], False
            for ins in bb.instructions:
                si = ins.sync_info
                waits = list(si.on_wait) if si is not None and si.on_wait else []
                if len(waits) > 1:
                    changed = True
                    for w in waits[:-1]:
                        ctr += 1
                        nop = mybir.InstNoOp(name=f"waitsplit_{ctr}", ins=[], outs=[])
                        nop.engine = ins.engine
                        nop.sync_info = mybir.SyncInfo(on_wait=[w], on_update=[])
                        out.append(nop)
                        nc.register_instruction(nop, overwrite=True)
                    ins.sync_info = mybir.SyncInfo(
                        on_wait=[waits[-1]], on_update=list(si.on_update or [])
                    )
                out.append(ins)
            if changed:
                bb.instructions = out


def build_program():
    nc = bass.Bass()
    xkv_d = nc.dram_tensor("xkv", [T, C], f32r, kind="ExternalInput")
    xq_d = nc.dram_tensor("xq", [TQ, C], f32r, kind="ExternalInput")
    mask_d = nc.dram_tensor("maskc", [8, P, 256], bf16, kind="ExternalInput")
    wq_d = nc.dram_tensor("wq", [8, P, 8, P], bf16, kind="ExternalInput")
    wk_d = nc.dram_tensor("wk", [8, P, 8, P], bf16, kind="ExternalInput")
    wv_d = nc.dram_tensor("wv", [P, 8, C], f8 if V_FP8 else bf16,
                          kind="ExternalInput")
    wp_d = nc.dram_tensor("wp", [P, 8, C], f8 if PROJ_FP8 else bf16,
                          kind="ExternalInput")
    w1_d = nc.dram_tensor("w1", [32, P, 8, P], f8 if FFN_FP8 else bf16,
                          kind="ExternalInput")
    w2_d = nc.dram_tensor("w2", [8, P, 32, P], f8 if FFN_FP8 else bf16,
                          kind="ExternalInput")
    bq_d = nc.dram_tensor("bq", [C], f32, kind="ExternalInput")
    bk_d = nc.dram_tensor("bk", [C], f32, kind="ExternalInput")
    bp_d = nc.dram_tensor("bp", [C], f32, kind="ExternalInput")
    b1_d = nc.dram_tensor("b1", [F], f32, kind="ExternalInput")
    b2_d = nc.dram_tensor("b2", [C], f32, kind="ExternalInput")
    identr_d = nc.dram_tensor("identr", [P, P], f32r, kind="ExternalInput")
    identb_d = nc.dram_tensor("identb", [P, P], bf16, kind="ExternalInput")
    onesb_d = nc.dram_tensor("onesb", [P, 16, 16, 1], bf16, kind="ExternalInput")
    onesr_d = nc.dram_tensor("onesr", [P, 1], f32r, kind="ExternalInput")
    y_d = nc.dram_tensor("y", [C, TQ], f32, kind="ExternalOutput")

    fv = f8 if V_FP8 else bf16
    fp = f8 if PROJ_FP8 else bf16
    ff_ = f8 if FFN_FP8 else bf16

    with tile.TileContext(nc) as tc:
      with tc.tile_pool(name="consts", bufs=1) as cpool, \
           tc.tile_pool(name="persist", bufs=1) as pers:
        # only identb (first LN transposes) loads before the x rows; the rest
        # of the constants are deferred into the LN stream so they don't
        # serialize in front of the first x-row DMA
        identb = cpool.tile([P, P], bf16)
        nc.sync.dma_start(identb, identb_d[:, :])
        identr = cpool.tile([P, P], f32r)
        onesbt = cpool.tile([P, 16, 16, 1], bf16)
        onesr = cpool.tile([P, 1], f32r)
        eps128 = cpool.tile([P, 1], f32)
        nc.vector.memset(eps128, LN_EPS)
        eps1 = cpool.tile([1, 1], f32)
        nc.vector.memset(eps1, LN_EPS)
        bq_sb = cpool.tile([P, 8], f32)
        bk_sb = cpool.tile([P, 8], f32)
        bp_sb = cpool.tile([P, 8], f32)
        b1_sb = cpool.tile([P, 32], f32)
        b2_sb = cpool.tile([P, 8], f32)
        mask_sb = cpool.tile([P, 8, 256], bf16)

        def emit_deferred_consts():
            nc.sync.dma_start(bq_sb, bq_d.rearrange("(o p) -> p o", p=P))
            nc.sync.dma_start(bk_sb, bk_d.rearrange("(o p) -> p o", p=P))
            nc.sync.dma_start(identr, identr_d[:, :])
            nc.sync.dma_start(onesbt, onesb_d[:, :, :, :])
            for i in range(8):
                nc.sync.dma_start(mask_sb[:, i, :], mask_d[i])
            nc.sync.dma_start(bp_sb, bp_d.rearrange("(o p) -> p o", p=P))
            nc.sync.dma_start(b1_sb, b1_d.rearrange("(o p) -> p o", p=P))
            nc.sync.dma_start(b2_sb, b2_d.rearrange("(o p) -> p o", p=P))
            nc.sync.dma_start(onesr, onesr_d[:, :])

        OT = pers.tile([P, 8, TQ], fp)        # attn out (x SV), transposed
        otq = pers.tile([P, 8, 16, 64], bf16)  # attn out row-major (q, head, d)
        xqT = pers.tile([P, 8, TQ], f32r)     # raw queries, transposed

        # ---------------- Phase A: LN1 + transposes + V + attention --------
        with tc.tile_pool(name="attn_big", bufs=1) as apool:
            hkvT = apool.tile([P, 8, T], bf16)
            hqT = apool.tile([P, 8, TQ], bf16)
            if V_FP8:
                hkv8 = apool.tile([P, 8, T], f8)
            else:
                hkv8 = None
            vaug = apool.tile([P, 16, 16, 65], bf16)

            cm_wpool = tc.tile_pool(name="kq_w", bufs=2)
            cm_gpool = tc.tile_pool(name="kq_sb", bufs=2)
            cm_kqv = tc.tile_pool(name="ps_kqv", bufs=2, space="PSUM")
            wpool = cm_wpool.__enter__()
            gpool = cm_gpool.__enter__()
            ps_kqv = cm_kqv.__enter__()

            def emit_k(g, wk_t, KT, rcs):
                for rc in rcs:
                    psum = ps_kqv.tile([P, 512], f32, tag="kqv")
                    for k in range(8):
                        nc.tensor.matmul(
                            psum, wk_t[:, k, :],
                            hkvT[:, k, rc * 512:(rc + 1) * 512],
                            start=(k == 0), stop=(k == 7))
                    if rc % 2 == 0:
                        nc.vector.tensor_scalar_add(
                            KT[:, rc * 512:(rc + 1) * 512], psum,
                            bk_sb[:, g:g + 1])
                    else:
                        nc.scalar.activation(
                            KT[:, rc * 512:(rc + 1) * 512], psum,
                            AF.Identity, bias=bk_sb[:, g:g + 1], scale=1.0)

            def emit_q(g, wq_t, QT, rcs):
                for rc in rcs:
                    psum = ps_kqv.tile([P, 512], f32, tag="kqv")
                    for k in range(8):
                        nc.tensor.matmul(
                            psum, wq_t[:, k, :],
                            hqT[:, k, rc * 512:(rc + 1) * 512],
                            start=(k == 0), stop=(k == 7))
                    if rc % 2 == 0:
                        nc.scalar.activation(
                            QT[:, rc * 512:(rc + 1) * 512], psum,
                            AF.Identity, bias=bq_sb[:, g:g + 1], scale=1.0)
                    else:
                        nc.vector.tensor_scalar_add(
                            QT[:, rc * 512:(rc + 1) * 512], psum,
                            bq_sb[:, g:g + 1])

            with tc.tile_pool(name="ln_sb", bufs=3) as lnsb, \
                 tc.tile_pool(name="ln_small", bufs=6) as lnsm, \
                 tc.tile_pool(name="ps_a", bufs=1, space="PSUM") as ps_a, \
                 tc.tile_pool(name="ps_v", bufs=2, space="PSUM") as ps_v:

                def ln_row_tile(src_ap, rt, is_q):
                    xrow = lnsb.tile([P, C], f32r, tag="xrow")
                    nc.sync.dma_start(xrow, src_ap)
                    stats = lnsm.tile([P, 2, 6], f32, tag="stats")
                    for sg in range(2):
                        nc.vector.bn_stats(stats[:, sg, :],
                                           xrow[:, sg * 512:(sg + 1) * 512])
                    mv = lnsm.tile([P, 2], f32, tag="mv")
                    nc.vector.bn_aggr(mv, stats)
                    rstd = lnsm.tile([P, 1], f32, tag="rstd")
                    nc.scalar.activation(rstd, mv[:, 1:2], AF.Sqrt,
                                         bias=eps128, scale=1.0)
                    nc.vector.reciprocal(rstd, rstd)
                    hrow = lnsb.tile([P, C], bf16, tag="hrow")
                    nc.gpsimd.tensor_scalar(hrow, xrow, mv[:, 0:1], rstd,
                                            op0=AL.subtract, op1=AL.mult)
                    dstT = hqT if is_q else hkvT
                    # one accumulation "start" per 2KB PSUM bank: later
                    # region-disjoint writes use start=False (pending-zero
                    # bytes are overwritten, not accumulated)
                    pstb = ps_a.tile([P, 8, P], bf16, tag="trb")
                    for ft in range(8):
                        nc.tensor.matmul(pstb[:, ft, :],
                                         hrow[:, ft * P:(ft + 1) * P],
                                         identb, is_transpose=True,
                                         start=(ft == 0), stop=True,
                                         skip_group_check=True)
                    if rt % 2 == 0:
                        nc.vector.tensor_copy(dstT[:, :, rt * P:(rt + 1) * P],
                                              pstb)
                        if not is_q and V_FP8:
                            nc.scalar.copy(hkv8[:, :, rt * P:(rt + 1) * P],
                                           pstb)
                    else:
                        nc.scalar.copy(dstT[:, :, rt * P:(rt + 1) * P], pstb)
                        if not is_q and V_FP8:
                            nc.vector.tensor_copy(
                                hkv8[:, :, rt * P:(rt + 1) * P], pstb)
                    if is_q:
                        pstr = ps_a.tile([P, 8, P], f32r, tag="trr")
                        for ft in range(8):
                            nc.tensor.matmul(pstr[:, ft, :],
                                             xrow[:, ft * P:(ft + 1) * P],
                                             identr, is_transpose=True,
                                             start=(ft % 4 == 0), stop=True,
                                             skip_group_check=True)
                        nc.scalar.copy(xqT[:, :, rt * P:(rt + 1) * P], pstr)

                def emit_v(tc_i):
                    for fh in range(2):
                        psv = ps_v.tile([P, 8, 64], f32, tag="v")
                        if V_FP8:
                            for q in range(2):
                                for kp in range(4):
                                    nc.tensor.matmul(
                                        psv[:, q * 4:(q + 1) * 4, :],
                                        hkv8[:, 2 * kp:2 * kp + 2,
                                             tc_i * P:(tc_i + 1) * P],
                                        wv_sb[:, 2 * kp:2 * kp + 2,
                                              fh * 512 + q * 256:
                                              fh * 512 + (q + 1) * 256],
                                        perf_mode=DR,
                                        start=(q == 0 and kp == 0),
                                        stop=(kp == 3),
                                        skip_group_check=True)
                        else:
                            for k in range(8):
                                nc.tensor.matmul(
                                    psv[:, :, :],
                                    hkvT[:, k, tc_i * P:(tc_i + 1) * P],
                                    wv_sb[:, k, fh * 512:(fh + 1) * 512],
                                    start=(k == 0), stop=(k == 7))
                        if (tc_i + fh) % 2 == 0:
                            nc.vector.tensor_copy(
                                vaug[:, tc_i, fh * 8:(fh + 1) * 8, 0:64], psv)
                        else:
                            nc.scalar.copy(
                                vaug[:, tc_i, fh * 8:(fh + 1) * 8, 0:64], psv)

                # software-pipelined emission: the engine queues are in-order,
                # so interleave V and the first two groups' K/Q projections
                # between LN tiles to keep PE fed during the DVE-bound LN.
                wv_sb = apool.tile([P, 8, C], fv)
                kq_early = []
                for g in (0, 1):
                    wk_t = wpool.tile([P, 8, P], bf16, tag="wk")
                    wq_t = wpool.tile([P, 8, P], bf16, tag="wq")
                    KT = gpool.tile([P, T], bf16, tag="KT")
                    QT = gpool.tile([P, TQ], bf16, tag="QT")
                    kq_early.append((wk_t, wq_t, KT, QT))
                for rt in range(16):
                    ln_row_tile(xkv_d[rt * P:(rt + 1) * P, :], rt, False)
                    if rt == 0:
                        nc.sync.dma_start(wv_sb, wv_d[:, :, :])
                        emit_deferred_consts()
                        nc.vector.tensor_copy(vaug[:, :, :, 64:65], onesbt)
                    elif rt == 1:
                        nc.sync.dma_start(kq_early[0][0], wk_d[0])
                        nc.sync.dma_start(kq_early[0][1], wq_d[0])
                    elif rt == 2:
                        nc.sync.dma_start(kq_early[1][0], wk_d[1])
                        nc.sync.dma_start(kq_early[1][1], wq_d[1])
                    if rt >= 2:
                        emit_v(rt - 2)
                    if rt % 4 == 3:
                        emit_k(0, kq_early[0][0], kq_early[0][2], [rt // 4])
                for rt in range(8):
                    ln_row_tile(xq_d[rt * P:(rt + 1) * P, :], rt, True)
                    if rt < 2:
                        emit_v(14 + rt)
                    if rt % 2 == 1:
                        emit_k(1, kq_early[1][0], kq_early[1][2], [rt // 2])
                emit_q(0, kq_early[0][1], kq_early[0][3], [0, 1])
                emit_q(1, kq_early[1][1], kq_early[1][3], [0, 1])

            # -------- per-group K/Q projection + attention --------
            with tc.tile_pool(name="pt_sb", bufs=2) as ptpool, \
                 tc.tile_pool(name="sm_sb", bufs=3) as smpool, \
                 tc.tile_pool(name="ps_sc", bufs=2, space="PSUM") as ps_sc, \
                 tc.tile_pool(name="ps_ob", bufs=2, space="PSUM") as ps_ob:
                for g in range(8):
                    if g < 2:
                        _, _, KT, QT = kq_early[g]
                    else:
                        wk_t = wpool.tile([P, 8, P], bf16, tag="wk")
                        nc.sync.dma_start(wk_t, wk_d[g])
                        wq_t = wpool.tile([P, 8, P], bf16, tag="wq")
                        nc.sync.dma_start(wq_t, wq_d[g])
                        KT = gpool.tile([P, T], bf16, tag="KT")
                        emit_k(g, wk_t, KT, range(4))
                        QT = gpool.tile([P, TQ], bf16, tag="QT")
                        emit_q(g, wq_t, QT, range(2))

                    for hh in range(2):
                        base = 64 * hh
                        for lc in range(8):
                            nkt = NKT[lc]
                            ptb = ptpool.tile([P, 16, P], bf16, tag="ptb")
                            ob = ps_ob.tile([P, 65], f32, tag="ob")
                            for sg in range((nkt + 7) // 8):
                                n_in_g = min(8, nkt - sg * 8)
                                ps_s = ps_sc.tile([P, 8, P], f32, tag="sc")
                                for sl in range(n_in_g):
                                    kt = sg * 8 + sl
                                    last2 = (kt >= nkt - 2)
                                    nc.tensor.matmul(
                                        ps_s[:, sl, :],
                                        KT[base:base + 64, kt * P:(kt + 1) * P],
                                        QT[base:base + 64, lc * P:(lc + 1) * P],
                                        start=(sl % 4 == 0), stop=not last2,
                                        skip_group_check=True)
                                # causal mask on the last two key tiles
                                sl0 = (nkt - 2) % 8
                                if sg == (nkt - 1) // 8:
                                    nc.tensor.matmul(
                                        ps_s[:, sl0:sl0 + 2, :],
                                        identb, mask_sb[:, lc, :],
                                        start=False, stop=True,
                                        skip_group_check=True)
                                nc.scalar.activation(
                                    ptb[:, sg * 8:sg * 8 + n_in_g, :],
                                    ps_s[:, 0:n_in_g, :], AF.Exp)
                            h_idx = 2 * g + hh
                            for kt in range(nkt):
                                # out[q, d] = sum_k pt[k, q] * vaug[k, d];
                                # column 64 accumulates the softmax denom
                                nc.tensor.matmul(
                                    ob, ptb[:, kt, :],
                                    vaug[:, kt, h_idx, :],
                                    start=(kt == 0), stop=(kt == nkt - 1))
                            rec = smpool.tile([P, 1], f32, tag="rec")
                            nc.vector.reciprocal(rec, ob[:, 64:65])
                            nc.vector.tensor_scalar_mul(
                                otq[:, lc, h_idx, :], ob[:, 0:64], rec)

            cm_kqv.__exit__(None, None, None)
            cm_gpool.__exit__(None, None, None)
            cm_wpool.__exit__(None, None, None)

        # transpose attn out back to feature-on-partition for proj
        with tc.tile_pool(name="ps_trq", bufs=2, space="PSUM") as ps_trq:
            for lc in range(8):
                pq = ps_trq.tile([P, 8, P], bf16, tag="trq")
                for ft in range(8):
                    nc.tensor.matmul(
                        pq[:, ft, :], otq[:, lc, 2 * ft:2 * ft + 2, :],
                        identb, is_transpose=True, start=(ft == 0), stop=True,
                        skip_group_check=True)
                if lc % 2 == 0:
                    nc.vector.tensor_copy(OT[:, :, lc * P:(lc + 1) * P], pq)
                else:
                    nc.scalar.copy(OT[:, :, lc * P:(lc + 1) * P], pq)

        # ---------------- Phase B: proj + residual + LN2 + FFN -------------
        with tc.tile_pool(name="late", bufs=1) as late:
            x2T = late.tile([P, 8, TQ], f32r)
            h2 = late.tile([P, 8, TQ], ff_)
            with tc.tile_pool(name="pr_w", bufs=1) as prpool, \
                 tc.tile_pool(name="pr_tmp", bufs=3) as prtmp, \
                 tc.tile_pool(name="ps_pr", bufs=3, space="PSUM") as ps_pr:
                wp_sb = prpool.tile([P, 8, C], fp)
                nc.sync.dma_start(wp_sb, wp_d[:, :, :])
                # OT carries SV from the V-weight scaling; wp carries SP_
                inv = 1.0 / ((SV if V_FP8 else 1.0) * (SP_ if PROJ_FP8 else 1.0))
                for rc in range(4):
                    for of in range(8):
                        psum = ps_pr.tile([P, 256], f32, tag="pr")
                        if PROJ_FP8:
                            for kp in range(4):
                                nc.tensor.matmul(
                                    psum,
                                    wp_sb[:, 2 * kp:2 * kp + 2,
                                          of * P:(of + 1) * P],
                                    OT[:, 2 * kp:2 * kp + 2,
                                       rc * 256:(rc + 1) * 256],
                                    perf_mode=DR,
                                    start=(kp == 0), stop=(kp == 3))
                        else:
                            for k in range(8):
                                nc.tensor.matmul(
                                    psum, wp_sb[:, k, of * P:(of + 1) * P],
                                    OT[:, k, rc * 256:(rc + 1) * 256],
                                    start=(k == 0), stop=(k == 7))
                        t1 = prtmp.tile([P, 256], f32, tag="t1")
                        nc.scalar.activation(t1, psum, AF.Identity,
                                             bias=bp_sb[:, of:of + 1],
                                             scale=inv)
                        nc.gpsimd.tensor_tensor(
                            x2T[:, of, rc * 256:(rc + 1) * 256], t1,
                            xqT[:, of, rc * 256:(rc + 1) * 256], AL.add)

            # LN2 in transposed layout
            with tc.tile_pool(name="ln2_sb", bufs=4) as ln2sb, \
                 tc.tile_pool(name="ln2_sm", bufs=4) as ln2sm, \
                 tc.tile_pool(name="ps_l2s", bufs=2, space="PSUM") as ps_l2s, \
                 tc.tile_pool(name="ps_l2b", bufs=2, space="PSUM") as ps_l2b:
                for rc in range(2):
                    psum_m = ps_l2s.tile([1, 512], f32, tag="m")
                    psum_q = ps_l2s.tile([1, 512], f32, tag="q")
                    for ft in range(8):
                        xt = x2T[:, ft, rc * 512:(rc + 1) * 512]
                        sq = ln2sb.tile([P, 512], f32r, tag="sq")
                        nc.scalar.square(sq, xt)
                        nc.tensor.matmul(psum_m, onesr, xt,
                                         start=(ft == 0), stop=(ft == 7))
                        nc.tensor.matmul(psum_q, onesr, sq,
                                         start=(ft == 0), stop=(ft == 7))
                    mean = ln2sm.tile([1, 512], f32, tag="mean")
                    nc.vector.tensor_scalar_mul(mean, psum_m, 1.0 / C)
                    msq = ln2sm.tile([1, 512], f32, tag="msq")
                    nc.vector.tensor_scalar_mul(msq, psum_q, 1.0 / C)
                    var = ln2sm.tile([1, 512], f32, tag="var")
                    nc.vector.tensor_tensor(var, mean, mean, AL.mult)
                    nc.vector.tensor_tensor(var, msq, var, AL.subtract)
                    std = ln2sm.tile([1, 512], f32, tag="std")
                    nc.scalar.activation(std, var, AF.Sqrt,
                                         bias=eps1[:, :], scale=1.0)
                    a = ln2sm.tile([1, 512], f32r, tag="a")
                    with nc.allow_low_precision(reason="f32r has f32 bits"):
                        nc.vector.reciprocal(a, std)
                    bneg = ln2sm.tile([1, 512], f32r, tag="b")
                    nc.vector.tensor_tensor(bneg, mean, a, AL.mult)
                    ps_abc = ps_l2b.tile([P, 512], f32, tag="abc")
                    nc.tensor.matmul(ps_abc,
                                     onesr[0:1, 0:1].broadcast_to((1, P)), a,
                                     start=True, stop=True)
                    ps_bbc = ps_l2b.tile([P, 512], f32, tag="bbc")
                    nc.tensor.matmul(ps_bbc,
                                     onesr[0:1, 0:1].broadcast_to((1, P)),
                                     bneg, start=True, stop=True)
                    # SBUF copies of the broadcasts so gpsimd (no PSUM
                    # access) can share the normalize work with DVE
                    abc_sb = ln2sm.tile([P, 512], f32r, tag="abcs")
                    nc.scalar.copy(abc_sb, ps_abc)
                    bbc_sb = ln2sm.tile([P, 512], f32r, tag="bbcs")
                    nc.scalar.copy(bbc_sb, ps_bbc)
                    for ft in range(8):
                        tmp = ln2sb.tile([P, 512], f32r, tag="tmp")
                        if ft % 2 == 0:
                            nc.vector.tensor_tensor(
                                tmp, x2T[:, ft, rc * 512:(rc + 1) * 512],
                                ps_abc, AL.mult)
                            nc.vector.tensor_tensor(
                                h2[:, ft, rc * 512:(rc + 1) * 512], tmp,
                                ps_bbc, AL.subtract)
                        else:
                            nc.gpsimd.tensor_tensor(
                                tmp, x2T[:, ft, rc * 512:(rc + 1) * 512],
                                abc_sb, AL.mult)
                            nc.gpsimd.tensor_tensor(
                                h2[:, ft, rc * 512:(rc + 1) * 512], tmp,
                                bbc_sb, AL.subtract)

            # FFN
            with tc.tile_pool(name="f_w1", bufs=3) as w1pool, \
                 tc.tile_pool(name="f_w2", bufs=2) as w2pool, \
                 tc.tile_pool(name="f_relu", bufs=1) as rpool, \
                 tc.tile_pool(name="f_y", bufs=3) as ypool, \
                 tc.tile_pool(name="ps_f1", bufs=4, space="PSUM") as ps_f1, \
                 tc.tile_pool(name="ps_f2", bufs=4, space="PSUM") as ps_f2:
                relu1 = rpool.tile([P, 32, TQ], ff_)
                for fk in range(32):
                    w1_t = w1pool.tile([P, 8, P], ff_, tag="w1")
                    nc.sync.dma_start(w1_t, w1_d[fk])
                    for rc in range(2):
                        psum = ps_f1.tile([P, 512], f32, tag="f1")
                        if FFN_FP8:
                            for q in range(2):
                                for kp in range(4):
                                    nc.tensor.matmul(
                                        psum[:, q * 256:(q + 1) * 256],
                                        w1_t[:, 2 * kp:2 * kp + 2, :],
                                        h2[:, 2 * kp:2 * kp + 2,
                                           rc * 512 + q * 256:
                                           rc * 512 + (q + 1) * 256],
                                        perf_mode=DR,
                                        start=(q == 0 and kp == 0),
                                        stop=(kp == 3),
                                        skip_group_check=True)
                        else:
                            for k in range(8):
                                nc.tensor.matmul(
                                    psum, w1_t[:, k, :],
                                    h2[:, k, rc * 512:(rc + 1) * 512],
                                    start=(k == 0), stop=(k == 7))
                        out_sl = relu1[:, fk, rc * 512:(rc + 1) * 512]
                        if fk % 2 == 0:
                            nc.scalar.activation(out_sl, psum, AF.Relu,
                                                 bias=b1_sb[:, fk:fk + 1],
                                                 scale=1.0)
                        else:
                            nc.vector.tensor_scalar(out_sl, psum,
                                                    b1_sb[:, fk:fk + 1], 0.0,
                                                    op0=AL.add, op1=AL.max)
                for of in range(8):
                    w2_t = w2pool.tile([P, 32, P], ff_, tag="w2")
                    nc.sync.dma_start(w2_t, w2_d[of])
                    for rc in range(2):
                        psum = ps_f2.tile([P, 512], f32, tag="f2")
                        if FFN_FP8:
                            for q in range(2):
                                for kp in range(16):
                                    nc.tensor.matmul(
                                        psum[:, q * 256:(q + 1) * 256],
                                        w2_t[:, 2 * kp:2 * kp + 2, :],
                                        relu1[:, 2 * kp:2 * kp + 2,
                                              rc * 512 + q * 256:
                                              rc * 512 + (q + 1) * 256],
                                        perf_mode=DR,
                                        start=(q == 0 and kp == 0),
                                        stop=(kp == 15),
                                        skip_group_check=True)
                        else:
                            for fk in range(32):
                                nc.tensor.matmul(
                                    psum, w2_t[:, fk, :],
                                    relu1[:, fk, rc * 512:(rc + 1) * 512],
                                    start=(fk == 0), stop=(fk == 31))
                        t2 = ypool.tile([P, 512], f32, tag="t2")
                        inv2 = 1.0 / (S1 * S2) if FFN_FP8 else 1.0
                        nc.scalar.activation(t2, psum, AF.Identity,
                                             bias=b2_sb[:, of:of + 1],
                                             scale=inv2)
                        y_sb = ypool.tile([P, 512], f32, tag="y")
                        nc.gpsimd.tensor_tensor(
                            y_sb, t2, x2T[:, of, rc * 512:(rc + 1) * 512],
                            AL.add)
                        nc.sync.dma_start(
                            y_d[of * P:(of + 1) * P,
                                rc * 512:(rc + 1) * 512], y_sb)
    _split_sync_waits(nc)
    return nc


_PROGRAM = None


def _get_program():
    global _PROGRAM
    if _PROGRAM is None:
        _PROGRAM = build_program()
    return _PROGRAM


def _to_bf16(a):
    return np.asarray(a, np.float32).astype(ml_dtypes.bfloat16)


def _to_f8(a):
    return np.asarray(a, np.float32).astype(ml_dtypes.float8_e4m3)


def _host_prep(x, Wk, Wq, Wv, Wproj, bproj, W1, b1, W2, b2, g1, beta1, g2, beta2):
    """Fold LN affine params / V bias into weights (exact); quantize; shard."""
    x = np.asarray(x, np.float32)
    scale = 1.0 / np.sqrt(D)
    Wq_f = (g1[:, None] * np.asarray(Wq, np.float32)) * scale
    bq_f = (beta1 @ np.asarray(Wq, np.float32)) * scale
    Wk_f = g1[:, None] * np.asarray(Wk, np.float32)
    bk_f = beta1 @ np.asarray(Wk, np.float32)
    Wv_f = g1[:, None] * np.asarray(Wv, np.float32)
    bv_f = beta1 @ np.asarray(Wv, np.float32)
    Wp_f = np.asarray(Wproj, np.float32)
    bp_f = np.asarray(bproj, np.float32) + bv_f @ Wp_f
    W1_f = g2[:, None] * np.asarray(W1, np.float32)
    b1_f = np.asarray(b1, np.float32) + beta2 @ np.asarray(W1, np.float32)
    W2_f = np.asarray(W2, np.float32)
    b2_f = np.asarray(b2, np.float32)

    def tile_in_out(W, n_in, n_out, q):
        # [in, out] -> [n_out, 128, n_in, 128]
        return np.ascontiguousarray(q(
            W.reshape(n_in, P, n_out, P).transpose(2, 1, 0, 3)))

    def pko(W, q):
        # [in, out] -> [128, n_in_tiles, out]
        return np.ascontiguousarray(q(
            W.reshape(8, P, W.shape[1]).transpose(1, 0, 2)))

    common = {
        "wq": tile_in_out(Wq_f, 8, 8, _to_bf16),
        "wk": tile_in_out(Wk_f, 8, 8, _to_bf16),
        "wv": pko(Wv_f * (SV if V_FP8 else 1.0),
                  _to_f8 if V_FP8 else _to_bf16),
        "wp": pko(Wp_f * (SP_ if PROJ_FP8 else 1.0),
                  _to_f8 if PROJ_FP8 else _to_bf16),
        "w1": tile_in_out(W1_f * (S1 if FFN_FP8 else 1.0), 8, 32,
                          _to_f8 if FFN_FP8 else _to_bf16),
        "w2": tile_in_out(W2_f * (S2 if FFN_FP8 else 1.0), 32, 8,
                          _to_f8 if FFN_FP8 else _to_bf16),
        "bq": np.ascontiguousarray(bq_f, np.float32),
        "bk": np.ascontiguousarray(bk_f, np.float32),
        "bp": np.ascontiguousarray(bp_f, np.float32),
        "b1": np.ascontiguousarray(b1_f * (S1 if FFN_FP8 else 1.0), np.float32),
        "b2": np.ascontiguousarray(b2_f, np.float32),
        "identr": np.eye(P, dtype=np.float32),
        "identb": np.eye(P, dtype=np.float32).astype(ml_dtypes.bfloat16),
        "onesb": np.ones((P, 16, 16, 1), np.float32).astype(ml_dtypes.bfloat16),
        "onesr": np.ones((P, 1), np.float32),
    }

    in_maps = []
    row_maps = []
    for core in range(8):
        b, s = core // 2, core % 2
        gmap = _chunk_map(s)
        rows = np.concatenate([np.arange(G * P, (G + 1) * P) for G in gmap])
        row_maps.append((b, rows))
        # mask[i, k, slot*128+q]: key pos 256i+128*slot+k vs query pos 128*G+q
        mask = np.empty((8, P, 256), np.float32)
        for i, G in enumerate(gmap):
            kpos = (256 * i + np.arange(256)).reshape(2, P).transpose(1, 0)
            qpos = G * P + np.arange(P)
            m = np.where(kpos[:, :, None] <= qpos[None, None, :], 0.0, NEG)
            mask[i] = m.reshape(P, 2, P).transpose(0, 1, 2).reshape(P, 256)
        in_maps.append({
            "xkv": np.ascontiguousarray(x[b]),
            "xq": np.ascontiguousarray(x[b][rows]),
            "maskc": mask.astype(ml_dtypes.bfloat16),
            **common,
        })
    return in_maps, row_maps


def kernel(**inputs):
    nc = _get_program()
    in_maps, row_maps = _host_prep(**inputs)
    res = run_bass_kernel_spmd(nc, in_maps, core_ids=list(range(8)))
    out = np.empty((B, T, C), np.float32)
    for core in range(8):
        b, rows = row_maps[core]
        out[b][rows] = res.results[core]["y"].T
    return out


# revision 86
# speedup vs baseline: 2.3658x; 1.0442x over previous
"""Trainium2 Bass kernel for one dense transformer block.

Full (unsharded) IO: x [4, 2048, 1024] -> out [4, 2048, 1024].
Sharding: 8 cores = 4 batches x 2 query-chunk sets. Each core owns one
batch's K/V (2048 rows) and 1024 query rows chosen as causally-balanced
128-row chunks (set A: global chunks {4j, 4j+3}, set B: {4j+1, 4j+2}).
Local chunk slot i attends to exactly 2*(i+1) key tiles on every core, so
the instruction stream is identical across cores (SPMD); a per-core data
mask handles the causal boundary in the last two key tiles. No collectives.

On-core dataflow is feature-on-partition ("transposed") throughout. Matmul
dtypes: Q/K projections and scores in bf16 (f32 psum), attention
probabilities bf16, V / output-projection / FFN in fp8-e4m3 with DoubleRow
perf mode (two 128-deep k-tiles per instruction) and power-of-two weight
scaling to avoid fp8 subnormals. LayerNorm runs in f32. Residuals in f32.
LN affine params and biases are folded host-side (exact); V bias is folded
into the projection bias (exact).
"""

import sys

sys.path.insert(0, "/opt/trn_rl_repo")

import numpy as np
import ml_dtypes

import concourse.bass as bass
import concourse.mybir as mybir
import concourse.tile as tile
from concourse.bass_utils import run_bass_kernel_spmd

f32 = mybir.dt.float32
f32r = mybir.dt.float32r
bf16 = mybir.dt.bfloat16
f8 = mybir.dt.float8e4
AL = mybir.AluOpType
AF = mybir.ActivationFunctionType
DR = mybir.MatmulPerfMode.DoubleRow

B, T, C = 4, 2048, 1024
H, D = 16, 64
F = 4 * C
P = 128
TQ = 1024
NEG = -30000.0
LN_EPS = 1e-5

# dtype strategy flags (validated against the 2e-2 rel-err budget)
V_FP8 = True      # V projection via fp8 DoubleRow
PROJ_FP8 = True   # output projection via fp8 DoubleRow
FFN_FP8 = True    # both FFN matmuls via fp8 DoubleRow
SV = 16.0         # host scale on Wv before fp8 cast (carried into OT)
SP_ = 16.0        # host scale on Wproj
S1 = 16.0         # host scale on W1
S2 = 64.0         # host scale on W2

# local chunk slot -> number of key tiles computed (set-independent)
NKT = [2 * (i + 1) for i in range(8)]


def _chunk_map(s):
    """Global 128-row chunk indices owned by set s, in slot order."""
    if s == 0:
        out = []
        for j in range(4):
            out += [4 * j, 4 * j + 3]
        return sorted(out)
    out = []
    for j in range(4):
        out += [4 * j + 1, 4 * j + 2]
    return sorted(out)


def _split_sync_waits(nc):
    """This container's walrus supports one sync-wait per instruction; Tile
    emits up to ~3. Hoist extras onto NoOps inserted before the owner."""
    ctr = 0
    for fn in nc.m.functions:
        for bb in fn.blocks:
            out, changed = [], False
            for ins in bb.instructions:
                si = ins.sync_info
                waits = list(si.on_wait) if si is not None and si.on_wait else []
                if len(waits) > 1:
                    changed = True
                    for w in waits[:-1]:
                        ctr += 1
                        nop = mybir.InstNoOp(name=f"waitsplit_{ctr}", ins=[], outs=[])
                        nop.engine = ins.engine
                        nop.sync_info = mybir.SyncInfo(on_wait=[w], on_update=[])
                        out.append(nop)
                        nc.register_instruction(nop, overwrite=True)
                    ins.sync_info = mybir.SyncInfo(
                        on_wait=[waits[-1]], on_update=list(si.on_update or [])
                    )
                out.append(ins)
            if changed:
                bb.instructions = out


def build_program():
    nc = bass.Bass()
    xkv_d = nc.dram_tensor("xkv", [T, C], f32r, kind="ExternalInput")
    xq_d = nc.dram_tensor("xq", [TQ, C], f32r, kind="ExternalInput")
    mask_d = nc.dram_tensor("maskc", [8, P, 256], bf16, kind="ExternalInput")
    wq_d = nc.dram_tensor("wq", [8, P, 8, P], bf16, kind="ExternalInput")
    wk_d = nc.dram_tensor("wk", [8, P, 8, P], bf16, kind="ExternalInput")
    wv_d = nc.dram_tensor("wv", [P, 8, C], f8 if V_FP8 else bf16,
                          kind="ExternalInput")
    wp_d = nc.dram_tensor("wp", [P, 8, C], f8 if PROJ_FP8 else bf16,
                          kind="ExternalInput")
    w1_d = nc.dram_tensor("w1", [32, P, 8, P], f8 if FFN_FP8 else bf16,
                          kind="ExternalInput")
    w2_d = nc.dram_tensor("w2", [8, P, 32, P], f8 if FFN_FP8 else bf16,
                          kind="ExternalInput")
    bq_d = nc.dram_tensor("bq", [C], f32, kind="ExternalInput")
    bk_d = nc.dram_tensor("bk", [C], f32, kind="ExternalInput")
    bp_d = nc.dram_tensor("bp", [C], f32, kind="ExternalInput")
    b1_d = nc.dram_tensor("b1", [F], f32, kind="ExternalInput")
    b2_d = nc.dram_tensor("b2", [C], f32, kind="ExternalInput")
    identr_d = nc.dram_tensor("identr", [P, P], f32r, kind="ExternalInput")
    identb_d = nc.dram_tensor("identb", [P, P], bf16, kind="ExternalInput")
    onesb_d = nc.dram_tensor("onesb", [P, 16, 16, 1], bf16, kind="ExternalInput")
    onesr_d = nc.dram_tensor("onesr", [P, 1], f32r, kind="ExternalInput")
    y_d = nc.dram_tensor("y", [C, TQ], f32, kind="ExternalOutput")

    fv = f8 if V_FP8 else bf16
    fp = f8 if PROJ_FP8 else bf16
    ff_ = f8 if FFN_FP8 else bf16

    with tile.TileContext(nc) as tc:
      with tc.tile_pool(name="consts", bufs=1) as cpool, \
           tc.tile_pool(name="persist", bufs=1) as pers:
        # only identb (first LN transposes) loads before the x rows; the rest
        # of the constants are deferred into the LN stream so they don't
        # serialize in front of the first x-row DMA
        identb = cpool.tile([P, P], bf16)
        nc.sync.dma_start(identb, identb_d[:, :])
        identr = cpool.tile([P, P], f32r)
        onesbt = cpool.tile([P, 16, 16, 1], bf16)
        onesr = cpool.tile([P, 1], f32r)
        eps128 = cpool.tile([P, 1], f32)
        nc.vector.memset(eps128, LN_EPS)
        eps1 = cpool.tile([1, 1], f32)
        nc.vector.memset(eps1, LN_EPS)
        bq_sb = cpool.tile([P, 8], f32)
        bk_sb = cpool.tile([P, 8], f32)
        bp_sb = cpool.tile([P, 8], f32)
        b1_sb = cpool.tile([P, 32], f32)
        b2_sb = cpool.tile([P, 8], f32)
        mask_sb = cpool.tile([P, 8, 256], bf16)

        def emit_deferred_consts(step):
            # spread across LN tiles so no batch delays the next x-row DMA
            if step == 0:
                nc.sync.dma_start(bq_sb, bq_d.rearrange("(o p) -> p o", p=P))
                nc.sync.dma_start(bk_sb, bk_d.rearrange("(o p) -> p o", p=P))
            elif step == 1:
                nc.sync.dma_start(onesbt, onesb_d[:, :, :, :])
                nc.sync.dma_start(identr, identr_d[:, :])
            elif step == 2:
                for i in range(4):
                    nc.sync.dma_start(mask_sb[:, i, :], mask_d[i])
            elif step == 3:
                for i in range(4, 8):
                    nc.sync.dma_start(mask_sb[:, i, :], mask_d[i])
            elif step == 4:
                nc.sync.dma_start(bp_sb, bp_d.rearrange("(o p) -> p o", p=P))
                nc.sync.dma_start(b1_sb, b1_d.rearrange("(o p) -> p o", p=P))
                nc.sync.dma_start(b2_sb, b2_d.rearrange("(o p) -> p o", p=P))
                nc.sync.dma_start(onesr, onesr_d[:, :])

        OT = pers.tile([P, 8, TQ], fp)        # attn out (x SV), transposed
        otq = pers.tile([P, 8, 16, 64], bf16)  # attn out row-major (q, head, d)
        xqT = pers.tile([P, 8, TQ], f32r)     # raw queries, transposed

        # ---------------- Phase A: LN1 + transposes + V + attention --------
        with tc.tile_pool(name="attn_big", bufs=1) as apool:
            hkvT = apool.tile([P, 8, T], bf16)
            hqT = apool.tile([P, 8, TQ], bf16)
            if V_FP8:
                hkv8 = apool.tile([P, 8, T], f8)
            else:
                hkv8 = None
            vaug = apool.tile([P, 16, 16, 65], bf16)

            cm_wpool = tc.tile_pool(name="kq_w", bufs=2)
            cm_gpool = tc.tile_pool(name="kq_sb", bufs=2)
            cm_kqv = tc.tile_pool(name="ps_kqv", bufs=2, space="PSUM")
            wpool = cm_wpool.__enter__()
            gpool = cm_gpool.__enter__()
            ps_kqv = cm_kqv.__enter__()

            def emit_k(g, wk_t, KT, rcs):
                for rc in rcs:
                    psum = ps_kqv.tile([P, 512], f32, tag="kqv")
                    for k in range(8):
                        nc.tensor.matmul(
                            psum, wk_t[:, k, :],
                            hkvT[:, k, rc * 512:(rc + 1) * 512],
                            start=(k == 0), stop=(k == 7))
                    if rc % 2 == 0:
                        nc.vector.tensor_scalar_add(
                            KT[:, rc * 512:(rc + 1) * 512], psum,
                            bk_sb[:, g:g + 1])
                    else:
                        nc.scalar.activation(
                            KT[:, rc * 512:(rc + 1) * 512], psum,
                            AF.Identity, bias=bk_sb[:, g:g + 1], scale=1.0)

            def emit_q(g, wq_t, QT, rcs):
                for rc in rcs:
                    psum = ps_kqv.tile([P, 512], f32, tag="kqv")
                    for k in range(8):
                        nc.tensor.matmul(
                            psum, wq_t[:, k, :],
                            hqT[:, k, rc * 512:(rc + 1) * 512],
                            start=(k == 0), stop=(k == 7))
                    if rc % 2 == 0:
                        nc.scalar.activation(
                            QT[:, rc * 512:(rc + 1) * 512], psum,
                            AF.Identity, bias=bq_sb[:, g:g + 1], scale=1.0)
                    else:
                        nc.vector.tensor_scalar_add(
                            QT[:, rc * 512:(rc + 1) * 512], psum,
                            bq_sb[:, g:g + 1])

            with tc.tile_pool(name="ln_sb", bufs=3) as lnsb, \
                 tc.tile_pool(name="ln_small", bufs=6) as lnsm, \
                 tc.tile_pool(name="ps_a", bufs=1, space="PSUM") as ps_a, \
                 tc.tile_pool(name="ps_v", bufs=2, space="PSUM") as ps_v:

                def ln_row_tile(src_ap, rt, is_q):
                    xrow = lnsb.tile([P, C], f32r, tag="xrow")
                    nc.sync.dma_start(xrow, src_ap)
                    stats = lnsm.tile([P, 2, 6], f32, tag="stats")
                    for sg in range(2):
                        nc.vector.bn_stats(stats[:, sg, :],
                                           xrow[:, sg * 512:(sg + 1) * 512])
                    mv = lnsm.tile([P, 2], f32, tag="mv")
                    nc.vector.bn_aggr(mv, stats)
                    rstd = lnsm.tile([P, 1], f32, tag="rstd")
                    nc.scalar.activation(rstd, mv[:, 1:2], AF.Sqrt,
                                         bias=eps128, scale=1.0)
                    nc.vector.reciprocal(rstd, rstd)
                    hrow = lnsb.tile([P, C], bf16, tag="hrow")
                    nc.gpsimd.tensor_scalar(hrow, xrow, mv[:, 0:1], rstd,
                                            op0=AL.subtract, op1=AL.mult)
                    dstT = hqT if is_q else hkvT
                    # one accumulation "start" per 2KB PSUM bank: later
                    # region-disjoint writes use start=False (pending-zero
                    # bytes are overwritten, not accumulated)
                    pstb = ps_a.tile([P, 8, P], bf16, tag="trb")
                    for ft in range(8):
                        nc.tensor.matmul(pstb[:, ft, :],
                                         hrow[:, ft * P:(ft + 1) * P],
                                         identb, is_transpose=True,
                                         start=(ft == 0), stop=True,
                                         skip_group_check=True)
                    if rt % 2 == 0:
                        nc.vector.tensor_copy(dstT[:, :, rt * P:(rt + 1) * P],
                                              pstb)
                        if not is_q and V_FP8:
                            nc.scalar.copy(hkv8[:, :, rt * P:(rt + 1) * P],
                                           pstb)
                    else:
                        nc.scalar.copy(dstT[:, :, rt * P:(rt + 1) * P], pstb)
                        if not is_q and V_FP8:
                            nc.vector.tensor_copy(
                                hkv8[:, :, rt * P:(rt + 1) * P], pstb)
                    if is_q:
                        pstr = ps_a.tile([P, 8, P], f32r, tag="trr")
                        for ft in range(8):
                            nc.tensor.matmul(pstr[:, ft, :],
                                             xrow[:, ft * P:(ft + 1) * P],
                                             identr, is_transpose=True,
                                             start=(ft % 4 == 0), stop=True,
                                             skip_group_check=True)
                        nc.scalar.copy(xqT[:, :, rt * P:(rt + 1) * P], pstr)

                def emit_v(tc_i):
                    for fh in range(2):
                        psv = ps_v.tile([P, 8, 64], f32, tag="v")
                        if V_FP8:
                            for q in range(2):
                                for kp in range(4):
                                    nc.tensor.matmul(
                                        psv[:, q * 4:(q + 1) * 4, :],
                                        hkv8[:, 2 * kp:2 * kp + 2,
                                             tc_i * P:(tc_i + 1) * P],
                                        wv_sb[:, 2 * kp:2 * kp + 2,
                                              fh * 512 + q * 256:
                                              fh * 512 + (q + 1) * 256],
                                        perf_mode=DR,
                                        start=(q == 0 and kp == 0),
                                        stop=(kp == 3),
                                        skip_group_check=True)
                        else:
                            for k in range(8):
                                nc.tensor.matmul(
                                    psv[:, :, :],
                                    hkvT[:, k, tc_i * P:(tc_i + 1) * P],
                                    wv_sb[:, k, fh * 512:(fh + 1) * 512],
                                    start=(k == 0), stop=(k == 7))
                        if (tc_i + fh) % 2 == 0:
                            nc.vector.tensor_copy(
                                vaug[:, tc_i, fh * 8:(fh + 1) * 8, 0:64], psv)
                        else:
                            nc.scalar.copy(
                                vaug[:, tc_i, fh * 8:(fh + 1) * 8, 0:64], psv)

                # software-pipelined emission: the engine queues are in-order,
                # so interleave V and the first two groups' K/Q projections
                # between LN tiles to keep PE fed during the DVE-bound LN.
                wv_sb = apool.tile([P, 8, C], fv)
                kq_early = []
                for g in (0, 1):
                    wk_t = wpool.tile([P, 8, P], bf16, tag="wk")
                    wq_t = wpool.tile([P, 8, P], bf16, tag="wq")
                    KT = gpool.tile([P, T], bf16, tag="KT")
                    QT = gpool.tile([P, TQ], bf16, tag="QT")
                    kq_early.append((wk_t, wq_t, KT, QT))
                for rt in range(16):
                    ln_row_tile(xkv_d[rt * P:(rt + 1) * P, :], rt, False)
                    if rt == 0:
                        nc.sync.dma_start(wv_sb, wv_d[:, :, :])
                        emit_deferred_consts(0)
                    elif rt == 1:
                        nc.sync.dma_start(kq_early[0][0], wk_d[0])
                        nc.sync.dma_start(kq_early[0][1], wq_d[0])
                    elif rt == 2:
                        nc.sync.dma_start(kq_early[1][0], wk_d[1])
                        nc.sync.dma_start(kq_early[1][1], wq_d[1])
                    if 2 <= rt <= 5:
                        emit_deferred_consts(rt - 1)
                        if rt == 2:
                            nc.vector.tensor_copy(vaug[:, :, :, 64:65],
                                                  onesbt)
                    if rt >= 2:
                        emit_v(rt - 2)
                    if rt % 4 == 3:
                        emit_k(0, kq_early[0][0], kq_early[0][2], [rt // 4])
                for rt in range(8):
                    ln_row_tile(xq_d[rt * P:(rt + 1) * P, :], rt, True)
                    if rt < 2:
                        emit_v(14 + rt)
                    if rt % 2 == 1:
                        emit_k(1, kq_early[1][0], kq_early[1][2], [rt // 2])
                emit_q(0, kq_early[0][1], kq_early[0][3], [0, 1])
                emit_q(1, kq_early[1][1], kq_early[1][3], [0, 1])

            # -------- per-group K/Q projection + attention --------
            with tc.tile_pool(name="pt_sb", bufs=2) as ptpool, \
                 tc.tile_pool(name="sm_sb", bufs=3) as smpool, \
                 tc.tile_pool(name="ps_sc", bufs=2, space="PSUM") as ps_sc, \
                 tc.tile_pool(name="ps_ob", bufs=2, space="PSUM") as ps_ob:
                for g in range(8):
                    if g < 2:
                        _, _, KT, QT = kq_early[g]
                    else:
                        wk_t = wpool.tile([P, 8, P], bf16, tag="wk")
                        nc.sync.dma_start(wk_t, wk_d[g])
                        wq_t = wpool.tile([P, 8, P], bf16, tag="wq")
                        nc.sync.dma_start(wq_t, wq_d[g])
                        KT = gpool.tile([P, T], bf16, tag="KT")
                        emit_k(g, wk_t, KT, range(4))
                        QT = gpool.tile([P, TQ], bf16, tag="QT")
                        emit_q(g, wq_t, QT, range(2))

                    for hh in range(2):
                        base = 64 * hh
                        for lc in range(8):
                            nkt = NKT[lc]
                            ptb = ptpool.tile([P, 16, P], bf16, tag="ptb")
                            ob = ps_ob.tile([P, 65], f32, tag="ob")
                            for sg in range((nkt + 7) // 8):
                                n_in_g = min(8, nkt - sg * 8)
                                ps_s = ps_sc.tile([P, 8, P], f32, tag="sc")
                                for sl in range(n_in_g):
                                    kt = sg * 8 + sl
                                    last2 = (kt >= nkt - 2)
                                    nc.tensor.matmul(
                                        ps_s[:, sl, :],
                                        KT[base:base + 64, kt * P:(kt + 1) * P],
                                        QT[base:base + 64, lc * P:(lc + 1) * P],
                                        start=(sl % 4 == 0), stop=not last2,
                                        skip_group_check=True)
                                # causal mask on the last two key tiles
                                sl0 = (nkt - 2) % 8
                                if sg == (nkt - 1) // 8:
                                    nc.tensor.matmul(
                                        ps_s[:, sl0:sl0 + 2, :],
                                        identb, mask_sb[:, lc, :],
                                        start=False, stop=True,
                                        skip_group_check=True)
                                nc.scalar.activation(
                                    ptb[:, sg * 8:sg * 8 + n_in_g, :],
                                    ps_s[:, 0:n_in_g, :], AF.Exp)
                            h_idx = 2 * g + hh
                            for kt in range(nkt):
                                # out[q, d] = sum_k pt[k, q] * vaug[k, d];
                                # column 64 accumulates the softmax denom
                                nc.tensor.matmul(
                                    ob, ptb[:, kt, :],
                                    vaug[:, kt, h_idx, :],
                                    start=(kt == 0), stop=(kt == nkt - 1))
                            rec = smpool.tile([P, 1], f32, tag="rec")
                            nc.vector.reciprocal(rec, ob[:, 64:65])
                            nc.vector.tensor_scalar_mul(
                                otq[:, lc, h_idx, :], ob[:, 0:64], rec)

            cm_kqv.__exit__(None, None, None)
            cm_gpool.__exit__(None, None, None)
            cm_wpool.__exit__(None, None, None)

        # transpose attn out back to feature-on-partition for proj
        with tc.tile_pool(name="ps_trq", bufs=2, space="PSUM") as ps_trq:
            for lc in range(8):
                pq = ps_trq.tile([P, 8, P], bf16, tag="trq")
                for ft in range(8):
                    nc.tensor.matmul(
                        pq[:, ft, :], otq[:, lc, 2 * ft:2 * ft + 2, :],
                        identb, is_transpose=True, start=(ft == 0), stop=True,
                        skip_group_check=True)
                if lc % 2 == 0:
                    nc.vector.tensor_copy(OT[:, :, lc * P:(lc + 1) * P], pq)
                else:
                    nc.scalar.copy(OT[:, :, lc * P:(lc + 1) * P], pq)

        # ---------------- Phase B: proj + residual + LN2 + FFN -------------
        with tc.tile_pool(name="late", bufs=1) as late:
            x2T = late.tile([P, 8, TQ], f32r)
            h2 = late.tile([P, 8, TQ], ff_)
            # proj and LN2 share a scope: each LN2 half's stats are emitted
            # as soon as its projection columns land, so only the short
            # small-op chain remains after the projection finishes
            with tc.tile_pool(name="pr_w", bufs=1) as prpool, \
                 tc.tile_pool(name="pr_tmp", bufs=3) as prtmp, \
                 tc.tile_pool(name="ln2_sb", bufs=4) as ln2sb, \
                 tc.tile_pool(name="ln2_sm", bufs=4) as ln2sm, \
                 tc.tile_pool(name="ps_pr", bufs=2, space="PSUM") as ps_pr, \
                 tc.tile_pool(name="ps_l2s", bufs=2, space="PSUM") as ps_l2s, \
                 tc.tile_pool(name="ps_l2b", bufs=1, space="PSUM") as ps_l2b:
                wp_sb = prpool.tile([P, 8, C], fp)
                nc.sync.dma_start(wp_sb, wp_d[:, :, :])
                # OT carries SV from the V-weight scaling; wp carries SP_
                inv = 1.0 / ((SV if V_FP8 else 1.0) * (SP_ if PROJ_FP8 else 1.0))
                l2stats = []

                def emit_ln2_stats(h):
                    psum_m = ps_l2s.tile([1, 512], f32, tag="m")
                    psum_q = ps_l2s.tile([1, 512], f32, tag="q")
                    for ft in range(8):
                        xt = x2T[:, ft, h * 512:(h + 1) * 512]
                        sq = ln2sb.tile([P, 512], f32r, tag="sq")
                        nc.vector.tensor_tensor(sq, xt, xt, AL.mult)
                        nc.tensor.matmul(psum_m, onesr, xt,
                                         start=(ft == 0), stop=(ft == 7))
                        nc.tensor.matmul(psum_q, onesr, sq,
                                         start=(ft == 0), stop=(ft == 7))
                    l2stats.append((psum_m, psum_q))

                def emit_ln2_finish(h):
                    psum_m, psum_q = l2stats[h]
                    mean = ln2sm.tile([1, 512], f32, tag="mean")
                    nc.vector.tensor_scalar_mul(mean, psum_m, 1.0 / C)
                    msq = ln2sm.tile([1, 512], f32, tag="msq")
                    nc.vector.tensor_scalar_mul(msq, psum_q, 1.0 / C)
                    var = ln2sm.tile([1, 512], f32, tag="var")
                    nc.vector.tensor_tensor(var, mean, mean, AL.mult)
                    nc.vector.tensor_tensor(var, msq, var, AL.subtract)
                    std = ln2sm.tile([1, 512], f32, tag="std")
                    nc.scalar.activation(std, var, AF.Sqrt,
                                         bias=eps1[:, :], scale=1.0)
                    a = ln2sm.tile([1, 512], f32r, tag="a")
                    with nc.allow_low_precision(reason="f32r has f32 bits"):
                        nc.vector.reciprocal(a, std)
                    bneg = ln2sm.tile([1, 512], f32r, tag="b")
                    nc.vector.tensor_tensor(bneg, mean, a, AL.mult)
                    ps_abc = ps_l2b.tile([P, 512], f32, tag="abc")
                    nc.tensor.matmul(ps_abc,
                                     onesr[0:1, 0:1].broadcast_to((1, P)), a,
                                     start=True, stop=True)
                    ps_bbc = ps_l2b.tile([P, 512], f32, tag="bbc")
                    nc.tensor.matmul(ps_bbc,
                                     onesr[0:1, 0:1].broadcast_to((1, P)),
                                     bneg, start=True, stop=True)
                    abc_sb = ln2sm.tile([P, 512], f32r, tag="abcs")
                    nc.scalar.copy(abc_sb, ps_abc)
                    bbc_sb = ln2sm.tile([P, 512], f32r, tag="bbcs")
                    nc.scalar.copy(bbc_sb, ps_bbc)
                    for ft in range(8):
                        tmp = ln2sb.tile([P, 512], f32r, tag="tmp")
                        if ft % 2 == 0:
                            nc.vector.tensor_tensor(
                                tmp, x2T[:, ft, h * 512:(h + 1) * 512],
                                ps_abc, AL.mult)
                            nc.vector.tensor_tensor(
                                h2[:, ft, h * 512:(h + 1) * 512], tmp,
                                ps_bbc, AL.subtract)
                        else:
                            nc.gpsimd.tensor_tensor(
                                tmp, x2T[:, ft, h * 512:(h + 1) * 512],
                                abc_sb, AL.mult)
                            nc.gpsimd.tensor_tensor(
                                h2[:, ft, h * 512:(h + 1) * 512], tmp,
                                bbc_sb, AL.subtract)

                for rc in range(4):
                    for of in range(8):
                        psum = ps_pr.tile([P, 256], f32, tag="pr")
                        if PROJ_FP8:
                            for kp in range(4):
                                nc.tensor.matmul(
                                    psum,
                                    wp_sb[:, 2 * kp:2 * kp + 2,
                                          of * P:(of + 1) * P],
                                    OT[:, 2 * kp:2 * kp + 2,
                                       rc * 256:(rc + 1) * 256],
                                    perf_mode=DR,
                                    start=(kp == 0), stop=(kp == 3))
                        else:
                            for k in range(8):
                                nc.tensor.matmul(
                                    psum, wp_sb[:, k, of * P:(of + 1) * P],
                                    OT[:, k, rc * 256:(rc + 1) * 256],
                                    start=(k == 0), stop=(k == 7))
                        t1 = prtmp.tile([P, 256], f32, tag="t1")
                        nc.scalar.activation(t1, psum, AF.Identity,
                                             bias=bp_sb[:, of:of + 1],
                                             scale=inv)
                        nc.gpsimd.tensor_tensor(
                            x2T[:, of, rc * 256:(rc + 1) * 256], t1,
                            xqT[:, of, rc * 256:(rc + 1) * 256], AL.add)
                    if rc % 2 == 1:
                        emit_ln2_stats(rc // 2)
                for h in range(2):
                    emit_ln2_finish(h)


            # FFN
            with tc.tile_pool(name="f_w1", bufs=3) as w1pool, \
                 tc.tile_pool(name="f_w2", bufs=2) as w2pool, \
                 tc.tile_pool(name="f_relu", bufs=1) as rpool, \
                 tc.tile_pool(name="f_y", bufs=3) as ypool, \
                 tc.tile_pool(name="ps_f1", bufs=4, space="PSUM") as ps_f1, \
                 tc.tile_pool(name="ps_f2", bufs=4, space="PSUM") as ps_f2:
                relu1 = rpool.tile([P, 32, TQ], ff_)
                for fk in range(32):
                    w1_t = w1pool.tile([P, 8, P], ff_, tag="w1")
                    nc.sync.dma_start(w1_t, w1_d[fk])
                    for rc in range(2):
                        psum = ps_f1.tile([P, 512], f32, tag="f1")
                        if FFN_FP8:
                            for q in range(2):
                                for kp in range(4):
                                    nc.tensor.matmul(
                                        psum[:, q * 256:(q + 1) * 256],
                                        w1_t[:, 2 * kp:2 * kp + 2, :],
                                        h2[:, 2 * kp:2 * kp + 2,
                                           rc * 512 + q * 256:
                                           rc * 512 + (q + 1) * 256],
                                        perf_mode=DR,
                                        start=(q == 0 and kp == 0),
                                        stop=(kp == 3),
                                        skip_group_check=True)
                        else:
                            for k in range(8):
                                nc.tensor.matmul(
                                    psum, w1_t[:, k, :],
                                    h2[:, k, rc * 512:(rc + 1) * 512],
                                    start=(k == 0), stop=(k == 7))
                        out_sl = relu1[:, fk, rc * 512:(rc + 1) * 512]
                        if fk % 2 == 0:
                            nc.scalar.activation(out_sl, psum, AF.Relu,
                                                 bias=b1_sb[:, fk:fk + 1],
                                                 scale=1.0)
                        else:
                            nc.vector.tensor_scalar(out_sl, psum,
                                                    b1_sb[:, fk:fk + 1], 0.0,
                                                    op0=AL.add, op1=AL.max)
                for of in range(8):
                    w2_t = w2pool.tile([P, 32, P], ff_, tag="w2")
                    nc.sync.dma_start(w2_t, w2_d[of])
                    for rc in range(2):
                        psum = ps_f2.tile([P, 512], f32, tag="f2")
                        if FFN_FP8:
                            for q in range(2):
                                for kp in range(16):
                                    nc.tensor.matmul(
                                        psum[:, q * 256:(q + 1) * 256],
                                        w2_t[:, 2 * kp:2 * kp + 2, :],
                                        relu1[:, 2 * kp:2 * kp + 2,
                                              rc * 512 + q * 256:
                                              rc * 512 + (q + 1) * 256],
                                        perf_mode=DR,
                                        start=(q == 0 and kp == 0),
                                        stop=(kp == 15),
                                        skip_group_check=True)
                        else:
                            for fk in range(32):
                                nc.tensor.matmul(
                                    psum, w2_t[:, fk, :],
                                    relu1[:, fk, rc * 512:(rc + 1) * 512],
                                    start=(fk == 0), stop=(fk == 31))
                        t2 = ypool.tile([P, 512], f32, tag="t2")
                        inv2 = 1.0 / (S1 * S2) if FFN_FP8 else 1.0
                        nc.scalar.activation(t2, psum, AF.Identity,
                                             bias=b2_sb[:, of:of + 1],
                                             scale=inv2)
                        y_sb = ypool.tile([P, 512], f32, tag="y")
                        nc.gpsimd.tensor_tensor(
                            y_sb, t2, x2T[:, of, rc * 512:(rc + 1) * 512],
                            AL.add)
                        nc.sync.dma_start(
                            y_d[of * P:(of + 1) * P,
                                rc * 512:(rc + 1) * 512], y_sb)
    _split_sync_waits(nc)
    return nc


_PROGRAM = None


def _get_program():
    global _PROGRAM
    if _PROGRAM is None:
        _PROGRAM = build_program()
    return _PROGRAM


def _to_bf16(a):
    return np.asarray(a, np.float32).astype(ml_dtypes.bfloat16)


def _to_f8(a):
    return np.asarray(a, np.float32).astype(ml_dtypes.float8_e4m3)


def _host_prep(x, Wk, Wq, Wv, Wproj, bproj, W1, b1, W2, b2, g1, beta1, g2, beta2):
    """Fold LN affine params / V bias into weights (exact); quantize; shard."""
    x = np.asarray(x, np.float32)
    scale = 1.0 / np.sqrt(D)
    Wq_f = (g1[:, None] * np.asarray(Wq, np.float32)) * scale
    bq_f = (beta1 @ np.asarray(Wq, np.float32)) * scale
    Wk_f = g1[:, None] * np.asarray(Wk, np.float32)
    bk_f = beta1 @ np.asarray(Wk, np.float32)
    Wv_f = g1[:, None] * np.asarray(Wv, np.float32)
    bv_f = beta1 @ np.asarray(Wv, np.float32)
    Wp_f = np.asarray(Wproj, np.float32)
    bp_f = np.asarray(bproj, np.float32) + bv_f @ Wp_f
    W1_f = g2[:, None] * np.asarray(W1, np.float32)
    b1_f = np.asarray(b1, np.float32) + beta2 @ np.asarray(W1, np.float32)
    W2_f = np.asarray(W2, np.float32)
    b2_f = np.asarray(b2, np.float32)

    def tile_in_out(W, n_in, n_out, q):
        # [in, out] -> [n_out, 128, n_in, 128]
        return np.ascontiguousarray(q(
            W.reshape(n_in, P, n_out, P).transpose(2, 1, 0, 3)))

    def pko(W, q):
        # [in, out] -> [128, n_in_tiles, out]
        return np.ascontiguousarray(q(
            W.reshape(8, P, W.shape[1]).transpose(1, 0, 2)))

    common = {
        "wq": tile_in_out(Wq_f, 8, 8, _to_bf16),
        "wk": tile_in_out(Wk_f, 8, 8, _to_bf16),
        "wv": pko(Wv_f * (SV if V_FP8 else 1.0),
                  _to_f8 if V_FP8 else _to_bf16),
        "wp": pko(Wp_f * (SP_ if PROJ_FP8 else 1.0),
                  _to_f8 if PROJ_FP8 else _to_bf16),
        "w1": tile_in_out(W1_f * (S1 if FFN_FP8 else 1.0), 8, 32,
                          _to_f8 if FFN_FP8 else _to_bf16),
        "w2": tile_in_out(W2_f * (S2 if FFN_FP8 else 1.0), 32, 8,
                          _to_f8 if FFN_FP8 else _to_bf16),
        "bq": np.ascontiguousarray(bq_f, np.float32),
        "bk": np.ascontiguousarray(bk_f, np.float32),
        "bp": np.ascontiguousarray(bp_f, np.float32),
        "b1": np.ascontiguousarray(b1_f * (S1 if FFN_FP8 else 1.0), np.float32),
        "b2": np.ascontiguousarray(b2_f, np.float32),
        "identr": np.eye(P, dtype=np.float32),
        "identb": np.eye(P, dtype=np.float32).astype(ml_dtypes.bfloat16),
        "onesb": np.ones((P, 16, 16, 1), np.float32).astype(ml_dtypes.bfloat16),
        "onesr": np.ones((P, 1), np.float32),
    }

    in_maps = []
    row_maps = []
    for core in range(8):
        b, s = core // 2, core % 2
        gmap = _chunk_map(s)
        rows = np.concatenate([np.arange(G * P, (G + 1) * P) for G in gmap])
        row_maps.append((b, rows))
        # mask[i, k, slot*128+q]: key pos 256i+128*slot+k vs query pos 128*G+q
        mask = np.empty((8, P, 256), np.float32)
        for i, G in enumerate(gmap):
            kpos = (256 * i + np.arange(256)).reshape(2, P).transpose(1, 0)
            qpos = G * P + np.arange(P)
            m = np.where(kpos[:, :, None] <= qpos[None, None, :], 0.0, NEG)
            mask[i] = m.reshape(P, 2, P).transpose(0, 1, 2).reshape(P, 256)
        in_maps.append({
            "xkv": np.ascontiguousarray(x[b]),
            "xq": np.ascontiguousarray(x[b][rows]),
            "maskc": mask.astype(ml_dtypes.bfloat16),
            **common,
        })
    return in_maps, row_maps


def kernel(**inputs):
    nc = _get_program()
    in_maps, row_maps = _host_prep(**inputs)
    res = run_bass_kernel_spmd(nc, in_maps, core_ids=list(range(8)))
    out = np.empty((B, T, C), np.float32)
    for core in range(8):
        b, rows = row_maps[core]
        out[b][rows] = res.results[core]["y"].T
    return out


# revision 95
# speedup vs baseline: 2.4939x; 1.0541x over previous
"""Trainium2 Bass kernel for one dense transformer block.

Full (unsharded) IO: x [4, 2048, 1024] -> out [4, 2048, 1024].
Sharding: 8 cores = 4 batches x 2 query-chunk sets. Each core owns one
batch's K/V (2048 rows) and 1024 query rows chosen as causally-balanced
128-row chunks (set A: global chunks {4j, 4j+3}, set B: {4j+1, 4j+2}).
Local chunk slot i attends to exactly 2*(i+1) key tiles on every core, so
the instruction stream is identical across cores (SPMD); a per-core data
mask handles the causal boundary in the last two key tiles. No collectives.

On-core dataflow is feature-on-partition ("transposed") throughout. Matmul
dtypes: Q/K projections and scores in bf16 (f32 psum), attention
probabilities bf16, V / output-projection / FFN in fp8-e4m3 with DoubleRow
perf mode (two 128-deep k-tiles per instruction) and power-of-two weight
scaling to avoid fp8 subnormals. LayerNorm runs in f32. Residuals in f32.
LN affine params and biases are folded host-side (exact); V bias is folded
into the projection bias (exact).
"""

import sys

sys.path.insert(0, "/opt/trn_rl_repo")

import numpy as np
import ml_dtypes

import concourse.bass as bass
import concourse.mybir as mybir
import concourse.tile as tile
from concourse.bass_utils import run_bass_kernel_spmd

f32 = mybir.dt.float32
f32r = mybir.dt.float32r
bf16 = mybir.dt.bfloat16
f8 = mybir.dt.float8e4
AL = mybir.AluOpType
AF = mybir.ActivationFunctionType
DR = mybir.MatmulPerfMode.DoubleRow

B, T, C = 4, 2048, 1024
H, D = 16, 64
F = 4 * C
P = 128
TQ = 1024
NEG = -30000.0
LN_EPS = 1e-5

# dtype strategy flags (validated against the 2e-2 rel-err budget)
V_FP8 = True      # V projection via fp8 DoubleRow
PROJ_FP8 = True   # output projection via fp8 DoubleRow
FFN_FP8 = True    # both FFN matmuls via fp8 DoubleRow
SV = 16.0         # host scale on Wv before fp8 cast (carried into OT)
SP_ = 16.0        # host scale on Wproj
S1 = 16.0         # host scale on W1
S2 = 64.0         # host scale on W2

# local chunk slot -> number of key tiles computed (set-independent)
NKT = [2 * (i + 1) for i in range(8)]


def _chunk_map(s):
    """Global 128-row chunk indices owned by set s, in slot order."""
    if s == 0:
        out = []
        for j in range(4):
            out += [4 * j, 4 * j + 3]
        return sorted(out)
    out = []
    for j in range(4):
        out += [4 * j + 1, 4 * j + 2]
    return sorted(out)


def _split_sync_waits(nc):
    """This container's walrus supports one sync-wait per instruction; Tile
    emits up to ~3. Hoist extras onto NoOps inserted before the owner."""
    ctr = 0
    for fn in nc.m.functions:
        for bb in fn.blocks:
            out, changed = [], False
            for ins in bb.instructions:
                si = ins.sync_info
                waits = list(si.on_wait) if si is not None and si.on_wait else []
                if len(waits) > 1:
                    changed = True
                    for w in waits[:-1]:
                        ctr += 1
                        nop = mybir.InstNoOp(name=f"waitsplit_{ctr}", ins=[], outs=[])
                        nop.engine = ins.engine
                        nop.sync_info = mybir.SyncInfo(on_wait=[w], on_update=[])
                        out.append(nop)
                        nc.register_instruction(nop, overwrite=True)
                    ins.sync_info = mybir.SyncInfo(
                        on_wait=[waits[-1]], on_update=list(si.on_update or [])
                    )
                out.append(ins)
            if changed:
                bb.instructions = out


def build_program():
    nc = bass.Bass()
    xkv_d = nc.dram_tensor("xkv", [T, C], f32r, kind="ExternalInput")
    xq_d = nc.dram_tensor("xq", [TQ, C], f32r, kind="ExternalInput")
    mask_d = nc.dram_tensor("maskc", [8, P, 256], bf16, kind="ExternalInput")
    wq_d = nc.dram_tensor("wq", [8, P, 8, P], bf16, kind="ExternalInput")
    wk_d = nc.dram_tensor("wk", [8, P, 8, P], bf16, kind="ExternalInput")
    wv_d = nc.dram_tensor("wv", [P, 8, C], f8 if V_FP8 else bf16,
                          kind="ExternalInput")
    wp_d = nc.dram_tensor("wp", [P, 8, C], f8 if PROJ_FP8 else bf16,
                          kind="ExternalInput")
    w1_d = nc.dram_tensor("w1", [32, P, 8, P], f8 if FFN_FP8 else bf16,
                          kind="ExternalInput")
    w2_d = nc.dram_tensor("w2", [8, P, 32, P], f8 if FFN_FP8 else bf16,
                          kind="ExternalInput")
    bq_d = nc.dram_tensor("bq", [C], f32, kind="ExternalInput")
    bk_d = nc.dram_tensor("bk", [C], f32, kind="ExternalInput")
    bp_d = nc.dram_tensor("bp", [C], f32, kind="ExternalInput")
    b1_d = nc.dram_tensor("b1", [F], f32, kind="ExternalInput")
    b2_d = nc.dram_tensor("b2", [C], f32, kind="ExternalInput")
    identr_d = nc.dram_tensor("identr", [P, P], f32r, kind="ExternalInput")
    identb_d = nc.dram_tensor("identb", [P, P], bf16, kind="ExternalInput")
    onesb_d = nc.dram_tensor("onesb", [P, 16, 16, 1], bf16, kind="ExternalInput")
    onesr_d = nc.dram_tensor("onesr", [P, 1], f32r, kind="ExternalInput")
    y_d = nc.dram_tensor("y", [C, TQ], f32, kind="ExternalOutput")

    fv = f8 if V_FP8 else bf16
    fp = f8 if PROJ_FP8 else bf16
    ff_ = f8 if FFN_FP8 else bf16

    with tile.TileContext(nc) as tc:
      with tc.tile_pool(name="consts", bufs=1) as cpool, \
           tc.tile_pool(name="persist", bufs=1) as pers:
        # only identb (first LN transposes) loads before the x rows; the rest
        # of the constants are deferred into the LN stream so they don't
        # serialize in front of the first x-row DMA
        identb = cpool.tile([P, P], bf16)
        nc.sync.dma_start(identb, identb_d[:, :])
        identr = cpool.tile([P, P], f32r)
        onesbt = cpool.tile([P, 16, 16, 1], bf16)
        onesr = cpool.tile([P, 1], f32r)
        eps128 = cpool.tile([P, 1], f32)
        nc.vector.memset(eps128, LN_EPS)
        eps1 = cpool.tile([1, 1], f32)
        nc.vector.memset(eps1, LN_EPS)
        bq_sb = cpool.tile([P, 8], f32)
        bk_sb = cpool.tile([P, 8], f32)
        bp_sb = cpool.tile([P, 8], f32)
        b1_sb = cpool.tile([P, 32], f32)
        b2_sb = cpool.tile([P, 8], f32)
        mask_sb = cpool.tile([P, 8, 256], bf16)

        def emit_deferred_consts(step):
            # spread across LN tiles so no batch delays the next x-row DMA
            if step == 0:
                nc.sync.dma_start(bq_sb, bq_d.rearrange("(o p) -> p o", p=P))
                nc.sync.dma_start(bk_sb, bk_d.rearrange("(o p) -> p o", p=P))
            elif step == 1:
                nc.sync.dma_start(onesbt, onesb_d[:, :, :, :])
                nc.sync.dma_start(identr, identr_d[:, :])
            elif step == 2:
                for i in range(4):
                    nc.sync.dma_start(mask_sb[:, i, :], mask_d[i])
            elif step == 3:
                for i in range(4, 8):
                    nc.sync.dma_start(mask_sb[:, i, :], mask_d[i])
            elif step == 4:
                nc.sync.dma_start(bp_sb, bp_d.rearrange("(o p) -> p o", p=P))
                nc.sync.dma_start(b1_sb, b1_d.rearrange("(o p) -> p o", p=P))
                nc.sync.dma_start(b2_sb, b2_d.rearrange("(o p) -> p o", p=P))
                nc.sync.dma_start(onesr, onesr_d[:, :])

        OT = pers.tile([P, 8, TQ], fp)        # attn out (x SV), transposed
        otq = pers.tile([P, 8, 16, 64], bf16)  # attn out row-major (q, head, d)
        xqT = pers.tile([P, 8, TQ], f32r)     # raw queries, transposed

        # ---------------- Phase A: LN1 + transposes + V + attention --------
        with tc.tile_pool(name="attn_big", bufs=1) as apool:
            hkvT = apool.tile([P, 8, T], bf16)
            hqT = apool.tile([P, 8, TQ], bf16)
            if V_FP8:
                hkv8 = apool.tile([P, 8, T], f8)
            else:
                hkv8 = None
            vaug = apool.tile([P, 16, 16, 65], bf16)

            cm_wpool = tc.tile_pool(name="kq_w", bufs=2)
            cm_gpool = tc.tile_pool(name="kq_sb", bufs=2)
            cm_kqv = tc.tile_pool(name="ps_kqv", bufs=2, space="PSUM")
            wpool = cm_wpool.__enter__()
            gpool = cm_gpool.__enter__()
            ps_kqv = cm_kqv.__enter__()

            def emit_k(g, wk_t, KT, rcs):
                for rc in rcs:
                    psum = ps_kqv.tile([P, 512], f32, tag="kqv")
                    for k in range(8):
                        nc.tensor.matmul(
                            psum, wk_t[:, k, :],
                            hkvT[:, k, rc * 512:(rc + 1) * 512],
                            start=(k == 0), stop=(k == 7))
                    nc.vector.tensor_scalar_add(
                        KT[:, rc * 512:(rc + 1) * 512], psum,
                        bk_sb[:, g:g + 1])

            def emit_q(g, wq_t, QT, rcs):
                for rc in rcs:
                    psum = ps_kqv.tile([P, 512], f32, tag="kqv")
                    for k in range(8):
                        nc.tensor.matmul(
                            psum, wq_t[:, k, :],
                            hqT[:, k, rc * 512:(rc + 1) * 512],
                            start=(k == 0), stop=(k == 7))
                    nc.vector.tensor_scalar_add(
                        QT[:, rc * 512:(rc + 1) * 512], psum,
                        bq_sb[:, g:g + 1])

            with tc.tile_pool(name="ln_x", bufs=4) as lnx, \
                 tc.tile_pool(name="ln_sb", bufs=2) as lnsb, \
                 tc.tile_pool(name="ln_small", bufs=6) as lnsm, \
                 tc.tile_pool(name="ps_a", bufs=1, space="PSUM") as ps_a, \
                 tc.tile_pool(name="ps_v", bufs=2, space="PSUM") as ps_v:

                def ln_row_tile(src_ap, rt, is_q):
                    xrow = lnx.tile([P, C], f32r, tag="xrow")
                    nc.sync.dma_start(xrow, src_ap)
                    stats = lnsm.tile([P, 2, 6], f32, tag="stats")
                    for sg in range(2):
                        nc.vector.bn_stats(stats[:, sg, :],
                                           xrow[:, sg * 512:(sg + 1) * 512])
                    mv = lnsm.tile([P, 2], f32, tag="mv")
                    nc.vector.bn_aggr(mv, stats)
                    rstd = lnsm.tile([P, 1], f32, tag="rstd")
                    nc.scalar.activation(rstd, mv[:, 1:2], AF.Sqrt,
                                         bias=eps128, scale=1.0)
                    nc.vector.reciprocal(rstd, rstd)
                    hrow = lnsb.tile([P, C], bf16, tag="hrow")
                    nc.gpsimd.tensor_scalar(hrow, xrow, mv[:, 0:1], rstd,
                                            op0=AL.subtract, op1=AL.mult)
                    dstT = hqT if is_q else hkvT
                    # one accumulation "start" per 2KB PSUM bank: later
                    # region-disjoint writes use start=False (pending-zero
                    # bytes are overwritten, not accumulated)
                    pstb = ps_a.tile([P, 8, P], bf16, tag="trb")
                    for ft in range(8):
                        nc.tensor.matmul(pstb[:, ft, :],
                                         hrow[:, ft * P:(ft + 1) * P],
                                         identb, is_transpose=True,
                                         start=(ft == 0), stop=True,
                                         skip_group_check=True)
                    if rt % 2 == 0:
                        nc.vector.tensor_copy(dstT[:, :, rt * P:(rt + 1) * P],
                                              pstb)
                        if not is_q and V_FP8:
                            nc.scalar.copy(hkv8[:, :, rt * P:(rt + 1) * P],
                                           pstb)
                    else:
                        nc.scalar.copy(dstT[:, :, rt * P:(rt + 1) * P], pstb)
                        if not is_q and V_FP8:
                            nc.vector.tensor_copy(
                                hkv8[:, :, rt * P:(rt + 1) * P], pstb)
                    if is_q:
                        pstr = ps_a.tile([P, 8, P], f32r, tag="trr")
                        for ft in range(8):
                            nc.tensor.matmul(pstr[:, ft, :],
                                             xrow[:, ft * P:(ft + 1) * P],
                                             identr, is_transpose=True,
                                             start=(ft % 4 == 0), stop=True,
                                             skip_group_check=True)
                        nc.scalar.copy(xqT[:, :, rt * P:(rt + 1) * P], pstr)

                def emit_v(tc_i):
                    for fh in range(2):
                        psv = ps_v.tile([P, 8, 64], f32, tag="v")
                        if V_FP8:
                            for q in range(2):
                                for kp in range(4):
                                    nc.tensor.matmul(
                                        psv[:, q * 4:(q + 1) * 4, :],
                                        hkv8[:, 2 * kp:2 * kp + 2,
                                             tc_i * P:(tc_i + 1) * P],
                                        wv_sb[:, 2 * kp:2 * kp + 2,
                                              fh * 512 + q * 256:
                                              fh * 512 + (q + 1) * 256],
                                        perf_mode=DR,
                                        start=(q == 0 and kp == 0),
                                        stop=(kp == 3),
                                        skip_group_check=True)
                        else:
                            for k in range(8):
                                nc.tensor.matmul(
                                    psv[:, :, :],
                                    hkvT[:, k, tc_i * P:(tc_i + 1) * P],
                                    wv_sb[:, k, fh * 512:(fh + 1) * 512],
                                    start=(k == 0), stop=(k == 7))
                        if (tc_i + fh) % 2 == 0:
                            nc.vector.tensor_copy(
                                vaug[:, tc_i, fh * 8:(fh + 1) * 8, 0:64], psv)
                        else:
                            nc.scalar.copy(
                                vaug[:, tc_i, fh * 8:(fh + 1) * 8, 0:64], psv)

                # software-pipelined emission: the engine queues are in-order,
                # so interleave V and the first two groups' K/Q projections
                # between LN tiles to keep PE fed during the DVE-bound LN.
                wv_sb = apool.tile([P, 8, C], fv)
                kq_early = []
                for g in (0, 1):
                    wk_t = wpool.tile([P, 8, P], bf16, tag="wk")
                    wq_t = wpool.tile([P, 8, P], bf16, tag="wq")
                    KT = gpool.tile([P, T], bf16, tag="KT")
                    QT = gpool.tile([P, TQ], bf16, tag="QT")
                    kq_early.append((wk_t, wq_t, KT, QT))
                for rt in range(16):
                    ln_row_tile(xkv_d[rt * P:(rt + 1) * P, :], rt, False)
                    if rt == 0:
                        nc.sync.dma_start(wv_sb, wv_d[:, :, :])
                        emit_deferred_consts(0)
                    elif rt == 1:
                        nc.sync.dma_start(kq_early[0][0], wk_d[0])
                        nc.sync.dma_start(kq_early[0][1], wq_d[0])
                    elif rt == 2:
                        nc.sync.dma_start(kq_early[1][0], wk_d[1])
                        nc.sync.dma_start(kq_early[1][1], wq_d[1])
                    if 2 <= rt <= 5:
                        emit_deferred_consts(rt - 1)
                        if rt == 2:
                            nc.vector.tensor_copy(vaug[:, :, :, 64:65],
                                                  onesbt)
                    if rt >= 2:
                        emit_v(rt - 2)
                    if rt % 4 == 3:
                        emit_k(0, kq_early[0][0], kq_early[0][2], [rt // 4])
                for rt in range(8):
                    ln_row_tile(xq_d[rt * P:(rt + 1) * P, :], rt, True)
                    if rt < 2:
                        emit_v(14 + rt)
                    if rt % 2 == 1:
                        emit_k(1, kq_early[1][0], kq_early[1][2], [rt // 2])
                emit_q(0, kq_early[0][1], kq_early[0][3], [0, 1])
                emit_q(1, kq_early[1][1], kq_early[1][3], [0, 1])

            # -------- per-group K/Q projection + attention --------
            with tc.tile_pool(name="pt_sb", bufs=2) as ptpool, \
                 tc.tile_pool(name="sm_sb", bufs=3) as smpool, \
                 tc.tile_pool(name="ps_sc", bufs=2, space="PSUM") as ps_sc, \
                 tc.tile_pool(name="ps_ob", bufs=2, space="PSUM") as ps_ob:
                for g in range(8):
                    if g < 2:
                        _, _, KT, QT = kq_early[g]
                    else:
                        wk_t = wpool.tile([P, 8, P], bf16, tag="wk")
                        nc.sync.dma_start(wk_t, wk_d[g])
                        wq_t = wpool.tile([P, 8, P], bf16, tag="wq")
                        nc.sync.dma_start(wq_t, wq_d[g])
                        KT = gpool.tile([P, T], bf16, tag="KT")
                        emit_k(g, wk_t, KT, range(4))
                        QT = gpool.tile([P, TQ], bf16, tag="QT")
                        emit_q(g, wq_t, QT, range(2))

                    for hh in range(2):
                        base = 64 * hh
                        for lc in range(8):
                            nkt = NKT[lc]
                            ptb = ptpool.tile([P, 16, P], bf16, tag="ptb")
                            ob = ps_ob.tile([P, 65], f32, tag="ob")
                            for sg in range((nkt + 7) // 8):
                                n_in_g = min(8, nkt - sg * 8)
                                ps_s = ps_sc.tile([P, 8, P], f32, tag="sc")
                                for sl in range(n_in_g):
                                    kt = sg * 8 + sl
                                    last2 = (kt >= nkt - 2)
                                    nc.tensor.matmul(
                                        ps_s[:, sl, :],
                                        KT[base:base + 64, kt * P:(kt + 1) * P],
                                        QT[base:base + 64, lc * P:(lc + 1) * P],
                                        start=(sl % 4 == 0), stop=not last2,
                                        skip_group_check=True)
                                # causal mask on the last two key tiles
                                sl0 = (nkt - 2) % 8
                                if sg == (nkt - 1) // 8:
                                    nc.tensor.matmul(
                                        ps_s[:, sl0:sl0 + 2, :],
                                        identb, mask_sb[:, lc, :],
                                        start=False, stop=True,
                                        skip_group_check=True)
                                nc.scalar.activation(
                                    ptb[:, sg * 8:sg * 8 + n_in_g, :],
                                    ps_s[:, 0:n_in_g, :], AF.Exp)
                            h_idx = 2 * g + hh
                            for kt in range(nkt):
                                # out[q, d] = sum_k pt[k, q] * vaug[k, d];
                                # column 64 accumulates the softmax denom
                                nc.tensor.matmul(
                                    ob, ptb[:, kt, :],
                                    vaug[:, kt, h_idx, :],
                                    start=(kt == 0), stop=(kt == nkt - 1))
                            rec = smpool.tile([P, 1], f32, tag="rec")
                            nc.vector.reciprocal(rec, ob[:, 64:65])
                            nc.vector.tensor_scalar_mul(
                                otq[:, lc, h_idx, :], ob[:, 0:64], rec)

            cm_kqv.__exit__(None, None, None)
            cm_gpool.__exit__(None, None, None)
            cm_wpool.__exit__(None, None, None)

        # transpose attn out back to feature-on-partition for proj
        with tc.tile_pool(name="ps_trq", bufs=2, space="PSUM") as ps_trq:
            for lc in range(8):
                pq = ps_trq.tile([P, 8, P], bf16, tag="trq")
                for ft in range(8):
                    nc.tensor.matmul(
                        pq[:, ft, :], otq[:, lc, 2 * ft:2 * ft + 2, :],
                        identb, is_transpose=True, start=(ft == 0), stop=True,
                        skip_group_check=True)
                if lc % 2 == 0:
                    nc.vector.tensor_copy(OT[:, :, lc * P:(lc + 1) * P], pq)
                else:
                    nc.scalar.copy(OT[:, :, lc * P:(lc + 1) * P], pq)

        # ---------------- Phase B: proj + residual + LN2 + FFN -------------
        with tc.tile_pool(name="late", bufs=1) as late:
            x2T = late.tile([P, 8, TQ], f32r)
            h2 = late.tile([P, 8, TQ], ff_)
            # proj and LN2 share a scope: each LN2 half's stats are emitted
            # as soon as its projection columns land, so only the short
            # small-op chain remains after the projection finishes
            with tc.tile_pool(name="pr_w", bufs=1) as prpool, \
                 tc.tile_pool(name="pr_tmp", bufs=3) as prtmp, \
                 tc.tile_pool(name="ln2_sb", bufs=4) as ln2sb, \
                 tc.tile_pool(name="ln2_sm", bufs=4) as ln2sm, \
                 tc.tile_pool(name="ps_pr", bufs=2, space="PSUM") as ps_pr, \
                 tc.tile_pool(name="ps_l2s", bufs=2, space="PSUM") as ps_l2s, \
                 tc.tile_pool(name="ps_l2b", bufs=1, space="PSUM") as ps_l2b:
                wp_sb = prpool.tile([P, 8, C], fp)
                nc.sync.dma_start(wp_sb, wp_d[:, :, :])
                # OT carries SV from the V-weight scaling; wp carries SP_
                inv = 1.0 / ((SV if V_FP8 else 1.0) * (SP_ if PROJ_FP8 else 1.0))
                l2stats = []

                def emit_ln2_stats(h):
                    psum_m = ps_l2s.tile([1, 512], f32, tag="m")
                    psum_q = ps_l2s.tile([1, 512], f32, tag="q")
                    for ft in range(8):
                        xt = x2T[:, ft, h * 512:(h + 1) * 512]
                        sq = ln2sb.tile([P, 512], f32r, tag="sq")
                        nc.vector.tensor_tensor(sq, xt, xt, AL.mult)
                        nc.tensor.matmul(psum_m, onesr, xt,
                                         start=(ft == 0), stop=(ft == 7))
                        nc.tensor.matmul(psum_q, onesr, sq,
                                         start=(ft == 0), stop=(ft == 7))
                    l2stats.append((psum_m, psum_q))

                def emit_ln2_finish(h):
                    psum_m, psum_q = l2stats[h]
                    mean = ln2sm.tile([1, 512], f32, tag="mean")
                    nc.vector.tensor_scalar_mul(mean, psum_m, 1.0 / C)
                    msq = ln2sm.tile([1, 512], f32, tag="msq")
                    nc.vector.tensor_scalar_mul(msq, psum_q, 1.0 / C)
                    var = ln2sm.tile([1, 512], f32, tag="var")
                    nc.vector.tensor_tensor(var, mean, mean, AL.mult)
                    nc.vector.tensor_tensor(var, msq, var, AL.subtract)
                    std = ln2sm.tile([1, 512], f32, tag="std")
                    nc.scalar.activation(std, var, AF.Sqrt,
                                         bias=eps1[:, :], scale=1.0)
                    a = ln2sm.tile([1, 512], f32r, tag="a")
                    with nc.allow_low_precision(reason="f32r has f32 bits"):
                        nc.vector.reciprocal(a, std)
                    bneg = ln2sm.tile([1, 512], f32r, tag="b")
                    nc.vector.tensor_tensor(bneg, mean, a, AL.mult)
                    ps_abc = ps_l2b.tile([P, 512], f32, tag="abc")
                    nc.tensor.matmul(ps_abc,
                                     onesr[0:1, 0:1].broadcast_to((1, P)), a,
                                     start=True, stop=True)
                    ps_bbc = ps_l2b.tile([P, 512], f32, tag="bbc")
                    nc.tensor.matmul(ps_bbc,
                                     onesr[0:1, 0:1].broadcast_to((1, P)),
                                     bneg, start=True, stop=True)
                    abc_sb = ln2sm.tile([P, 512], f32r, tag="abcs")
                    nc.scalar.copy(abc_sb, ps_abc)
                    bbc_sb = ln2sm.tile([P, 512], f32r, tag="bbcs")
                    nc.scalar.copy(bbc_sb, ps_bbc)
                    for ft in range(8):
                        tmp = ln2sb.tile([P, 512], f32r, tag="tmp")
                        if ft % 2 == 0:
                            nc.vector.tensor_tensor(
                                tmp, x2T[:, ft, h * 512:(h + 1) * 512],
                                ps_abc, AL.mult)
                            nc.vector.tensor_tensor(
                                h2[:, ft, h * 512:(h + 1) * 512], tmp,
                                ps_bbc, AL.subtract)
                        else:
                            nc.gpsimd.tensor_tensor(
                                tmp, x2T[:, ft, h * 512:(h + 1) * 512],
                                abc_sb, AL.mult)
                            nc.gpsimd.tensor_tensor(
                                h2[:, ft, h * 512:(h + 1) * 512], tmp,
                                bbc_sb, AL.subtract)

                for rc in range(4):
                    for of in range(8):
                        psum = ps_pr.tile([P, 256], f32, tag="pr")
                        if PROJ_FP8:
                            for kp in range(4):
                                nc.tensor.matmul(
                                    psum,
                                    wp_sb[:, 2 * kp:2 * kp + 2,
                                          of * P:(of + 1) * P],
                                    OT[:, 2 * kp:2 * kp + 2,
                                       rc * 256:(rc + 1) * 256],
                                    perf_mode=DR,
                                    start=(kp == 0), stop=(kp == 3))
                        else:
                            for k in range(8):
                                nc.tensor.matmul(
                                    psum, wp_sb[:, k, of * P:(of + 1) * P],
                                    OT[:, k, rc * 256:(rc + 1) * 256],
                                    start=(k == 0), stop=(k == 7))
                        t1 = prtmp.tile([P, 256], f32, tag="t1")
                        nc.scalar.activation(t1, psum, AF.Identity,
                                             bias=bp_sb[:, of:of + 1],
                                             scale=inv)
                        nc.gpsimd.tensor_tensor(
                            x2T[:, of, rc * 256:(rc + 1) * 256], t1,
                            xqT[:, of, rc * 256:(rc + 1) * 256], AL.add)
                    if rc % 2 == 1:
                        emit_ln2_stats(rc // 2)
                for h in range(2):
                    emit_ln2_finish(h)


            # FFN
            with tc.tile_pool(name="f_w1", bufs=6) as w1pool, \
                 tc.tile_pool(name="f_w2", bufs=3) as w2pool, \
                 tc.tile_pool(name="f_relu", bufs=1) as rpool, \
                 tc.tile_pool(name="f_y", bufs=3) as ypool, \
                 tc.tile_pool(name="ps_f1", bufs=4, space="PSUM") as ps_f1, \
                 tc.tile_pool(name="ps_f2", bufs=4, space="PSUM") as ps_f2:
                relu1 = rpool.tile([P, 32, TQ], ff_)
                for fk in range(32):
                    w1_t = w1pool.tile([P, 8, P], ff_, tag="w1")
                    nc.sync.dma_start(w1_t, w1_d[fk])
                    for rc in range(2):
                        psum = ps_f1.tile([P, 512], f32, tag="f1")
                        if FFN_FP8:
                            for q in range(2):
                                for kp in range(4):
                                    nc.tensor.matmul(
                                        psum[:, q * 256:(q + 1) * 256],
                                        w1_t[:, 2 * kp:2 * kp + 2, :],
                                        h2[:, 2 * kp:2 * kp + 2,
                                           rc * 512 + q * 256:
                                           rc * 512 + (q + 1) * 256],
                                        perf_mode=DR,
                                        start=(q == 0 and kp == 0),
                                        stop=(kp == 3),
                                        skip_group_check=True)
                        else:
                            for k in range(8):
                                nc.tensor.matmul(
                                    psum, w1_t[:, k, :],
                                    h2[:, k, rc * 512:(rc + 1) * 512],
                                    start=(k == 0), stop=(k == 7))
                        out_sl = relu1[:, fk, rc * 512:(rc + 1) * 512]
                        if fk % 2 == 0:
                            nc.scalar.activation(out_sl, psum, AF.Relu,
                                                 bias=b1_sb[:, fk:fk + 1],
                                                 scale=1.0)
                        else:
                            nc.vector.tensor_scalar(out_sl, psum,
                                                    b1_sb[:, fk:fk + 1], 0.0,
                                                    op0=AL.add, op1=AL.max)
                for of in range(8):
                    w2_t = w2pool.tile([P, 32, P], ff_, tag="w2")
                    nc.sync.dma_start(w2_t, w2_d[of])
                    for rc in range(2):
                        psum = ps_f2.tile([P, 512], f32, tag="f2")
                        if FFN_FP8:
                            for q in range(2):
                                for kp in range(16):
                                    nc.tensor.matmul(
                                        psum[:, q * 256:(q + 1) * 256],
                                        w2_t[:, 2 * kp:2 * kp + 2, :],
                                        relu1[:, 2 * kp:2 * kp + 2,
                                              rc * 512 + q * 256:
                                              rc * 512 + (q + 1) * 256],
                                        perf_mode=DR,
                                        start=(q == 0 and kp == 0),
                                        stop=(kp == 15),
                                        skip_group_check=True)
                        else:
                            for fk in range(32):
                                nc.tensor.matmul(
                                    psum, w2_t[:, fk, :],
                                    relu1[:, fk, rc * 512:(rc + 1) * 512],
                                    start=(fk == 0), stop=(fk == 31))
                        t2 = ypool.tile([P, 512], f32, tag="t2")
                        inv2 = 1.0 / (S1 * S2) if FFN_FP8 else 1.0
                        nc.scalar.activation(t2, psum, AF.Identity,
                                             bias=b2_sb[:, of:of + 1],
                                             scale=inv2)
                        y_sb = ypool.tile([P, 512], f32, tag="y")
                        nc.gpsimd.tensor_tensor(
                            y_sb, t2, x2T[:, of, rc * 512:(rc + 1) * 512],
                            AL.add)
                        nc.sync.dma_start(
                            y_d[of * P:(of + 1) * P,
                                rc * 512:(rc + 1) * 512], y_sb)
    _split_sync_waits(nc)
    return nc


_PROGRAM = None


def _get_program():
    global _PROGRAM
    if _PROGRAM is None:
        _PROGRAM = build_program()
    return _PROGRAM


def _to_bf16(a):
    return np.asarray(a, np.float32).astype(ml_dtypes.bfloat16)


def _to_f8(a):
    return np.asarray(a, np.float32).astype(ml_dtypes.float8_e4m3)


def _host_prep(x, Wk, Wq, Wv, Wproj, bproj, W1, b1, W2, b2, g1, beta1, g2, beta2):
    """Fold LN affine params / V bias into weights (exact); quantize; shard."""
    x = np.asarray(x, np.float32)
    scale = 1.0 / np.sqrt(D)
    Wq_f = (g1[:, None] * np.asarray(Wq, np.float32)) * scale
    bq_f = (beta1 @ np.asarray(Wq, np.float32)) * scale
    Wk_f = g1[:, None] * np.asarray(Wk, np.float32)
    bk_f = beta1 @ np.asarray(Wk, np.float32)
    Wv_f = g1[:, None] * np.asarray(Wv, np.float32)
    bv_f = beta1 @ np.asarray(Wv, np.float32)
    Wp_f = np.asarray(Wproj, np.float32)
    bp_f = np.asarray(bproj, np.float32) + bv_f @ Wp_f
    W1_f = g2[:, None] * np.asarray(W1, np.float32)
    b1_f = np.asarray(b1, np.float32) + beta2 @ np.asarray(W1, np.float32)
    W2_f = np.asarray(W2, np.float32)
    b2_f = np.asarray(b2, np.float32)

    def tile_in_out(W, n_in, n_out, q):
        # [in, out] -> [n_out, 128, n_in, 128]
        return np.ascontiguousarray(q(
            W.reshape(n_in, P, n_out, P).transpose(2, 1, 0, 3)))

    def pko(W, q):
        # [in, out] -> [128, n_in_tiles, out]
        return np.ascontiguousarray(q(
            W.reshape(8, P, W.shape[1]).transpose(1, 0, 2)))

    common = {
        "wq": tile_in_out(Wq_f, 8, 8, _to_bf16),
        "wk": tile_in_out(Wk_f, 8, 8, _to_bf16),
        "wv": pko(Wv_f * (SV if V_FP8 else 1.0),
                  _to_f8 if V_FP8 else _to_bf16),
        "wp": pko(Wp_f * (SP_ if PROJ_FP8 else 1.0),
                  _to_f8 if PROJ_FP8 else _to_bf16),
        "w1": tile_in_out(W1_f * (S1 if FFN_FP8 else 1.0), 8, 32,
                          _to_f8 if FFN_FP8 else _to_bf16),
        "w2": tile_in_out(W2_f * (S2 if FFN_FP8 else 1.0), 32, 8,
                          _to_f8 if FFN_FP8 else _to_bf16),
        "bq": np.ascontiguousarray(bq_f, np.float32),
        "bk": np.ascontiguousarray(bk_f, np.float32),
        "bp": np.ascontiguousarray(bp_f, np.float32),
        "b1": np.ascontiguousarray(b1_f * (S1 if FFN_FP8 else 1.0), np.float32),
        "b2": np.ascontiguousarray(b2_f, np.float32),
        "identr": np.eye(P, dtype=np.float32),
        "identb": np.eye(P, dtype=np.float32).astype(ml_dtypes.bfloat16),
        "onesb": np.ones((P, 16, 16, 1), np.float32).astype(ml_dtypes.bfloat16),
        "onesr": np.ones((P, 1), np.float32),
    }

    in_maps = []
    row_maps = []
    for core in range(8):
        b, s = core // 2, core % 2
        gmap = _chunk_map(s)
        rows = np.concatenate([np.arange(G * P, (G + 1) * P) for G in gmap])
        row_maps.append((b, rows))
        # mask[i, k, slot*128+q]: key pos 256i+128*slot+k vs query pos 128*G+q
        mask = np.empty((8, P, 256), np.float32)
        for i, G in enumerate(gmap):
            kpos = (256 * i + np.arange(256)).reshape(2, P).transpose(1, 0)
            qpos = G * P + np.arange(P)
            m = np.where(kpos[:, :, None] <= qpos[None, None, :], 0.0, NEG)
            mask[i] = m.reshape(P, 2, P).transpose(0, 1, 2).reshape(P, 256)
        in_maps.append({
            "xkv": np.ascontiguousarray(x[b]),
            "xq": np.ascontiguousarray(x[b][rows]),
            "maskc": mask.astype(ml_dtypes.bfloat16),
            **common,
        })
    return in_maps, row_maps


def kernel(**inputs):
    nc = _get_program()
    in_maps, row_maps = _host_prep(**inputs)
    res = run_bass_kernel_spmd(nc, in_maps, core_ids=list(range(8)))
    out = np.empty((B, T, C), np.float32)
    for core in range(8):
        b, rows = row_maps[core]
        out[b][rows] = res.results[core]["y"].T
    return out
